# revision 14
# baseline (speedup 1.0000x reference)
"""Trainium2 Bass kernel for nn_CAM (GNN message passing, 8-core SPMD).

Strategy (per core i of 8, owning node rows R_i = [1024*i, 1024*(i+1))):
  - Host ships the TRANSPOSED column-block of each adjacency as
    fp8_e4m3 pre-scaled by meta*2^13 (feature) and (1-meta)*2^13
    (spatial).  The blend  con = meta*A_f + (1-meta)*A_s  then reduces
    to a pure ADD, which rides the DMA engines' inline CCE ALU:
    the spatial slab DMAs land in the resident conT8 tile and the
    feature slab DMAs accumulate into it (gpsimd SWDGE, accum_op=add).
    No vector-engine blend pass at all; the 2^-13 fold-back is a
    compile-time ACT scale.
  - All three adj@support rounds run as fp8 DoubleRow matmuls in the
    transposed [h, 1024] domain against the resident conT8.
  - Support matrices are exchanged across cores via AllGather bounced
    through shared DRAM.  s1 goes as FOUR quarter-AGs (Mesh algorithm,
    lower latency than RDH halves) consumed by four round-1 waves; s2
    and s3 as two half-AGs each.  The final wave of each round is
    ordered h0-before-h1 so the next support chain and its first AG
    trigger fire at the ~75% point of the round.
  - Discarded warmer matmuls bridge every collective wait so the PE's
    HAM clock gate stays released (cold PE runs at 1.2 GHz vs 2.4).
  - The attention fusion runs in the transposed [64, 1024] domain;
    com-independent views compute inside the AG-s2 window and the
    com-dependent tail is pipelined in two 512-column halves so the
    output DMA starts as soon as the first half of round 3 closes.
"""

import sys

if "/opt/trn_rl_repo" not in sys.path:
    sys.path.insert(0, "/opt/trn_rl_repo")

from contextlib import ExitStack

import numpy as np

import concourse.bass as bass
import concourse.tile as tile
from concourse import bacc, mybir
from concourse.bass_utils import run_bass_kernel_spmd
from concourse.masks import make_identity


F32 = mybir.dt.float32
BF16 = mybir.dt.bfloat16
F8 = mybir.dt.float8e4
AF = mybir.ActivationFunctionType
DR = mybir.MatmulPerfMode.DoubleRow
MUL = mybir.AluOpType.mult
ADD = mybir.AluOpType.add

N = 8192
D_IN = 3000
H1, H2, Z = 256, 128, 64
N_CORES = 8
NL = N // N_CORES           # 1024 local nodes per core
KC = N // 128               # 64 contraction chunks of 128
XC = (D_IN + 127) // 128    # 24 x-feature chunks (last partial: 56)
XB = 4                      # x chunks per batched DMA
RG = [list(range(N_CORES))]
INV13 = 1.0 / 8192.0        # 2^-13 fold-back for the fp8 exponent shift

USE_ACCUM_BLEND = True      # blend via DMA CCE add (else DVE/Pool adds)
NSLAB = 8                   # adjacency stream slabs (KC/NSLAB chunks each)
SCH = KC // NSLAB


def _emit(nc, tc, io):
    adjT_f, adjT_s, xT = io["adjT_f"], io["adjT_s"], io["xT"]
    zfT, zsT = io["zfT"], io["zsT"]
    W1, W2, W3 = io["W1"], io["W2"], io["W3"]
    wl_W, mlp_W = io["wl_W"], io["mlp_W"]
    wl_b, mlp_b = io["wl_b"], io["mlp_b"]
    outT = io["outT"]

    ctx = ExitStack()
    with ctx:
        const = ctx.enter_context(tc.tile_pool(name="const", bufs=1))
        dram = ctx.enter_context(tc.tile_pool(name="dram", bufs=1, space="DRAM"))

        ident_bf = const.tile([128, 128], BF16)
        make_identity(nc, ident_bf)
        wl_b_sb = const.tile([64, 1], F32)
        nc.gpsimd.dma_start(out=wl_b_sb, in_=wl_b[:, None])
        mlp_b_sb = const.tile([64, 1], F32)
        nc.gpsimd.dma_start(out=mlp_b_sb, in_=mlp_b[:, None])
        W2_sb = const.tile([128, 2, H2], BF16)
        nc.gpsimd.dma_start(out=W2_sb, in_=W2.rearrange("(b k) c -> k b c", b=2))
        W3_sb = const.tile([128, Z], BF16)
        nc.gpsimd.dma_start(out=W3_sb, in_=W3[:, :])

        # resident blended adjacency (transposed, fp8, x2^13)
        conT8 = const.tile([128, KC, NL], F8)
        z1sb = const.tile([128, 2, NL], BF16)

        # attention operands live across phases -> const pool
        com_bf = const.tile([64, NL], BF16)
        zf_bf = const.tile([64, NL], BF16)
        zs_bf = const.tile([64, NL], BF16)
        wlW_sb = const.tile([64, 64], BF16)
        mlpW_sb = const.tile([64, 3, 64], BF16)

        # AG bounce buffers (fp8); s1 as four quarters, s2/s3 as halves
        s1_ins = [dram.tile([128, 2, H1], F8, name=f"s1in{t}") for t in range(4)]
        s1_outs = [dram.tile([NL, 2, H1], F8, addr_space="Shared",
                             name=f"s1out{t}") for t in range(4)]
        s2_ins = [dram.tile([128, 4, H2], F8, name=f"s2in{t}") for t in range(2)]
        s2_outs = [dram.tile([NL, 4, H2], F8, addr_space="Shared",
                             name=f"s2out{t}") for t in range(2)]
        s3_ins = [dram.tile([128, 4, Z], F8, name=f"s3in{t}") for t in range(2)]
        s3_outs = [dram.tile([NL, 4, Z], F8, addr_space="Shared",
                             name=f"s3out{t}") for t in range(2)]

        # ======== phase A: stream+blend, s1 chain, round 1 ============
        with tc.tile_pool(name="phaseA", bufs=1) as pA, \
             tc.tile_pool(name="psZ", bufs=1, space="PSUM") as psZ:
            s1T_bf = pA.tile([128, 2, NL], BF16)
            s1loc = pA.tile([128, 8, H1], F8)
            s1f = pA.tile([128, 8, 8, H1], F8)

            psA_ctx = ExitStack()
            psA = psA_ctx.enter_context(
                tc.tile_pool(name="psA", bufs=1, space="PSUM")
            )
            s1T_ps = [psA.tile([128, NL], F32, name=f"s1T{g}") for g in range(2)]

            # ---- batched fp8 x/W1 loads: first on both HWDGE rings ----
            xbigs, w1bigs = [], []
            for b in range(6):
                eng = nc.sync if b < 3 else nc.scalar
                r0 = b * XB * 128
                na = 3 if b == 5 else XB  # chunk 23 is partial (56 rows)
                xb = pA.tile([128, XB, NL], F8, name="xbig", bufs=6)
                eng.dma_start(
                    out=xb[:, :na, :],
                    in_=xT[r0 : r0 + na * 128, :].rearrange(
                        "(a p) m -> p a m", p=128),
                )
                xbigs.append(xb)
                wb = pA.tile([128, XB, H1], F8, name="w1big", bufs=6)
                eng.dma_start(
                    out=wb[:, :na, :],
                    in_=W1[r0 : r0 + na * 128, :].rearrange(
                        "(a p) m -> p a m", p=128),
                )
                w1bigs.append(wb)
            xsm = pA.tile([128, NL], F8)
            nc.scalar.dma_start(out=xsm[:56], in_=xT[2944:3000, :])
            w1sm = pA.tile([128, H1], F8)
            nc.scalar.dma_start(out=w1sm[:56], in_=W1[2944:3000, :])

            # ---- adjacency stream + blend-on-DMA ----
            # spatial slabs (pre-scaled by (1-meta)*2^13) land straight in
            # the resident conT8; feature slabs (pre-scaled by meta*2^13)
            # accumulate into it via the SDMA CCE adder.  The vector
            # engine never touches the 8M-element blend.
            for j in range(NSLAB):
                r0, r1 = j * SCH * 128, (j + 1) * SCH * 128
                dst = conT8[:, j * SCH : (j + 1) * SCH, :]
                nc.sync.dma_start(
                    out=dst,
                    in_=adjT_s[r0:r1, :].rearrange("(a p) m -> p a m", p=128),
                )
                if USE_ACCUM_BLEND:
                    nc.gpsimd.dma_start(
                        out=dst,
                        in_=adjT_f[r0:r1, :].rearrange("(a p) m -> p a m", p=128),
                        accum_op=ADD,
                    )
                else:
                    af = pA.tile([128, SCH, NL], F8, name="af", bufs=4)
                    nc.scalar.dma_start(
                        out=af,
                        in_=adjT_f[r0:r1, :].rearrange("(a p) m -> p a m", p=128),
                    )
                    eng = nc.vector if j % 2 == 0 else nc.gpsimd
                    eng.tensor_add(dst, dst, af)

            # attention inputs (small; gpsimd SWDGE casts f32->bf16)
            nc.gpsimd.dma_start(out=zf_bf, in_=zfT[:, :])
            nc.gpsimd.dma_start(out=zs_bf, in_=zsT[:, :])
            nc.gpsimd.dma_start(out=wlW_sb, in_=wl_W[:, :])
            nc.gpsimd.dma_start(
                out=mlpW_sb, in_=mlp_W.rearrange("(v c) d -> c v d", v=3)
            )

            # ---- s1 = tanh(xT.T @ W1) in the transposed domain ----
            # early identity matmuls release the HAM clock gate before
            # the real s1 stream arrives; they scribble on s1T_ps, which
            # the first real accumulation (start=True) resets anyway
            for _ in range(24):
                nc.tensor.matmul(s1T_ps[0][:, 0:128], lhsT=ident_bf,
                                 rhs=ident_bf, start=True, stop=True)

            for kp2 in range(11):
                b, a = (2 * kp2) // XB, (2 * kp2) % XB
                for g in range(2):
                    for h in range(2):
                        nc.tensor.matmul(
                            s1T_ps[g][:, h * 512 : (h + 1) * 512],
                            lhsT=w1bigs[b][:, a : a + 2,
                                           g * 128 : (g + 1) * 128],
                            rhs=xbigs[b][:, a : a + 2,
                                         h * 512 : (h + 1) * 512],
                            start=(kp2 == 0),
                            stop=False,
                            perf_mode=DR,
                        )
            for kx in (22, 23):
                kp = min(128, D_IN - kx * 128)
                if kx == 23:
                    lhs_src, rhs_src = w1sm[:kp, :], xsm[:kp, :]
                else:
                    lhs_src, rhs_src = (w1bigs[5][:kp, 2, :],
                                        xbigs[5][:kp, 2, :])
                for g in range(2):
                    for h in range(2):
                        nc.tensor.matmul(
                            s1T_ps[g][:, h * 512 : (h + 1) * 512],
                            lhsT=lhs_src[:, g * 128 : (g + 1) * 128],
                            rhs=rhs_src[:, h * 512 : (h + 1) * 512],
                            start=False,
                            stop=(kx == 23),
                        )

            # ---- s1 chain: tanh, transpose, stage quarter-AGs ----
            for g in range(2):
                nc.scalar.activation(s1T_bf[:, g], s1T_ps[g], AF.Tanh)
            psA_ctx.close()
            z1_ps = [psZ.tile([128, NL], F32, name=f"z1g{g}") for g in range(2)]
            with tc.tile_pool(name="psT", bufs=2, space="PSUM") as psT:
                for q in range(4):
                    for mb in (2 * q, 2 * q + 1):
                        for g in range(2):
                            tp = psT.tile([128, 128], BF16, name="tp")
                            nc.tensor.transpose(
                                tp,
                                s1T_bf[:, g, mb * 128 : (mb + 1) * 128],
                                ident_bf,
                            )
                            nc.scalar.activation(
                                s1loc[:, mb, g * 128 : (g + 1) * 128],
                                tp, AF.Copy,
                            )
                    nc.scalar.dma_start(out=s1_ins[q][:, :, :],
                                        in_=s1loc[:, 2 * q : 2 * q + 2, :])
                    nc.gpsimd.collective_compute(
                        "AllGather", mybir.AluOpType.bypass,
                        replica_groups=RG,
                        ins=[s1_ins[q].opt()], outs=[s1_outs[q].opt()],
                    )

            # warmers bridging the collectives-init barrier window so
            # round 1 opens at full clock
            with tc.tile_pool(name="psW1", bufs=1, space="PSUM") as psW1:
                wps = psW1.tile([128, 512], F32)
                for w in range(56):
                    nc.tensor.matmul(
                        wps, lhsT=w1bigs[0][:, 0:2, 0:128],
                        rhs=xbigs[0][:, 0:2, 0:512],
                        start=True, stop=True, perf_mode=DR,
                    )

            # ---- round 1: z1 = conT8.T-domain matmuls in 4 AG waves;
            # final wave ordered h0-first so the s2 chain starts early.
            for q in range(4):
                nc.sync.dma_start(
                    out=s1f[:, :, 2 * q : 2 * q + 2, :],
                    in_=s1_outs[q].rearrange("(r p) a c -> p r a c", p=128),
                )

            def r1_mm(q, r, g, h):
                sl = slice(h * 512, (h + 1) * 512)
                k = 8 * r + 2 * q
                nc.tensor.matmul(
                    z1_ps[g][:, sl],
                    lhsT=s1f[:, r, 2 * q : 2 * q + 2,
                             g * 128 : (g + 1) * 128],
                    rhs=conT8[:, k : k + 2, sl],
                    start=(q == 0 and r == 0),
                    stop=(q == 3 and r == 7),
                    perf_mode=DR,
                )

            for q in range(3):
                for r in range(8):
                    for g in range(2):
                        for h in range(2):
                            r1_mm(q, r, g, h)

            s2T_bf = pA.tile([128, NL], BF16)
            s2loc = pA.tile([128, 8, H2], F8)
            s2T_psx = ExitStack()
            psC = s2T_psx.enter_context(
                tc.tile_pool(name="psC", bufs=1, space="PSUM"))
            s2T_ps = psC.tile([128, NL], F32)

            def s2_chain(h):
                # z1sb copies ride DVE+ACT; the W2 matmuls interleave on
                # the PE behind the h1 wave
                sl = slice(h * 512, (h + 1) * 512)
                nc.vector.tensor_copy(z1sb[:, 0, sl], z1_ps[0][:, sl])
                nc.scalar.copy(z1sb[:, 1, sl], z1_ps[1][:, sl])
                for b in range(2):
                    nc.tensor.matmul(
                        s2T_ps[:, sl], lhsT=W2_sb[:, b], rhs=z1sb[:, b, sl],
                        start=(b == 0), stop=(b == 1),
                    )
                nc.scalar.activation(s2T_bf[:, sl], s2T_ps[:, sl],
                                     AF.Tanh, scale=INV13)

            def s2_stage(t):
                with tc.tile_pool(name=f"psT2{t}", bufs=2, space="PSUM") as p2:
                    for mb in range(4 * t, 4 * t + 4):
                        tp = p2.tile([128, 128], BF16, name="tp2")
                        nc.tensor.transpose(
                            tp, s2T_bf[:, mb * 128 : (mb + 1) * 128], ident_bf
                        )
                        nc.scalar.activation(s2loc[:, mb], tp, AF.Copy)
                nc.scalar.dma_start(out=s2_ins[t][:, :, :],
                                    in_=s2loc[:, 4 * t : 4 * t + 4, :])
                nc.gpsimd.collective_compute(
                    "AllGather", mybir.AluOpType.bypass,
                    replica_groups=RG,
                    ins=[s2_ins[t].opt()], outs=[s2_outs[t].opt()],
                )

            # wave 3: h0 for both g, then the h0 s2-chain, then h1
            for h in range(2):
                for g in range(2):
                    for r in range(8):
                        r1_mm(3, r, g, h)
                s2_chain(h)
                s2_stage(h)   # half t = h here: t0 = mb 0..3 needs s2T h0
            s2T_psx.close()

        # ================= merged tail: phases B + C + D ===============
        with tc.tile_pool(name="tail", bufs=1) as tl, \
             tc.tile_pool(name="psG", bufs=1, space="PSUM") as psG:
            aTs = [None, None, None]
            sqs = [None, None, None]
            embs_bf = [zf_bf, com_bf, zs_bf]

            def emit_attn_view(v, hs=(0, 1)):
                if aTs[v] is None:
                    aTs[v] = tl.tile([64, NL], F32, name=f"aT{v}")
                    sqs[v] = tl.tile([64, NL], F32, name=f"sq{v}")
                a_ps = psG.tile([64, NL], F32, name="aps", bufs=1)
                for h in hs:
                    sl = slice(h * 512, (h + 1) * 512)
                    nc.tensor.matmul(a_ps[:, sl], lhsT=wlW_sb,
                                     rhs=embs_bf[v][:, sl])
                    nc.vector.tensor_scalar_add(aTs[v][:, sl], a_ps[:, sl],
                                                wl_b_sb)
                    # square folds the wl_b bias: (a_ps + b)^2 on ACT
                    nc.scalar.activation(sqs[v][:, sl], a_ps[:, sl],
                                         AF.Square, bias=wl_b_sb)

            # com-independent attention views fill the AG-s2 gap
            emit_attn_view(0)
            emit_attn_view(2)
            sq02 = tl.tile([64, NL], F32)
            nc.gpsimd.tensor_add(sq02, sqs[0], sqs[2])

            # warmers through the AG-s2 mesh wait
            with tc.tile_pool(name="psW2", bufs=1, space="PSUM") as psW2:
                wps2 = psW2.tile([128, 512], F32)
                for w in range(14):
                    nc.tensor.matmul(
                        wps2, lhsT=conT8[:, 0:2, 0:128],
                        rhs=conT8[:, 2:4, 0:512],
                        start=True, stop=True, perf_mode=DR,
                    )

            # ---- round 2 (z2 = conT8-dom @ s2), 2 AG waves ----
            z2sb = tl.tile([128, NL], BF16)
            s3T_bf = tl.tile([64, NL], BF16)
            s3loc = tl.tile([128, 8, Z], F8)
            s2f = tl.tile([128, 8, 8, H2], F8)
            with tc.tile_pool(name="psD", bufs=1, space="PSUM") as psD, \
                 tc.tile_pool(name="psE", bufs=1, space="PSUM") as psE:
                z2_ps = psD.tile([128, NL], F32)
                s3T_ps = psE.tile([64, NL], F32)
                for t in range(2):
                    nc.sync.dma_start(
                        out=s2f[:, :, 4 * t : 4 * t + 4, :],
                        in_=s2_outs[t].rearrange("(r p) a c -> p r a c", p=128),
                    )

                def r2_mm(t, r, a0, h):
                    sl = slice(h * 512, (h + 1) * 512)
                    k = 8 * r + a0
                    nc.tensor.matmul(
                        z2_ps[:, sl],
                        lhsT=s2f[:, r, a0 : a0 + 2, :],
                        rhs=conT8[:, k : k + 2, sl],
                        start=(t == 0 and r == 0 and a0 == 0),
                        stop=(t == 1 and r == 7 and a0 == 6),
                        perf_mode=DR,
                    )

                for r in range(8):
                    for a0 in (0, 2):
                        for h in range(2):
                            r2_mm(0, r, a0, h)

                def s3_chain(h):
                    sl = slice(h * 512, (h + 1) * 512)
                    nc.vector.tensor_copy(z2sb[:, sl], z2_ps[:, sl])
                    nc.tensor.matmul(s3T_ps[:, sl], lhsT=W3_sb,
                                     rhs=z2sb[:, sl])
                    nc.scalar.activation(s3T_bf[:, sl], s3T_ps[:, sl],
                                         AF.Copy, scale=INV13)

                def s3_stage(t):
                    with tc.tile_pool(name=f"psT3{t}", bufs=2,
                                      space="PSUM") as p3:
                        for mb in range(4 * t, 4 * t + 4):
                            tp = p3.tile([128, 64], BF16, name="tp3")
                            nc.tensor.transpose(
                                tp, s3T_bf[:, mb * 128 : (mb + 1) * 128],
                                ident_bf[:64, :64],
                            )
                            nc.scalar.activation(s3loc[:, mb], tp, AF.Copy)
                    nc.scalar.dma_start(out=s3_ins[t][:, :, :],
                                        in_=s3loc[:, 4 * t : 4 * t + 4, :])
                    nc.gpsimd.collective_compute(
                        "AllGather", mybir.AluOpType.bypass,
                        replica_groups=RG,
                        ins=[s3_ins[t].opt()], outs=[s3_outs[t].opt()],
                    )

                # wave 1: h0 first, then the h0 s3-chain, then h1
                for h in range(2):
                    for r in range(8):
                        for a0 in (4, 6):
                            r2_mm(1, r, a0, h)
                    s3_chain(h)
                    s3_stage(h)

            # warmers through the AG-s3 mesh wait
            with tc.tile_pool(name="psW3", bufs=1, space="PSUM") as psW3:
                wps3 = psW3.tile([128, 512], F32)
                for w in range(10):
                    nc.tensor.matmul(
                        wps3, lhsT=conT8[:, 0:2, 0:128],
                        rhs=conT8[:, 2:4, 0:512],
                        start=True, stop=True, perf_mode=DR,
                    )

            # ---- round 3 (com = conT8-dom @ s3) + pipelined fusion ----
            s3f = tl.tile([128, 8, 8, Z], F8)
            sq = tl.tile([64, NL], F32)
            nrm = tl.tile([64, NL], F32)
            rec = tl.tile([64, NL], F32)
            us = [tl.tile([64, NL], BF16, name=f"u{v}") for v in range(3)]
            uts = [tl.tile([64, NL], F32, name=f"ut{v}") for v in range(3)]
            outT_sb = tl.tile([64, NL], F32)

            with tc.tile_pool(name="psF", bufs=1, space="PSUM") as psF:
                com_ps = psF.tile([64, NL], F32)
                out_ps = psF.tile([64, NL], F32, name="ops")
                for t in range(2):
                    nc.sync.dma_start(
                        out=s3f[:, :, 4 * t : 4 * t + 4, :],
                        in_=s3_outs[t].rearrange("(r p) a c -> p r a c", p=128),
                    )

                def r3_mm(t, r, a0, h):
                    sl = slice(h * 512, (h + 1) * 512)
                    k = 8 * r + a0
                    nc.tensor.matmul(
                        com_ps[:, sl],
                        lhsT=s3f[:, r, a0 : a0 + 2, :],
                        rhs=conT8[:, k : k + 2, sl],
                        start=(t == 0 and r == 0 and a0 == 0),
                        stop=(t == 1 and r == 7 and a0 == 6),
                        perf_mode=DR,
                    )

                for r in range(8):
                    for a0 in (0, 2):
                        for h in range(2):
                            r3_mm(0, r, a0, h)

                def fuse_half(h):
                    sl = slice(h * 512, (h + 1) * 512)
                    # com lands in bf16 via the ACT copy (scale folds 2^-13)
                    nc.scalar.activation(com_bf[:, sl], com_ps[:, sl],
                                         AF.Copy, scale=INV13)
                    emit_attn_view(1, hs=(h,))
                    nc.vector.tensor_add(sq[:, sl], sq02[:, sl],
                                         sqs[1][:, sl])
                    nc.scalar.activation(nrm[:, sl], sq[:, sl], AF.Sqrt)
                    nc.vector.reciprocal_approx_fast(rec[:, sl], nrm[:, sl])
                    for v, eng in ((1, nc.vector), (0, nc.gpsimd),
                                   (2, nc.gpsimd)):
                        eng.tensor_mul(uts[v][:, sl], aTs[v][:, sl],
                                       rec[:, sl])
                        eng.tensor_mul(us[v][:, sl], uts[v][:, sl],
                                       embs_bf[v][:, sl])
                    for vi, v in enumerate((1, 0, 2)):
                        nc.tensor.matmul(
                            out_ps[:, sl], lhsT=mlpW_sb[:, v],
                            rhs=us[v][:, sl],
                            start=(vi == 0), stop=(vi == 2),
                        )
                    nc.vector.tensor_scalar_add(outT_sb[:, sl],
                                                out_ps[:, sl], mlp_b_sb)
                    nc.sync.dma_start(out=outT[:, sl], in_=outT_sb[:, sl])

                # wave 1: h0 first, fuse h0 while h1 finishes
                for h in range(2):
                    for r in range(8):
                        for a0 in (4, 6):
                            r3_mm(1, r, a0, h)
                    fuse_half(h)


_CACHE = {}


def _build():
    if "nc" in _CACHE:
        return _CACHE["nc"]
    nc = bacc.Bacc("TRN2", target_bir_lowering=False, debug=False,
                   num_devices=N_CORES)
    io = {
        "adjT_f": nc.dram_tensor("adjT_f", [N, NL], F8, kind="ExternalInput"),
        "adjT_s": nc.dram_tensor("adjT_s", [N, NL], F8, kind="ExternalInput"),
        "xT": nc.dram_tensor("xT", [D_IN, NL], F8, kind="ExternalInput"),
        "zfT": nc.dram_tensor("zfT", [Z, NL], F32, kind="ExternalInput"),
        "zsT": nc.dram_tensor("zsT", [Z, NL], F32, kind="ExternalInput"),
        "W1": nc.dram_tensor("W1", [D_IN, H1], F8, kind="ExternalInput"),
        "W2": nc.dram_tensor("W2", [H1, H2], F32, kind="ExternalInput"),
        "W3": nc.dram_tensor("W3", [H2, Z], F32, kind="ExternalInput"),
        "wl_W": nc.dram_tensor("wl_W", [Z, Z], F32, kind="ExternalInput"),
        "mlp_W": nc.dram_tensor("mlp_W", [3 * Z, Z], F32, kind="ExternalInput"),
        "wl_b": nc.dram_tensor("wl_b", [Z], F32, kind="ExternalInput"),
        "mlp_b": nc.dram_tensor("mlp_b", [Z], F32, kind="ExternalInput"),
        "outT": nc.dram_tensor("outT", [Z, NL], F32, kind="ExternalOutput"),
    }
    with tile.TileContext(nc) as tc:
        _emit(nc, tc, io)
    nc.compile()
    _CACHE["nc"] = nc
    return nc


def _shard_inputs(inputs):
    """Full inputs -> per-core input maps (host-side sharding only)."""
    f32 = np.float32
    adj_f = np.asarray(inputs["adj_feature"], f32)
    adj_s = np.asarray(inputs["adj_spatial"], f32)
    x = np.asarray(inputs["x"], f32)
    zf = np.asarray(inputs["z_feature"], f32)
    zs = np.asarray(inputs["z_spatial"], f32)
    meta = float(np.asarray(inputs["meta"], f32).reshape(-1)[0])
    import ml_dtypes
    fp8 = ml_dtypes.float8_e4m3fn
    rep = {
        "W1": np.ascontiguousarray(np.asarray(inputs["W1"], f32).astype(fp8)),
        "W2": np.ascontiguousarray(np.asarray(inputs["W2"], f32)),
        "W3": np.ascontiguousarray(np.asarray(inputs["W3"], f32)),
        "wl_W": np.ascontiguousarray(np.asarray(inputs["wl_W"], f32)),
        "mlp_W": np.ascontiguousarray(np.asarray(inputs["mlp_W"], f32)),
        "wl_b": np.ascontiguousarray(np.asarray(inputs["wl_b"], f32)),
        "mlp_b": np.ascontiguousarray(np.asarray(inputs["mlp_b"], f32)),
    }
    # fp8 staging with a fixed 2^13 exponent shift; the meta /(1-meta)
    # blend weights fold into the per-matrix quantization scales so the
    # on-device blend is a pure add (ridden by the DMA CCE adder).
    adj_fT8 = (adj_f.T * (8192.0 * meta)).astype(fp8)
    adj_sT8 = (adj_s.T * (8192.0 * (1.0 - meta))).astype(fp8)
    xT = np.ascontiguousarray(x.T)
    zfT = np.ascontiguousarray(zf.T)
    zsT = np.ascontiguousarray(zs.T)
    in_maps = []
    for i in range(N_CORES):
        r = slice(NL * i, NL * (i + 1))
        m = {
            "adjT_f": np.ascontiguousarray(adj_fT8[:, r]),
            "adjT_s": np.ascontiguousarray(adj_sT8[:, r]),
            "xT": np.ascontiguousarray(xT[:, r]).astype(fp8),
            "zfT": np.ascontiguousarray(zfT[:, r]),
            "zsT": np.ascontiguousarray(zsT[:, r]),
        }
        m.update(rep)
        in_maps.append(m)
    return in_maps


def run(trace=False, **inputs):
    nc = _build()
    in_maps = _shard_inputs(inputs)
    res = run_bass_kernel_spmd(nc, in_maps, list(range(N_CORES)), trace=trace)
    out = np.concatenate(
        [np.asarray(res.results[i]["outT"]).T for i in range(N_CORES)], axis=0
    ).astype(np.float32)
    return out, res


def kernel(**inputs):
    out, _ = run(trace=False, **inputs)
    return out


# revision 17
# speedup vs baseline: 1.0443x; 1.0443x over previous
"""Trainium2 Bass kernel for nn_CAM (GNN message passing, 8-core SPMD).

Strategy (per core i of 8, owning node rows R_i = [1024*i, 1024*(i+1))):
  - Host ships the TRANSPOSED column-block of each adjacency as
    fp8_e4m3 pre-scaled by meta*2^13 (feature) and (1-meta)*2^13
    (spatial).  The blend  con = meta*A_f + (1-meta)*A_s  then reduces
    to a pure ADD, which rides the DMA engines' inline CCE ALU:
    the spatial slab DMAs land in the resident conT8 tile and the
    feature slab DMAs accumulate into it (gpsimd SWDGE, accum_op=add).
    No vector-engine blend pass at all; the 2^-13 fold-back is a
    compile-time ACT scale.
  - All three adj@support rounds run as fp8 DoubleRow matmuls in the
    transposed [h, 1024] domain against the resident conT8.
  - Support matrices are exchanged across cores via AllGather bounced
    through shared DRAM.  s1 goes as FOUR quarter-AGs (Mesh algorithm,
    lower latency than RDH halves) consumed by four round-1 waves; s2
    and s3 as two half-AGs each.  The final wave of each round is
    ordered h0-before-h1 so the next support chain and its first AG
    trigger fire at the ~75% point of the round.
  - Discarded warmer matmuls bridge every collective wait so the PE's
    HAM clock gate stays released (cold PE runs at 1.2 GHz vs 2.4).
  - The attention fusion runs in the transposed [64, 1024] domain;
    com-independent views compute inside the AG-s2 window and the
    com-dependent tail is pipelined in two 512-column halves so the
    output DMA starts as soon as the first half of round 3 closes.
"""

import sys

if "/opt/trn_rl_repo" not in sys.path:
    sys.path.insert(0, "/opt/trn_rl_repo")

from contextlib import ExitStack

import numpy as np

import concourse.bass as bass
import concourse.tile as tile
from concourse import bacc, mybir
from concourse.bass_utils import run_bass_kernel_spmd
from concourse.masks import make_identity


F32 = mybir.dt.float32
BF16 = mybir.dt.bfloat16
F8 = mybir.dt.float8e4
AF = mybir.ActivationFunctionType
DR = mybir.MatmulPerfMode.DoubleRow
MUL = mybir.AluOpType.mult
ADD = mybir.AluOpType.add

N = 8192
D_IN = 3000
H1, H2, Z = 256, 128, 64
N_CORES = 8
NL = N // N_CORES           # 1024 local nodes per core
KC = N // 128               # 64 contraction chunks of 128
XC = (D_IN + 127) // 128    # 24 x-feature chunks (last partial: 56)
XB = 4                      # x chunks per batched DMA
RG = [list(range(N_CORES))]
INV13 = 1.0 / 8192.0        # 2^-13 fold-back for the fp8 exponent shift

USE_ACCUM_BLEND = False     # blend via DMA CCE add (else DVE/Pool adds)
NSLAB = 16                  # adjacency stream slabs (KC/NSLAB chunks each)
SCH = KC // NSLAB


def _emit(nc, tc, io):
    adjT_f, adjT_s, xT = io["adjT_f"], io["adjT_s"], io["xT"]
    zfT, zsT = io["zfT"], io["zsT"]
    W1, W2, W3 = io["W1"], io["W2"], io["W3"]
    wl_W, mlp_W = io["wl_W"], io["mlp_W"]
    wl_b, mlp_b = io["wl_b"], io["mlp_b"]
    outT = io["outT"]

    ctx = ExitStack()
    with ctx:
        const = ctx.enter_context(tc.tile_pool(name="const", bufs=1))
        dram = ctx.enter_context(tc.tile_pool(name="dram", bufs=1, space="DRAM"))

        ident_bf = const.tile([128, 128], BF16)
        make_identity(nc, ident_bf)
        wl_b_sb = const.tile([64, 1], F32)
        nc.gpsimd.dma_start(out=wl_b_sb, in_=wl_b[:, None])
        mlp_b_sb = const.tile([64, 1], F32)
        nc.gpsimd.dma_start(out=mlp_b_sb, in_=mlp_b[:, None])
        W2_sb = const.tile([128, 2, H2], BF16)
        nc.gpsimd.dma_start(out=W2_sb, in_=W2.rearrange("(b k) c -> k b c", b=2))
        W3_sb = const.tile([128, Z], BF16)
        nc.gpsimd.dma_start(out=W3_sb, in_=W3[:, :])

        # resident blended adjacency (transposed, fp8, x2^13)
        conT8 = const.tile([128, KC, NL], F8)
        z1sb = const.tile([128, 2, NL], BF16)

        # attention operands live across phases -> const pool
        com_bf = const.tile([64, NL], BF16)
        zf_bf = const.tile([64, NL], BF16)
        zs_bf = const.tile([64, NL], BF16)
        wlW_sb = const.tile([64, 64], BF16)
        mlpW_sb = const.tile([64, 3, 64], BF16)

        # AG bounce buffers (fp8); s1 as four quarters, s2/s3 as halves
        s1_ins = [dram.tile([128, 2, H1], F8, name=f"s1in{t}") for t in range(4)]
        s1_outs = [dram.tile([NL, 2, H1], F8, addr_space="Shared",
                             name=f"s1out{t}") for t in range(4)]
        s2_ins = [dram.tile([128, 4, H2], F8, name=f"s2in{t}") for t in range(2)]
        s2_outs = [dram.tile([NL, 4, H2], F8, addr_space="Shared",
                             name=f"s2out{t}") for t in range(2)]
        s3_ins = [dram.tile([128, 4, Z], F8, name=f"s3in{t}") for t in range(2)]
        s3_outs = [dram.tile([NL, 4, Z], F8, addr_space="Shared",
                             name=f"s3out{t}") for t in range(2)]

        # ======== phase A: stream+blend, s1 chain, round 1 ============
        with tc.tile_pool(name="phaseA", bufs=1) as pA, \
             tc.tile_pool(name="psZ", bufs=1, space="PSUM") as psZ:
            s1T_bf = pA.tile([128, 2, NL], BF16)
            s1loc = pA.tile([128, 8, H1], F8)
            s1f = pA.tile([128, 8, 8, H1], F8)

            psA_ctx = ExitStack()
            psA = psA_ctx.enter_context(
                tc.tile_pool(name="psA", bufs=1, space="PSUM")
            )
            s1T_ps = [psA.tile([128, NL], F32, name=f"s1T{g}") for g in range(2)]

            # ---- batched fp8 x/W1 loads: first on both HWDGE rings ----
            xbigs, w1bigs = [], []
            for b in range(6):
                eng = nc.sync if b < 3 else nc.scalar
                r0 = b * XB * 128
                na = 3 if b == 5 else XB  # chunk 23 is partial (56 rows)
                xb = pA.tile([128, XB, NL], F8, name="xbig", bufs=6)
                eng.dma_start(
                    out=xb[:, :na, :],
                    in_=xT[r0 : r0 + na * 128, :].rearrange(
                        "(a p) m -> p a m", p=128),
                )
                xbigs.append(xb)
                wb = pA.tile([128, XB, H1], F8, name="w1big", bufs=6)
                eng.dma_start(
                    out=wb[:, :na, :],
                    in_=W1[r0 : r0 + na * 128, :].rearrange(
                        "(a p) m -> p a m", p=128),
                )
                w1bigs.append(wb)
            xsm = pA.tile([128, NL], F8)
            nc.scalar.dma_start(out=xsm[:56], in_=xT[2944:3000, :])
            w1sm = pA.tile([128, H1], F8)
            nc.scalar.dma_start(out=w1sm[:56], in_=W1[2944:3000, :])

            # ---- adjacency stream + blend-on-DMA ----
            # spatial slabs (pre-scaled by (1-meta)*2^13) land straight in
            # the resident conT8; feature slabs (pre-scaled by meta*2^13)
            # accumulate into it via the SDMA CCE adder.  The vector
            # engine never touches the 8M-element blend.
            for j in range(NSLAB):
                r0, r1 = j * SCH * 128, (j + 1) * SCH * 128
                dst = conT8[:, j * SCH : (j + 1) * SCH, :]
                nc.sync.dma_start(
                    out=dst,
                    in_=adjT_s[r0:r1, :].rearrange("(a p) m -> p a m", p=128),
                )
                if USE_ACCUM_BLEND:
                    nc.gpsimd.dma_start(
                        out=dst,
                        in_=adjT_f[r0:r1, :].rearrange("(a p) m -> p a m", p=128),
                        accum_op=ADD,
                    )
                else:
                    af = pA.tile([128, SCH, NL], F8, name="af", bufs=6)
                    nc.scalar.dma_start(
                        out=af,
                        in_=adjT_f[r0:r1, :].rearrange("(a p) m -> p a m", p=128),
                    )
                    eng = nc.vector if j % 2 == 0 else nc.gpsimd
                    eng.tensor_add(dst, dst, af)

            # attention inputs (small; gpsimd SWDGE casts f32->bf16)
            nc.gpsimd.dma_start(out=zf_bf, in_=zfT[:, :])
            nc.gpsimd.dma_start(out=zs_bf, in_=zsT[:, :])
            nc.gpsimd.dma_start(out=wlW_sb, in_=wl_W[:, :])
            nc.gpsimd.dma_start(
                out=mlpW_sb, in_=mlp_W.rearrange("(v c) d -> c v d", v=3)
            )

            # ---- s1 = tanh(xT.T @ W1) in the transposed domain ----
            # early identity matmuls release the HAM clock gate before
            # the real s1 stream arrives; they scribble on s1T_ps, which
            # the first real accumulation (start=True) resets anyway
            for _ in range(24):
                nc.tensor.matmul(s1T_ps[0][:, 0:128], lhsT=ident_bf,
                                 rhs=ident_bf, start=True, stop=True)

            for kp2 in range(11):
                b, a = (2 * kp2) // XB, (2 * kp2) % XB
                for g in range(2):
                    for h in range(2):
                        nc.tensor.matmul(
                            s1T_ps[g][:, h * 512 : (h + 1) * 512],
                            lhsT=w1bigs[b][:, a : a + 2,
                                           g * 128 : (g + 1) * 128],
                            rhs=xbigs[b][:, a : a + 2,
                                         h * 512 : (h + 1) * 512],
                            start=(kp2 == 0),
                            stop=False,
                            perf_mode=DR,
                        )
            for kx in (22, 23):
                kp = min(128, D_IN - kx * 128)
                if kx == 23:
                    lhs_src, rhs_src = w1sm[:kp, :], xsm[:kp, :]
                else:
                    lhs_src, rhs_src = (w1bigs[5][:kp, 2, :],
                                        xbigs[5][:kp, 2, :])
                for g in range(2):
                    for h in range(2):
                        nc.tensor.matmul(
                            s1T_ps[g][:, h * 512 : (h + 1) * 512],
                            lhsT=lhs_src[:, g * 128 : (g + 1) * 128],
                            rhs=rhs_src[:, h * 512 : (h + 1) * 512],
                            start=False,
                            stop=(kx == 23),
                        )

            # ---- s1 chain: tanh, transpose, stage quarter-AGs ----
            for g in range(2):
                nc.scalar.activation(s1T_bf[:, g], s1T_ps[g], AF.Tanh)
            psA_ctx.close()
            z1_ps = [psZ.tile([128, NL], F32, name=f"z1g{g}") for g in range(2)]
            with tc.tile_pool(name="psT", bufs=2, space="PSUM") as psT:
                for q in range(4):
                    for mb in (2 * q, 2 * q + 1):
                        for g in range(2):
                            tp = psT.tile([128, 128], BF16, name="tp")
                            nc.tensor.transpose(
                                tp,
                                s1T_bf[:, g, mb * 128 : (mb + 1) * 128],
                                ident_bf,
                            )
                            nc.scalar.activation(
                                s1loc[:, mb, g * 128 : (g + 1) * 128],
                                tp, AF.Copy,
                            )
                    nc.scalar.dma_start(out=s1_ins[q][:, :, :],
                                        in_=s1loc[:, 2 * q : 2 * q + 2, :])
                    nc.gpsimd.collective_compute(
                        "AllGather", mybir.AluOpType.bypass,
                        replica_groups=RG,
                        ins=[s1_ins[q].opt()], outs=[s1_outs[q].opt()],
                    )

            # warmers bridging the collectives-init barrier window so
            # round 1 opens at full clock
            with tc.tile_pool(name="psW1", bufs=1, space="PSUM") as psW1:
                wps = psW1.tile([128, 512], F32)
                for w in range(150):
                    nc.tensor.matmul(
                        wps, lhsT=w1bigs[0][:, 0:2, 0:128],
                        rhs=xbigs[0][:, 0:2, 0:512],
                        start=True, stop=True, perf_mode=DR,
                    )

            # ---- round 1: z1 = conT8.T-domain matmuls in 4 AG waves;
            # final wave ordered h0-first so the s2 chain starts early.
            for q in range(4):
                nc.sync.dma_start(
                    out=s1f[:, :, 2 * q : 2 * q + 2, :],
                    in_=s1_outs[q].rearrange("(r p) a c -> p r a c", p=128),
                )

            def r1_mm(q, r, g, h):
                sl = slice(h * 512, (h + 1) * 512)
                k = 8 * r + 2 * q
                nc.tensor.matmul(
                    z1_ps[g][:, sl],
                    lhsT=s1f[:, r, 2 * q : 2 * q + 2,
                             g * 128 : (g + 1) * 128],
                    rhs=conT8[:, k : k + 2, sl],
                    start=(q == 0 and r == 0),
                    stop=(q == 3 and r == 7),
                    perf_mode=DR,
                )

            for q in range(3):
                for r in range(8):
                    for g in range(2):
                        for h in range(2):
                            r1_mm(q, r, g, h)

            s2T_bf = pA.tile([128, NL], BF16)
            s2loc = pA.tile([128, 8, H2], F8)
            s2T_psx = ExitStack()
            psC = s2T_psx.enter_context(
                tc.tile_pool(name="psC", bufs=1, space="PSUM"))
            s2T_ps = psC.tile([128, NL], F32)

            def s2_chain(h):
                # z1sb copies ride DVE+ACT; the W2 matmuls interleave on
                # the PE behind the h1 wave
                sl = slice(h * 512, (h + 1) * 512)
                nc.vector.tensor_copy(z1sb[:, 0, sl], z1_ps[0][:, sl])
                nc.scalar.copy(z1sb[:, 1, sl], z1_ps[1][:, sl])
                for b in range(2):
                    nc.tensor.matmul(
                        s2T_ps[:, sl], lhsT=W2_sb[:, b], rhs=z1sb[:, b, sl],
                        start=(b == 0), stop=(b == 1),
                    )
                nc.scalar.activation(s2T_bf[:, sl], s2T_ps[:, sl],
                                     AF.Tanh, scale=INV13)

            def s2_stage(t):
                with tc.tile_pool(name=f"psT2{t}", bufs=2, space="PSUM") as p2:
                    for mb in range(4 * t, 4 * t + 4):
                        tp = p2.tile([128, 128], BF16, name="tp2")
                        nc.tensor.transpose(
                            tp, s2T_bf[:, mb * 128 : (mb + 1) * 128], ident_bf
                        )
                        nc.scalar.activation(s2loc[:, mb], tp, AF.Copy)
                nc.scalar.dma_start(out=s2_ins[t][:, :, :],
                                    in_=s2loc[:, 4 * t : 4 * t + 4, :])
                nc.gpsimd.collective_compute(
                    "AllGather", mybir.AluOpType.bypass,
                    replica_groups=RG,
                    ins=[s2_ins[t].opt()], outs=[s2_outs[t].opt()],
                )

            # wave 3: h0 for both g, then the h0 s2-chain, then h1
            for h in range(2):
                for g in range(2):
                    for r in range(8):
                        r1_mm(3, r, g, h)
                s2_chain(h)
                s2_stage(h)   # half t = h here: t0 = mb 0..3 needs s2T h0
            s2T_psx.close()

        # ================= merged tail: phases B + C + D ===============
        with tc.tile_pool(name="tail", bufs=1) as tl, \
             tc.tile_pool(name="psG", bufs=1, space="PSUM") as psG:
            aTs = [None, None, None]
            sqs = [None, None, None]
            embs_bf = [zf_bf, com_bf, zs_bf]

            def emit_attn_view(v, hs=(0, 1)):
                if aTs[v] is None:
                    aTs[v] = tl.tile([64, NL], F32, name=f"aT{v}")
                    sqs[v] = tl.tile([64, NL], F32, name=f"sq{v}")
                a_ps = psG.tile([64, NL], F32, name="aps", bufs=1)
                for h in hs:
                    sl = slice(h * 512, (h + 1) * 512)
                    nc.tensor.matmul(a_ps[:, sl], lhsT=wlW_sb,
                                     rhs=embs_bf[v][:, sl])
                    nc.vector.tensor_scalar_add(aTs[v][:, sl], a_ps[:, sl],
                                                wl_b_sb)
                    # square folds the wl_b bias: (a_ps + b)^2 on ACT
                    nc.scalar.activation(sqs[v][:, sl], a_ps[:, sl],
                                         AF.Square, bias=wl_b_sb)

            # com-independent attention views fill the AG-s2 gap
            emit_attn_view(0)
            emit_attn_view(2)
            sq02 = tl.tile([64, NL], F32)
            nc.gpsimd.tensor_add(sq02, sqs[0], sqs[2])

            # warmers through the AG-s2 mesh wait
            with tc.tile_pool(name="psW2", bufs=1, space="PSUM") as psW2:
                wps2 = psW2.tile([128, 512], F32)
                for w in range(14):
                    nc.tensor.matmul(
                        wps2, lhsT=conT8[:, 0:2, 0:128],
                        rhs=conT8[:, 2:4, 0:512],
                        start=True, stop=True, perf_mode=DR,
                    )

            # ---- round 2 (z2 = conT8-dom @ s2), 2 AG waves ----
            z2sb = tl.tile([128, NL], BF16)
            s3T_bf = tl.tile([64, NL], BF16)
            s3loc = tl.tile([128, 8, Z], F8)
            s2f = tl.tile([128, 8, 8, H2], F8)
            with tc.tile_pool(name="psD", bufs=1, space="PSUM") as psD, \
                 tc.tile_pool(name="psE", bufs=1, space="PSUM") as psE:
                z2_ps = psD.tile([128, NL], F32)
                s3T_ps = psE.tile([64, NL], F32)
                for t in range(2):
                    nc.sync.dma_start(
                        out=s2f[:, :, 4 * t : 4 * t + 4, :],
                        in_=s2_outs[t].rearrange("(r p) a c -> p r a c", p=128),
                    )

                def r2_mm(t, r, a0, h):
                    sl = slice(h * 512, (h + 1) * 512)
                    k = 8 * r + a0
                    nc.tensor.matmul(
                        z2_ps[:, sl],
                        lhsT=s2f[:, r, a0 : a0 + 2, :],
                        rhs=conT8[:, k : k + 2, sl],
                        start=(t == 0 and r == 0 and a0 == 0),
                        stop=(t == 1 and r == 7 and a0 == 6),
                        perf_mode=DR,
                    )

                for r in range(8):
                    for a0 in (0, 2):
                        for h in range(2):
                            r2_mm(0, r, a0, h)

                def s3_chain(h):
                    sl = slice(h * 512, (h + 1) * 512)
                    nc.vector.tensor_copy(z2sb[:, sl], z2_ps[:, sl])
                    nc.tensor.matmul(s3T_ps[:, sl], lhsT=W3_sb,
                                     rhs=z2sb[:, sl])
                    nc.scalar.activation(s3T_bf[:, sl], s3T_ps[:, sl],
                                         AF.Copy, scale=INV13)

                def s3_stage(t):
                    with tc.tile_pool(name=f"psT3{t}", bufs=2,
                                      space="PSUM") as p3:
                        for mb in range(4 * t, 4 * t + 4):
                            tp = p3.tile([128, 64], BF16, name="tp3")
                            nc.tensor.transpose(
                                tp, s3T_bf[:, mb * 128 : (mb + 1) * 128],
                                ident_bf[:64, :64],
                            )
                            nc.scalar.activation(s3loc[:, mb], tp, AF.Copy)
                    nc.scalar.dma_start(out=s3_ins[t][:, :, :],
                                        in_=s3loc[:, 4 * t : 4 * t + 4, :])
                    nc.gpsimd.collective_compute(
                        "AllGather", mybir.AluOpType.bypass,
                        replica_groups=RG,
                        ins=[s3_ins[t].opt()], outs=[s3_outs[t].opt()],
                    )

                # wave 1: h0 first, then the h0 s3-chain, then h1
                for h in range(2):
                    for r in range(8):
                        for a0 in (4, 6):
                            r2_mm(1, r, a0, h)
                    s3_chain(h)
                    s3_stage(h)

            # warmers through the AG-s3 mesh wait
            with tc.tile_pool(name="psW3", bufs=1, space="PSUM") as psW3:
                wps3 = psW3.tile([128, 512], F32)
                for w in range(10):
                    nc.tensor.matmul(
                        wps3, lhsT=conT8[:, 0:2, 0:128],
                        rhs=conT8[:, 2:4, 0:512],
                        start=True, stop=True, perf_mode=DR,
                    )

            # ---- round 3 (com = conT8-dom @ s3) + pipelined fusion ----
            s3f = tl.tile([128, 8, 8, Z], F8)
            sq = tl.tile([64, NL], F32)
            nrm = tl.tile([64, NL], F32)
            rec = tl.tile([64, NL], F32)
            us = [tl.tile([64, NL], BF16, name=f"u{v}") for v in range(3)]
            uts = [tl.tile([64, NL], F32, name=f"ut{v}") for v in range(3)]
            outT_sb = tl.tile([64, NL], F32)

            with tc.tile_pool(name="psF", bufs=1, space="PSUM") as psF:
                com_ps = psF.tile([64, NL], F32)
                out_ps = psF.tile([64, NL], F32, name="ops")
                for t in range(2):
                    nc.sync.dma_start(
                        out=s3f[:, :, 4 * t : 4 * t + 4, :],
                        in_=s3_outs[t].rearrange("(r p) a c -> p r a c", p=128),
                    )

                def r3_mm(t, r, a0, h):
                    sl = slice(h * 512, (h + 1) * 512)
                    k = 8 * r + a0
                    nc.tensor.matmul(
                        com_ps[:, sl],
                        lhsT=s3f[:, r, a0 : a0 + 2, :],
                        rhs=conT8[:, k : k + 2, sl],
                        start=(t == 0 and r == 0 and a0 == 0),
                        stop=(t == 1 and r == 7 and a0 == 6),
                        perf_mode=DR,
                    )

                for r in range(8):
                    for a0 in (0, 2):
                        for h in range(2):
                            r3_mm(0, r, a0, h)

                def fuse_half(h):
                    sl = slice(h * 512, (h + 1) * 512)
                    # com lands in bf16 via the ACT copy (scale folds 2^-13)
                    nc.scalar.activation(com_bf[:, sl], com_ps[:, sl],
                                         AF.Copy, scale=INV13)
                    emit_attn_view(1, hs=(h,))
                    nc.vector.tensor_add(sq[:, sl], sq02[:, sl],
                                         sqs[1][:, sl])
                    nc.scalar.activation(nrm[:, sl], sq[:, sl], AF.Sqrt)
                    nc.vector.reciprocal_approx_fast(rec[:, sl], nrm[:, sl])
                    for v, eng in ((1, nc.vector), (0, nc.gpsimd),
                                   (2, nc.gpsimd)):
                        eng.tensor_mul(uts[v][:, sl], aTs[v][:, sl],
                                       rec[:, sl])
                        eng.tensor_mul(us[v][:, sl], uts[v][:, sl],
                                       embs_bf[v][:, sl])
                    for vi, v in enumerate((1, 0, 2)):
                        nc.tensor.matmul(
                            out_ps[:, sl], lhsT=mlpW_sb[:, v],
                            rhs=us[v][:, sl],
                            start=(vi == 0), stop=(vi == 2),
                        )
                    nc.vector.tensor_scalar_add(outT_sb[:, sl],
                                                out_ps[:, sl], mlp_b_sb)
                    nc.sync.dma_start(out=outT[:, sl], in_=outT_sb[:, sl])

                # wave 1: h0 first, fuse h0 while h1 finishes
                for h in range(2):
                    for r in range(8):
                        for a0 in (4, 6):
                            r3_mm(1, r, a0, h)
                    fuse_half(h)


_CACHE = {}


def _build():
    if "nc" in _CACHE:
        return _CACHE["nc"]
    nc = bacc.Bacc("TRN2", target_bir_lowering=False, debug=False,
                   num_devices=N_CORES)
    io = {
        "adjT_f": nc.dram_tensor("adjT_f", [N, NL], F8, kind="ExternalInput"),
        "adjT_s": nc.dram_tensor("adjT_s", [N, NL], F8, kind="ExternalInput"),
        "xT": nc.dram_tensor("xT", [D_IN, NL], F8, kind="ExternalInput"),
        "zfT": nc.dram_tensor("zfT", [Z, NL], F32, kind="ExternalInput"),
        "zsT": nc.dram_tensor("zsT", [Z, NL], F32, kind="ExternalInput"),
        "W1": nc.dram_tensor("W1", [D_IN, H1], F8, kind="ExternalInput"),
        "W2": nc.dram_tensor("W2", [H1, H2], F32, kind="ExternalInput"),
        "W3": nc.dram_tensor("W3", [H2, Z], F32, kind="ExternalInput"),
        "wl_W": nc.dram_tensor("wl_W", [Z, Z], F32, kind="ExternalInput"),
        "mlp_W": nc.dram_tensor("mlp_W", [3 * Z, Z], F32, kind="ExternalInput"),
        "wl_b": nc.dram_tensor("wl_b", [Z], F32, kind="ExternalInput"),
        "mlp_b": nc.dram_tensor("mlp_b", [Z], F32, kind="ExternalInput"),
        "outT": nc.dram_tensor("outT", [Z, NL], F32, kind="ExternalOutput"),
    }
    with tile.TileContext(nc) as tc:
        _emit(nc, tc, io)
    nc.compile()
    _CACHE["nc"] = nc
    return nc


def _shard_inputs(inputs):
    """Full inputs -> per-core input maps (host-side sharding only)."""
    f32 = np.float32
    adj_f = np.asarray(inputs["adj_feature"], f32)
    adj_s = np.asarray(inputs["adj_spatial"], f32)
    x = np.asarray(inputs["x"], f32)
    zf = np.asarray(inputs["z_feature"], f32)
    zs = np.asarray(inputs["z_spatial"], f32)
    meta = float(np.asarray(inputs["meta"], f32).reshape(-1)[0])
    import ml_dtypes
    fp8 = ml_dtypes.float8_e4m3fn
    rep = {
        "W1": np.ascontiguousarray(np.asarray(inputs["W1"], f32).astype(fp8)),
        "W2": np.ascontiguousarray(np.asarray(inputs["W2"], f32)),
        "W3": np.ascontiguousarray(np.asarray(inputs["W3"], f32)),
        "wl_W": np.ascontiguousarray(np.asarray(inputs["wl_W"], f32)),
        "mlp_W": np.ascontiguousarray(np.asarray(inputs["mlp_W"], f32)),
        "wl_b": np.ascontiguousarray(np.asarray(inputs["wl_b"], f32)),
        "mlp_b": np.ascontiguousarray(np.asarray(inputs["mlp_b"], f32)),
    }
    # fp8 staging with a fixed 2^13 exponent shift; the meta /(1-meta)
    # blend weights fold into the per-matrix quantization scales so the
    # on-device blend is a pure add (ridden by the DMA CCE adder).
    adj_fT8 = (adj_f.T * (8192.0 * meta)).astype(fp8)
    adj_sT8 = (adj_s.T * (8192.0 * (1.0 - meta))).astype(fp8)
    xT = np.ascontiguousarray(x.T)
    zfT = np.ascontiguousarray(zf.T)
    zsT = np.ascontiguousarray(zs.T)
    in_maps = []
    for i in range(N_CORES):
        r = slice(NL * i, NL * (i + 1))
        m = {
            "adjT_f": np.ascontiguousarray(adj_fT8[:, r]),
            "adjT_s": np.ascontiguousarray(adj_sT8[:, r]),
            "xT": np.ascontiguousarray(xT[:, r]).astype(fp8),
            "zfT": np.ascontiguousarray(zfT[:, r]),
            "zsT": np.ascontiguousarray(zsT[:, r]),
        }
        m.update(rep)
        in_maps.append(m)
    return in_maps


def run(trace=False, **inputs):
    nc = _build()
    in_maps = _shard_inputs(inputs)
    res = run_bass_kernel_spmd(nc, in_maps, list(range(N_CORES)), trace=trace)
    out = np.concatenate(
        [np.asarray(res.results[i]["outT"]).T for i in range(N_CORES)], axis=0
    ).astype(np.float32)
    return out, res


def kernel(**inputs):
    out, _ = run(trace=False, **inputs)
    return out


# revision 21
# speedup vs baseline: 1.0801x; 1.0343x over previous
"""Trainium2 Bass kernel for nn_CAM (GNN message passing, 8-core SPMD).

Strategy (per core i of 8, owning node rows R_i = [1024*i, 1024*(i+1))):
  - Host ships the TRANSPOSED column-block of each adjacency as
    fp8_e4m3 pre-scaled by meta*2^13 (feature) and (1-meta)*2^13
    (spatial).  The blend  con = meta*A_f + (1-meta)*A_s  then reduces
    to a pure ADD, which rides the DMA engines' inline CCE ALU:
    the spatial slab DMAs land in the resident conT8 tile and the
    feature slab DMAs accumulate into it (gpsimd SWDGE, accum_op=add).
    No vector-engine blend pass at all; the 2^-13 fold-back is a
    compile-time ACT scale.
  - All three adj@support rounds run as fp8 DoubleRow matmuls in the
    transposed [h, 1024] domain against the resident conT8.
  - Support matrices are exchanged across cores via AllGather bounced
    through shared DRAM.  s1 goes as FOUR quarter-AGs (Mesh algorithm,
    lower latency than RDH halves) consumed by four round-1 waves; s2
    and s3 as two half-AGs each.  The final wave of each round is
    ordered h0-before-h1 so the next support chain and its first AG
    trigger fire at the ~75% point of the round.
  - Discarded warmer matmuls bridge every collective wait so the PE's
    HAM clock gate stays released (cold PE runs at 1.2 GHz vs 2.4).
  - The attention fusion runs in the transposed [64, 1024] domain;
    com-independent views compute inside the AG-s2 window and the
    com-dependent tail is pipelined in two 512-column halves so the
    output DMA starts as soon as the first half of round 3 closes.
"""

import sys

if "/opt/trn_rl_repo" not in sys.path:
    sys.path.insert(0, "/opt/trn_rl_repo")

from contextlib import ExitStack

import numpy as np

import concourse.bass as bass
import concourse.tile as tile
from concourse import bacc, mybir
from concourse.bass_utils import run_bass_kernel_spmd
from concourse.masks import make_identity


F32 = mybir.dt.float32
BF16 = mybir.dt.bfloat16
F8 = mybir.dt.float8e4
AF = mybir.ActivationFunctionType
DR = mybir.MatmulPerfMode.DoubleRow
MUL = mybir.AluOpType.mult
ADD = mybir.AluOpType.add

N = 8192
D_IN = 3000
H1, H2, Z = 256, 128, 64
N_CORES = 8
NL = N // N_CORES           # 1024 local nodes per core
KC = N // 128               # 64 contraction chunks of 128
XC = (D_IN + 127) // 128    # 24 x-feature chunks (last partial: 56)
XB = 4                      # x chunks per batched DMA
RG = [list(range(N_CORES))]
INV13 = 1.0 / 8192.0        # 2^-13 fold-back for the fp8 exponent shift

USE_ACCUM_BLEND = False     # blend via DMA CCE add (else DVE/Pool adds)
NSLAB = 16                  # adjacency stream slabs (KC/NSLAB chunks each)
SCH = KC // NSLAB


def _emit(nc, tc, io):
    adjT_f, adjT_s, xT = io["adjT_f"], io["adjT_s"], io["xT"]
    zfT, zsT = io["zfT"], io["zsT"]
    W1, W2, W3 = io["W1"], io["W2"], io["W3"]
    wl_W, mlp_W = io["wl_W"], io["mlp_W"]
    wl_b, mlp_b = io["wl_b"], io["mlp_b"]
    outT = io["outT"]

    ctx = ExitStack()
    with ctx:
        const = ctx.enter_context(tc.tile_pool(name="const", bufs=1))
        dram = ctx.enter_context(tc.tile_pool(name="dram", bufs=1, space="DRAM"))

        ident_bf = const.tile([128, 128], BF16)
        make_identity(nc, ident_bf)
        ones_sb = const.tile([128, 1], F32)
        nc.vector.memset(ones_sb, 1.0)
        wl_b_sb = const.tile([64, 1], F32)
        nc.gpsimd.dma_start(out=wl_b_sb, in_=wl_b[:, None])
        mlp_b_sb = const.tile([64, 1], F32)
        nc.gpsimd.dma_start(out=mlp_b_sb, in_=mlp_b[:, None])
        W2_sb = const.tile([128, 2, H2], BF16)
        nc.gpsimd.dma_start(out=W2_sb, in_=W2.rearrange("(b k) c -> k b c", b=2))
        W3_sb = const.tile([128, Z], BF16)
        nc.gpsimd.dma_start(out=W3_sb, in_=W3[:, :])

        # resident blended adjacency (transposed, fp8, x2^13)
        conT8 = const.tile([128, KC, NL], F8)
        z1sb = const.tile([128, 2, NL], BF16)

        # attention operands live across phases -> const pool
        com_bf = const.tile([64, NL], BF16)
        zf_bf = const.tile([64, NL], BF16)
        zs_bf = const.tile([64, NL], BF16)
        wlW_sb = const.tile([64, 64], BF16)
        mlpW_sb = const.tile([64, 3, 64], BF16)

        # AG bounce buffers (fp8); s1 as four quarters, s2/s3 as halves
        s1_ins = [dram.tile([128, 2, H1], F8, name=f"s1in{t}") for t in range(4)]
        s1_outs = [dram.tile([NL, 2, H1], F8, addr_space="Shared",
                             name=f"s1out{t}") for t in range(4)]
        s2_ins = [dram.tile([128, 4, H2], F8, name=f"s2in{t}") for t in range(2)]
        s2_outs = [dram.tile([NL, 4, H2], F8, addr_space="Shared",
                             name=f"s2out{t}") for t in range(2)]
        s3_ins = [dram.tile([128, 4, Z], F8, name=f"s3in{t}") for t in range(2)]
        s3_outs = [dram.tile([NL, 4, Z], F8, addr_space="Shared",
                             name=f"s3out{t}") for t in range(2)]

        # ======== phase A: stream+blend, s1 chain, round 1 ============
        with tc.tile_pool(name="phaseA", bufs=1) as pA, \
             tc.tile_pool(name="psZ", bufs=1, space="PSUM") as psZ:
            s1T_bf = pA.tile([128, 2, NL], BF16)
            s1loc = pA.tile([128, 8, H1], F8)
            s1f = pA.tile([128, 8, 8, H1], F8)

            psA_ctx = ExitStack()
            psA = psA_ctx.enter_context(
                tc.tile_pool(name="psA", bufs=1, space="PSUM")
            )
            s1T_ps = [psA.tile([128, NL], F32, name=f"s1T{g}") for g in range(2)]

            # ---- batched fp8 x/W1 loads: first on both HWDGE rings ----
            xbigs, w1bigs = [], []
            for b in range(6):
                eng = nc.sync if b < 3 else nc.scalar
                r0 = b * XB * 128
                na = 3 if b == 5 else XB  # chunk 23 is partial (56 rows)
                xb = pA.tile([128, XB, NL], F8, name="xbig", bufs=6)
                eng.dma_start(
                    out=xb[:, :na, :],
                    in_=xT[r0 : r0 + na * 128, :].rearrange(
                        "(a p) m -> p a m", p=128),
                )
                xbigs.append(xb)
                wb = pA.tile([128, XB, H1], F8, name="w1big", bufs=6)
                eng.dma_start(
                    out=wb[:, :na, :],
                    in_=W1[r0 : r0 + na * 128, :].rearrange(
                        "(a p) m -> p a m", p=128),
                )
                w1bigs.append(wb)
            xsm = pA.tile([128, NL], F8)
            nc.scalar.dma_start(out=xsm[:56], in_=xT[2944:3000, :])
            w1sm = pA.tile([128, H1], F8)
            nc.scalar.dma_start(out=w1sm[:56], in_=W1[2944:3000, :])

            # ---- adjacency stream + blend ----
            # both pre-scaled matrices are staged (ALL on the sync ring:
            # the scalar ring must stay clear so the s1 chain is not
            # queued behind adjacency issue stalls) and summed into the
            # resident conT8 (out not aliased with inputs).  Early slabs
            # blend on Pool so its queue is clear when the collective
            # triggers need to fire; late slabs go to the vector engine.
            for j in range(NSLAB):
                r0, r1 = j * SCH * 128, (j + 1) * SCH * 128
                dst = conT8[:, j * SCH : (j + 1) * SCH, :]
                asl = pA.tile([128, SCH, NL], F8, name="asl", bufs=6)
                nc.sync.dma_start(
                    out=asl,
                    in_=adjT_s[r0:r1, :].rearrange("(a p) m -> p a m", p=128),
                )
                af = pA.tile([128, SCH, NL], F8, name="af", bufs=6)
                nc.sync.dma_start(
                    out=af,
                    in_=adjT_f[r0:r1, :].rearrange("(a p) m -> p a m", p=128),
                )
                if j < 8:
                    nc.gpsimd.tensor_add(dst, af, asl)
                else:
                    nc.vector.scalar_tensor_tensor(
                        out=dst, in0=af, scalar=ones_sb, in1=asl,
                        op0=MUL, op1=ADD,
                    )

            # attention inputs (small; gpsimd SWDGE casts f32->bf16)
            nc.gpsimd.dma_start(out=zf_bf, in_=zfT[:, :])
            nc.gpsimd.dma_start(out=zs_bf, in_=zsT[:, :])
            nc.gpsimd.dma_start(out=wlW_sb, in_=wl_W[:, :])
            nc.gpsimd.dma_start(
                out=mlpW_sb, in_=mlp_W.rearrange("(v c) d -> c v d", v=3)
            )

            # ---- s1 = tanh(xT.T @ W1) in the transposed domain ----
            # early identity matmuls release the HAM clock gate before
            # the real s1 stream arrives; they scribble on s1T_ps, which
            # the first real accumulation (start=True) resets anyway
            for _ in range(24):
                nc.tensor.matmul(s1T_ps[0][:, 0:128], lhsT=ident_bf,
                                 rhs=ident_bf, start=True, stop=True)

            for kp2 in range(11):
                b, a = (2 * kp2) // XB, (2 * kp2) % XB
                for g in range(2):
                    for h in range(2):
                        nc.tensor.matmul(
                            s1T_ps[g][:, h * 512 : (h + 1) * 512],
                            lhsT=w1bigs[b][:, a : a + 2,
                                           g * 128 : (g + 1) * 128],
                            rhs=xbigs[b][:, a : a + 2,
                                         h * 512 : (h + 1) * 512],
                            start=(kp2 == 0),
                            stop=False,
                            perf_mode=DR,
                        )
            for kx in (22, 23):
                kp = min(128, D_IN - kx * 128)
                if kx == 23:
                    lhs_src, rhs_src = w1sm[:kp, :], xsm[:kp, :]
                else:
                    lhs_src, rhs_src = (w1bigs[5][:kp, 2, :],
                                        xbigs[5][:kp, 2, :])
                for g in range(2):
                    for h in range(2):
                        nc.tensor.matmul(
                            s1T_ps[g][:, h * 512 : (h + 1) * 512],
                            lhsT=lhs_src[:, g * 128 : (g + 1) * 128],
                            rhs=rhs_src[:, h * 512 : (h + 1) * 512],
                            start=False,
                            stop=(kx == 23),
                        )

            # ---- s1 chain: tanh, transpose, stage quarter-AGs ----
            for g in range(2):
                nc.scalar.activation(s1T_bf[:, g], s1T_ps[g], AF.Tanh)
            psA_ctx.close()
            z1_ps = [psZ.tile([128, NL], F32, name=f"z1g{g}") for g in range(2)]
            with tc.tile_pool(name="psT", bufs=2, space="PSUM") as psT:
                for q in range(4):
                    for mb in (2 * q, 2 * q + 1):
                        for g in range(2):
                            tp = psT.tile([128, 128], BF16, name="tp")
                            nc.tensor.transpose(
                                tp,
                                s1T_bf[:, g, mb * 128 : (mb + 1) * 128],
                                ident_bf,
                            )
                            nc.scalar.activation(
                                s1loc[:, mb, g * 128 : (g + 1) * 128],
                                tp, AF.Copy,
                            )
                    nc.scalar.dma_start(out=s1_ins[q][:, :, :],
                                        in_=s1loc[:, 2 * q : 2 * q + 2, :])
                    nc.gpsimd.collective_compute(
                        "AllGather", mybir.AluOpType.bypass,
                        replica_groups=RG,
                        ins=[s1_ins[q].opt()], outs=[s1_outs[q].opt()],
                    )

            # warmers bridging the collectives-init barrier window so
            # round 1 opens at full clock
            with tc.tile_pool(name="psW1", bufs=1, space="PSUM") as psW1:
                wps = psW1.tile([128, 512], F32)
                for w in range(24):
                    nc.tensor.matmul(
                        wps, lhsT=w1bigs[0][:, 0:2, 0:128],
                        rhs=xbigs[0][:, 0:2, 0:512],
                        start=True, stop=True, perf_mode=DR,
                    )

            # ---- round 1: z1 = conT8.T-domain matmuls in 4 AG waves;
            # final wave ordered h0-first so the s2 chain starts early.
            for q in range(4):
                nc.sync.dma_start(
                    out=s1f[:, :, 2 * q : 2 * q + 2, :],
                    in_=s1_outs[q].rearrange("(r p) a c -> p r a c", p=128),
                )

            def r1_mm(q, r, g, h):
                sl = slice(h * 512, (h + 1) * 512)
                k = 8 * r + 2 * q
                nc.tensor.matmul(
                    z1_ps[g][:, sl],
                    lhsT=s1f[:, r, 2 * q : 2 * q + 2,
                             g * 128 : (g + 1) * 128],
                    rhs=conT8[:, k : k + 2, sl],
                    start=(q == 0 and r == 0),
                    stop=(q == 3 and r == 7),
                    perf_mode=DR,
                )

            for q in range(3):
                for r in range(8):
                    for g in range(2):
                        for h in range(2):
                            r1_mm(q, r, g, h)

            s2T_bf = pA.tile([128, NL], BF16)
            s2loc = pA.tile([128, 8, H2], F8)
            s2T_psx = ExitStack()
            psC = s2T_psx.enter_context(
                tc.tile_pool(name="psC", bufs=1, space="PSUM"))
            s2T_ps = psC.tile([128, NL], F32)

            def s2_chain(h):
                # z1sb copies ride DVE+ACT; the W2 matmuls interleave on
                # the PE behind the h1 wave
                sl = slice(h * 512, (h + 1) * 512)
                nc.vector.tensor_copy(z1sb[:, 0, sl], z1_ps[0][:, sl])
                nc.scalar.copy(z1sb[:, 1, sl], z1_ps[1][:, sl])
                for b in range(2):
                    nc.tensor.matmul(
                        s2T_ps[:, sl], lhsT=W2_sb[:, b], rhs=z1sb[:, b, sl],
                        start=(b == 0), stop=(b == 1),
                    )
                nc.scalar.activation(s2T_bf[:, sl], s2T_ps[:, sl],
                                     AF.Tanh, scale=INV13)

            def s2_stage(t):
                with tc.tile_pool(name=f"psT2{t}", bufs=2, space="PSUM") as p2:
                    for mb in range(4 * t, 4 * t + 4):
                        tp = p2.tile([128, 128], BF16, name="tp2")
                        nc.tensor.transpose(
                            tp, s2T_bf[:, mb * 128 : (mb + 1) * 128], ident_bf
                        )
                        nc.scalar.activation(s2loc[:, mb], tp, AF.Copy)
                nc.scalar.dma_start(out=s2_ins[t][:, :, :],
                                    in_=s2loc[:, 4 * t : 4 * t + 4, :])
                nc.gpsimd.collective_compute(
                    "AllGather", mybir.AluOpType.bypass,
                    replica_groups=RG,
                    ins=[s2_ins[t].opt()], outs=[s2_outs[t].opt()],
                )

            # wave 3: h0 for both g, then the h0 s2-chain, then h1
            for h in range(2):
                for g in range(2):
                    for r in range(8):
                        r1_mm(3, r, g, h)
                s2_chain(h)
                s2_stage(h)   # half t = h here: t0 = mb 0..3 needs s2T h0
            s2T_psx.close()

        # ================= merged tail: phases B + C + D ===============
        with tc.tile_pool(name="tail", bufs=1) as tl, \
             tc.tile_pool(name="psG", bufs=1, space="PSUM") as psG:
            aTs = [None, None, None]
            sqs = [None, None, None]
            embs_bf = [zf_bf, com_bf, zs_bf]

            def emit_attn_view(v, hs=(0, 1)):
                if aTs[v] is None:
                    aTs[v] = tl.tile([64, NL], F32, name=f"aT{v}")
                    sqs[v] = tl.tile([64, NL], F32, name=f"sq{v}")
                a_ps = psG.tile([64, NL], F32, name="aps", bufs=1)
                for h in hs:
                    sl = slice(h * 512, (h + 1) * 512)
                    nc.tensor.matmul(a_ps[:, sl], lhsT=wlW_sb,
                                     rhs=embs_bf[v][:, sl])
                    nc.vector.tensor_scalar_add(aTs[v][:, sl], a_ps[:, sl],
                                                wl_b_sb)
                    # square folds the wl_b bias: (a_ps + b)^2 on ACT
                    nc.scalar.activation(sqs[v][:, sl], a_ps[:, sl],
                                         AF.Square, bias=wl_b_sb)

            # com-independent attention views fill the AG-s2 gap
            emit_attn_view(0)
            emit_attn_view(2)
            sq02 = tl.tile([64, NL], F32)
            nc.gpsimd.tensor_add(sq02, sqs[0], sqs[2])

            # warmers through the AG-s2 mesh wait
            with tc.tile_pool(name="psW2", bufs=1, space="PSUM") as psW2:
                wps2 = psW2.tile([128, 512], F32)
                for w in range(14):
                    nc.tensor.matmul(
                        wps2, lhsT=conT8[:, 0:2, 0:128],
                        rhs=conT8[:, 2:4, 0:512],
                        start=True, stop=True, perf_mode=DR,
                    )

            # ---- round 2 (z2 = conT8-dom @ s2), 2 AG waves ----
            z2sb = tl.tile([128, NL], BF16)
            s3T_bf = tl.tile([64, NL], BF16)
            s3loc = tl.tile([128, 8, Z], F8)
            s2f = tl.tile([128, 8, 8, H2], F8)
            with tc.tile_pool(name="psD", bufs=1, space="PSUM") as psD, \
                 tc.tile_pool(name="psE", bufs=1, space="PSUM") as psE:
                z2_ps = psD.tile([128, NL], F32)
                s3T_ps = psE.tile([64, NL], F32)
                for t in range(2):
                    nc.sync.dma_start(
                        out=s2f[:, :, 4 * t : 4 * t + 4, :],
                        in_=s2_outs[t].rearrange("(r p) a c -> p r a c", p=128),
                    )

                def r2_mm(t, r, a0, h):
                    sl = slice(h * 512, (h + 1) * 512)
                    k = 8 * r + a0
                    nc.tensor.matmul(
                        z2_ps[:, sl],
                        lhsT=s2f[:, r, a0 : a0 + 2, :],
                        rhs=conT8[:, k : k + 2, sl],
                        start=(t == 0 and r == 0 and a0 == 0),
                        stop=(t == 1 and r == 7 and a0 == 6),
                        perf_mode=DR,
                    )

                for r in range(8):
                    for a0 in (0, 2):
                        for h in range(2):
                            r2_mm(0, r, a0, h)

                def s3_chain(h):
                    sl = slice(h * 512, (h + 1) * 512)
                    nc.vector.tensor_copy(z2sb[:, sl], z2_ps[:, sl])
                    nc.tensor.matmul(s3T_ps[:, sl], lhsT=W3_sb,
                                     rhs=z2sb[:, sl])
                    nc.scalar.activation(s3T_bf[:, sl], s3T_ps[:, sl],
                                         AF.Copy, scale=INV13)

                def s3_stage(t):
                    with tc.tile_pool(name=f"psT3{t}", bufs=2,
                                      space="PSUM") as p3:
                        for mb in range(4 * t, 4 * t + 4):
                            tp = p3.tile([128, 64], BF16, name="tp3")
                            nc.tensor.transpose(
                                tp, s3T_bf[:, mb * 128 : (mb + 1) * 128],
                                ident_bf[:64, :64],
                            )
                            nc.scalar.activation(s3loc[:, mb], tp, AF.Copy)
                    nc.scalar.dma_start(out=s3_ins[t][:, :, :],
                                        in_=s3loc[:, 4 * t : 4 * t + 4, :])
                    nc.gpsimd.collective_compute(
                        "AllGather", mybir.AluOpType.bypass,
                        replica_groups=RG,
                        ins=[s3_ins[t].opt()], outs=[s3_outs[t].opt()],
                    )

                # wave 1: h0 first, then the h0 s3-chain, then h1
                for h in range(2):
                    for r in range(8):
                        for a0 in (4, 6):
                            r2_mm(1, r, a0, h)
                    s3_chain(h)
                    s3_stage(h)

            # warmers through the AG-s3 mesh wait
            with tc.tile_pool(name="psW3", bufs=1, space="PSUM") as psW3:
                wps3 = psW3.tile([128, 512], F32)
                for w in range(10):
                    nc.tensor.matmul(
                        wps3, lhsT=conT8[:, 0:2, 0:128],
                        rhs=conT8[:, 2:4, 0:512],
                        start=True, stop=True, perf_mode=DR,
                    )

            # ---- round 3 (com = conT8-dom @ s3) + pipelined fusion ----
            s3f = tl.tile([128, 8, 8, Z], F8)
            sq = tl.tile([64, NL], F32)
            nrm = tl.tile([64, NL], F32)
            rec = tl.tile([64, NL], F32)
            us = [tl.tile([64, NL], BF16, name=f"u{v}") for v in range(3)]
            uts = [tl.tile([64, NL], F32, name=f"ut{v}") for v in range(3)]
            outT_sb = tl.tile([64, NL], F32)

            with tc.tile_pool(name="psF", bufs=1, space="PSUM") as psF:
                com_ps = psF.tile([64, NL], F32)
                out_ps = psF.tile([64, NL], F32, name="ops")
                for t in range(2):
                    nc.sync.dma_start(
                        out=s3f[:, :, 4 * t : 4 * t + 4, :],
                        in_=s3_outs[t].rearrange("(r p) a c -> p r a c", p=128),
                    )

                def r3_mm(t, r, a0, h):
                    sl = slice(h * 512, (h + 1) * 512)
                    k = 8 * r + a0
                    nc.tensor.matmul(
                        com_ps[:, sl],
                        lhsT=s3f[:, r, a0 : a0 + 2, :],
                        rhs=conT8[:, k : k + 2, sl],
                        start=(t == 0 and r == 0 and a0 == 0),
                        stop=(t == 1 and r == 7 and a0 == 6),
                        perf_mode=DR,
                    )

                for r in range(8):
                    for a0 in (0, 2):
                        for h in range(2):
                            r3_mm(0, r, a0, h)

                def fuse_half(h):
                    sl = slice(h * 512, (h + 1) * 512)
                    # com lands in bf16 via the ACT copy (scale folds 2^-13)
                    nc.scalar.activation(com_bf[:, sl], com_ps[:, sl],
                                         AF.Copy, scale=INV13)
                    emit_attn_view(1, hs=(h,))
                    nc.vector.tensor_add(sq[:, sl], sq02[:, sl],
                                         sqs[1][:, sl])
                    nc.scalar.activation(nrm[:, sl], sq[:, sl], AF.Sqrt)
                    nc.vector.reciprocal_approx_fast(rec[:, sl], nrm[:, sl])
                    for v, eng in ((1, nc.vector), (0, nc.gpsimd),
                                   (2, nc.gpsimd)):
                        eng.tensor_mul(uts[v][:, sl], aTs[v][:, sl],
                                       rec[:, sl])
                        eng.tensor_mul(us[v][:, sl], uts[v][:, sl],
                                       embs_bf[v][:, sl])
                    for vi, v in enumerate((1, 0, 2)):
                        nc.tensor.matmul(
                            out_ps[:, sl], lhsT=mlpW_sb[:, v],
                            rhs=us[v][:, sl],
                            start=(vi == 0), stop=(vi == 2),
                        )
                    nc.vector.tensor_scalar_add(outT_sb[:, sl],
                                                out_ps[:, sl], mlp_b_sb)
                    nc.sync.dma_start(out=outT[:, sl], in_=outT_sb[:, sl])

                # wave 1: h0 first, fuse h0 while h1 finishes
                for h in range(2):
                    for r in range(8):
                        for a0 in (4, 6):
                            r3_mm(1, r, a0, h)
                    fuse_half(h)


_CACHE = {}


def _build():
    if "nc" in _CACHE:
        return _CACHE["nc"]
    nc = bacc.Bacc("TRN2", target_bir_lowering=False, debug=False,
                   num_devices=N_CORES)
    io = {
        "adjT_f": nc.dram_tensor("adjT_f", [N, NL], F8, kind="ExternalInput"),
        "adjT_s": nc.dram_tensor("adjT_s", [N, NL], F8, kind="ExternalInput"),
        "xT": nc.dram_tensor("xT", [D_IN, NL], F8, kind="ExternalInput"),
        "zfT": nc.dram_tensor("zfT", [Z, NL], F32, kind="ExternalInput"),
        "zsT": nc.dram_tensor("zsT", [Z, NL], F32, kind="ExternalInput"),
        "W1": nc.dram_tensor("W1", [D_IN, H1], F8, kind="ExternalInput"),
        "W2": nc.dram_tensor("W2", [H1, H2], F32, kind="ExternalInput"),
        "W3": nc.dram_tensor("W3", [H2, Z], F32, kind="ExternalInput"),
        "wl_W": nc.dram_tensor("wl_W", [Z, Z], F32, kind="ExternalInput"),
        "mlp_W": nc.dram_tensor("mlp_W", [3 * Z, Z], F32, kind="ExternalInput"),
        "wl_b": nc.dram_tensor("wl_b", [Z], F32, kind="ExternalInput"),
        "mlp_b": nc.dram_tensor("mlp_b", [Z], F32, kind="ExternalInput"),
        "outT": nc.dram_tensor("outT", [Z, NL], F32, kind="ExternalOutput"),
    }
    with tile.TileContext(nc) as tc:
        _emit(nc, tc, io)
    nc.compile()
    _CACHE["nc"] = nc
    return nc


def _shard_inputs(inputs):
    """Full inputs -> per-core input maps (host-side sharding only)."""
    f32 = np.float32
    adj_f = np.asarray(inputs["adj_feature"], f32)
    adj_s = np.asarray(inputs["adj_spatial"], f32)
    x = np.asarray(inputs["x"], f32)
    zf = np.asarray(inputs["z_feature"], f32)
    zs = np.asarray(inputs["z_spatial"], f32)
    meta = float(np.asarray(inputs["meta"], f32).reshape(-1)[0])
    import ml_dtypes
    fp8 = ml_dtypes.float8_e4m3fn
    rep = {
        "W1": np.ascontiguousarray(np.asarray(inputs["W1"], f32).astype(fp8)),
        "W2": np.ascontiguousarray(np.asarray(inputs["W2"], f32)),
        "W3": np.ascontiguousarray(np.asarray(inputs["W3"], f32)),
        "wl_W": np.ascontiguousarray(np.asarray(inputs["wl_W"], f32)),
        "mlp_W": np.ascontiguousarray(np.asarray(inputs["mlp_W"], f32)),
        "wl_b": np.ascontiguousarray(np.asarray(inputs["wl_b"], f32)),
        "mlp_b": np.ascontiguousarray(np.asarray(inputs["mlp_b"], f32)),
    }
    # fp8 staging with a fixed 2^13 exponent shift; the meta /(1-meta)
    # blend weights fold into the per-matrix quantization scales so the
    # on-device blend is a pure add (ridden by the DMA CCE adder).
    adj_fT8 = (adj_f.T * (8192.0 * meta)).astype(fp8)
    adj_sT8 = (adj_s.T * (8192.0 * (1.0 - meta))).astype(fp8)
    xT = np.ascontiguousarray(x.T)
    zfT = np.ascontiguousarray(zf.T)
    zsT = np.ascontiguousarray(zs.T)
    in_maps = []
    for i in range(N_CORES):
        r = slice(NL * i, NL * (i + 1))
        m = {
            "adjT_f": np.ascontiguousarray(adj_fT8[:, r]),
            "adjT_s": np.ascontiguousarray(adj_sT8[:, r]),
            "xT": np.ascontiguousarray(xT[:, r]).astype(fp8),
            "zfT": np.ascontiguousarray(zfT[:, r]),
            "zsT": np.ascontiguousarray(zsT[:, r]),
        }
        m.update(rep)
        in_maps.append(m)
    return in_maps


def run(trace=False, **inputs):
    nc = _build()
    in_maps = _shard_inputs(inputs)
    res = run_bass_kernel_spmd(nc, in_maps, list(range(N_CORES)), trace=trace)
    out = np.concatenate(
        [np.asarray(res.results[i]["outT"]).T for i in range(N_CORES)], axis=0
    ).astype(np.float32)
    return out, res


def kernel(**inputs):
    out, _ = run(trace=False, **inputs)
    return out


# revision 28
# speedup vs baseline: 1.1008x; 1.0191x over previous
"""Trainium2 Bass kernel for nn_CAM (GNN message passing, 8-core SPMD).

Strategy (per core i of 8, owning node rows R_i = [1024*i, 1024*(i+1))):
  - Host ships the TRANSPOSED column-block of each adjacency as
    fp8_e4m3 pre-scaled by meta*2^13 (feature) and (1-meta)*2^13
    (spatial).  The blend  con = meta*A_f + (1-meta)*A_s  then reduces
    to a pure ADD, which rides the DMA engines' inline CCE ALU:
    the spatial slab DMAs land in the resident conT8 tile and the
    feature slab DMAs accumulate into it (gpsimd SWDGE, accum_op=add).
    No vector-engine blend pass at all; the 2^-13 fold-back is a
    compile-time ACT scale.
  - All three adj@support rounds run as fp8 DoubleRow matmuls in the
    transposed [h, 1024] domain against the resident conT8.
  - Support matrices are exchanged across cores via AllGather bounced
    through shared DRAM.  s1 goes as FOUR quarter-AGs (Mesh algorithm,
    lower latency than RDH halves) consumed by four round-1 waves; s2
    and s3 as two half-AGs each.  The final wave of each round is
    ordered h0-before-h1 so the next support chain and its first AG
    trigger fire at the ~75% point of the round.
  - Discarded warmer matmuls bridge every collective wait so the PE's
    HAM clock gate stays released (cold PE runs at 1.2 GHz vs 2.4).
  - The attention fusion runs in the transposed [64, 1024] domain;
    com-independent views compute inside the AG-s2 window and the
    com-dependent tail is pipelined in two 512-column halves so the
    output DMA starts as soon as the first half of round 3 closes.
"""

import sys

if "/opt/trn_rl_repo" not in sys.path:
    sys.path.insert(0, "/opt/trn_rl_repo")

from contextlib import ExitStack

import numpy as np

import concourse.bass as bass
import concourse.tile as tile
from concourse import bacc, mybir
from concourse.bass_utils import run_bass_kernel_spmd
from concourse.masks import make_identity


F32 = mybir.dt.float32
BF16 = mybir.dt.bfloat16
F8 = mybir.dt.float8e4
AF = mybir.ActivationFunctionType
DR = mybir.MatmulPerfMode.DoubleRow
MUL = mybir.AluOpType.mult
ADD = mybir.AluOpType.add

N = 8192
D_IN = 3000
H1, H2, Z = 256, 128, 64
N_CORES = 8
NL = N // N_CORES           # 1024 local nodes per core
KC = N // 128               # 64 contraction chunks of 128
XC = (D_IN + 127) // 128    # 24 x-feature chunks (zero-padded to 3072)
XB = 8                      # x chunks per batched DMA
RG = [list(range(N_CORES))]
INV13 = 1.0 / 8192.0        # 2^-13 fold-back for the fp8 exponent shift

USE_ACCUM_BLEND = False     # blend via DMA CCE add (else DVE/Pool adds)
NSLAB = 16                  # adjacency stream slabs (KC/NSLAB chunks each)
SCH = KC // NSLAB


def _emit(nc, tc, io):
    adjT_f, adjT_s, xT = io["adjT_f"], io["adjT_s"], io["xT"]
    zfT, zsT = io["zfT"], io["zsT"]
    W1, W2, W3 = io["W1"], io["W2"], io["W3"]
    wl_W, mlp_W = io["wl_W"], io["mlp_W"]
    wl_b, mlp_b = io["wl_b"], io["mlp_b"]
    outT = io["outT"]

    ctx = ExitStack()
    with ctx:
        const = ctx.enter_context(tc.tile_pool(name="const", bufs=1))
        dram = ctx.enter_context(tc.tile_pool(name="dram", bufs=1, space="DRAM"))

        ident_bf = const.tile([128, 128], BF16)
        make_identity(nc, ident_bf)
        ones_sb = const.tile([128, 1], F32)
        nc.vector.memset(ones_sb, 1.0)
        wl_b_sb = const.tile([64, 1], F32)
        nc.gpsimd.dma_start(out=wl_b_sb, in_=wl_b[:, None])
        mlp_b_sb = const.tile([64, 1], F32)
        nc.gpsimd.dma_start(out=mlp_b_sb, in_=mlp_b[:, None])
        W2_sb = const.tile([128, 2, H2], BF16)
        nc.gpsimd.dma_start(out=W2_sb, in_=W2.rearrange("(b k) c -> k b c", b=2))
        W3_sb = const.tile([128, Z], BF16)
        nc.gpsimd.dma_start(out=W3_sb, in_=W3[:, :])

        # resident blended adjacency (transposed, fp8, x2^13)
        conT8 = const.tile([128, KC, NL], F8)
        z1sb = const.tile([128, 2, NL], BF16)

        # attention operands live across phases -> const pool
        com_bf = const.tile([64, NL], BF16)
        zf_bf = const.tile([64, NL], BF16)
        zs_bf = const.tile([64, NL], BF16)
        wlW_sb = const.tile([64, 64], BF16)
        mlpW_sb = const.tile([64, 3, 64], BF16)

        # AG bounce buffers (fp8); s1 as four quarters, s2/s3 as halves
        s1_ins = [dram.tile([128, 2, H1], F8, name=f"s1in{t}") for t in range(4)]
        s1_outs = [dram.tile([NL, 2, H1], F8, addr_space="Shared",
                             name=f"s1out{t}") for t in range(4)]
        s2_ins = [dram.tile([128, 4, H2], F8, name=f"s2in{t}") for t in range(2)]
        s2_outs = [dram.tile([NL, 4, H2], F8, addr_space="Shared",
                             name=f"s2out{t}") for t in range(2)]
        s3_ins = [dram.tile([128, 4, Z], F8, name=f"s3in{t}") for t in range(2)]
        s3_outs = [dram.tile([NL, 4, Z], F8, addr_space="Shared",
                             name=f"s3out{t}") for t in range(2)]

        # ======== phase A: stream+blend, s1 chain, round 1 ============
        with tc.tile_pool(name="phaseA", bufs=1) as pA, \
             tc.tile_pool(name="psZ", bufs=1, space="PSUM") as psZ:
            s1T_bf = pA.tile([128, 2, NL], BF16)
            s1loc = pA.tile([128, 8, H1], F8)
            s1f = pA.tile([128, 8, 8, H1], F8)

            psA_ctx = ExitStack()
            psA = psA_ctx.enter_context(
                tc.tile_pool(name="psA", bufs=1, space="PSUM")
            )
            s1T_ps = [psA.tile([128, NL], F32, name=f"s1T{g}") for g in range(2)]

            # ---- batched fp8 x/W1 loads (host pre-interleaved so every
            # DMA is per-partition contiguous; x padded to 24 chunks) ----
            xbigs, w1bigs = [], []
            for b in range(3):
                eng = nc.sync if b < 2 else nc.scalar
                xb = pA.tile([128, XB, NL], F8, name="xbig", bufs=3)
                eng.dma_start(
                    out=xb, in_=xT[:, b * XB * NL : (b + 1) * XB * NL]
                )
                xbigs.append(xb)
                wb = pA.tile([128, XB, H1], F8, name="w1big", bufs=3)
                eng.dma_start(
                    out=wb, in_=W1[:, b * XB * H1 : (b + 1) * XB * H1]
                )
                w1bigs.append(wb)

            # ---- adjacency stream + blend ----
            # both pre-scaled matrices are staged (ALL on the sync ring:
            # the scalar ring must stay clear so the s1 chain is not
            # queued behind adjacency issue stalls) and summed into the
            # resident conT8 (out not aliased with inputs).  Early slabs
            # blend on Pool so its queue is clear when the collective
            # triggers need to fire; late slabs go to the vector engine.
            for j in range(NSLAB):
                r0, r1 = j * SCH * 128, (j + 1) * SCH * 128
                dst = conT8[:, j * SCH : (j + 1) * SCH, :]
                asl = pA.tile([128, SCH, NL], F8, name="asl", bufs=6)
                nc.sync.dma_start(
                    out=asl, in_=adjT_s[:, r0 * 8 : r1 * 8]
                )
                af = pA.tile([128, SCH, NL], F8, name="af", bufs=6)
                nc.sync.dma_start(
                    out=af, in_=adjT_f[:, r0 * 8 : r1 * 8]
                )
                if j < 8:
                    nc.gpsimd.tensor_add(dst, af, asl)
                else:
                    nc.vector.scalar_tensor_tensor(
                        out=dst, in0=af, scalar=ones_sb, in1=asl,
                        op0=MUL, op1=ADD,
                    )

            # attention inputs (small; gpsimd SWDGE casts f32->bf16)
            nc.gpsimd.dma_start(out=zf_bf, in_=zfT[:, :])
            nc.gpsimd.dma_start(out=zs_bf, in_=zsT[:, :])
            nc.gpsimd.dma_start(out=wlW_sb, in_=wl_W[:, :])
            nc.gpsimd.dma_start(
                out=mlpW_sb, in_=mlp_W.rearrange("(v c) d -> c v d", v=3)
            )

            # ---- s1 = tanh(xT.T @ W1) in the transposed domain ----
            # early identity matmuls release the HAM clock gate before
            # the real s1 stream arrives; they scribble on s1T_ps, which
            # the first real accumulation (start=True) resets anyway
            for _ in range(24):
                nc.tensor.matmul(s1T_ps[0][:, 0:128], lhsT=ident_bf,
                                 rhs=ident_bf, start=True, stop=True)

            for kp2 in range(12):
                b, a = (2 * kp2) // XB, (2 * kp2) % XB
                for g in range(2):
                    for h in range(2):
                        nc.tensor.matmul(
                            s1T_ps[g][:, h * 512 : (h + 1) * 512],
                            lhsT=w1bigs[b][:, a : a + 2,
                                           g * 128 : (g + 1) * 128],
                            rhs=xbigs[b][:, a : a + 2,
                                         h * 512 : (h + 1) * 512],
                            start=(kp2 == 0),
                            stop=(kp2 == 11),
                            perf_mode=DR,
                        )

            # ---- s1 chain: tanh, transpose, stage quarter-AGs ----
            for g in range(2):
                nc.scalar.activation(s1T_bf[:, g], s1T_ps[g], AF.Tanh)
            psA_ctx.close()
            z1_ps = [psZ.tile([128, NL], F32, name=f"z1g{g}") for g in range(2)]
            with tc.tile_pool(name="psT", bufs=2, space="PSUM") as psT:
                for q in range(4):
                    for mb in (2 * q, 2 * q + 1):
                        for g in range(2):
                            tp = psT.tile([128, 128], BF16, name="tp")
                            nc.tensor.transpose(
                                tp,
                                s1T_bf[:, g, mb * 128 : (mb + 1) * 128],
                                ident_bf,
                            )
                            nc.scalar.activation(
                                s1loc[:, mb, g * 128 : (g + 1) * 128],
                                tp, AF.Copy,
                            )
                    nc.scalar.dma_start(out=s1_ins[q][:, :, :],
                                        in_=s1loc[:, 2 * q : 2 * q + 2, :])
                    nc.gpsimd.collective_compute(
                        "AllGather", mybir.AluOpType.bypass,
                        replica_groups=RG,
                        ins=[s1_ins[q].opt()], outs=[s1_outs[q].opt()],
                    )

            # warmers bridging the collectives-init barrier window so
            # round 1 opens at full clock
            with tc.tile_pool(name="psW1", bufs=1, space="PSUM") as psW1:
                wps = psW1.tile([128, 512], F32)
                for w in range(24):
                    nc.tensor.matmul(
                        wps, lhsT=w1bigs[0][:, 0:2, 0:128],
                        rhs=xbigs[0][:, 0:2, 0:512],
                        start=True, stop=True, perf_mode=DR,
                    )

            # ---- round 1: z1 = conT8.T-domain matmuls in 4 AG waves;
            # final wave ordered h0-first so the s2 chain starts early.
            for q in range(4):
                nc.sync.dma_start(
                    out=s1f[:, :, 2 * q : 2 * q + 2, :],
                    in_=s1_outs[q].rearrange("(r p) a c -> p r a c", p=128),
                )

            def r1_mm(q, r, g, h):
                sl = slice(h * 512, (h + 1) * 512)
                k = 8 * r + 2 * q
                nc.tensor.matmul(
                    z1_ps[g][:, sl],
                    lhsT=s1f[:, r, 2 * q : 2 * q + 2,
                             g * 128 : (g + 1) * 128],
                    rhs=conT8[:, k : k + 2, sl],
                    start=(q == 0 and r == 0),
                    stop=(q == 3 and r == 7),
                    perf_mode=DR,
                )

            for q in range(3):
                for r in range(8):
                    for g in range(2):
                        for h in range(2):
                            r1_mm(q, r, g, h)

            s2T_bf = pA.tile([128, NL], BF16)
            s2loc = pA.tile([128, 8, H2], F8)
            s2T_psx = ExitStack()
            psC = s2T_psx.enter_context(
                tc.tile_pool(name="psC", bufs=1, space="PSUM"))
            s2T_ps = psC.tile([128, NL], F32)

            def s2_chain(h):
                # z1sb copies ride DVE+ACT; the W2 matmuls interleave on
                # the PE behind the h1 wave
                sl = slice(h * 512, (h + 1) * 512)
                nc.vector.tensor_copy(z1sb[:, 0, sl], z1_ps[0][:, sl])
                nc.scalar.copy(z1sb[:, 1, sl], z1_ps[1][:, sl])
                for b in range(2):
                    nc.tensor.matmul(
                        s2T_ps[:, sl], lhsT=W2_sb[:, b], rhs=z1sb[:, b, sl],
                        start=(b == 0), stop=(b == 1),
                    )
                nc.scalar.activation(s2T_bf[:, sl], s2T_ps[:, sl],
                                     AF.Tanh, scale=INV13)

            def s2_stage(t):
                with tc.tile_pool(name=f"psT2{t}", bufs=2, space="PSUM") as p2:
                    for mb in range(4 * t, 4 * t + 4):
                        tp = p2.tile([128, 128], BF16, name="tp2")
                        nc.tensor.transpose(
                            tp, s2T_bf[:, mb * 128 : (mb + 1) * 128], ident_bf
                        )
                        nc.scalar.activation(s2loc[:, mb], tp, AF.Copy)
                nc.scalar.dma_start(out=s2_ins[t][:, :, :],
                                    in_=s2loc[:, 4 * t : 4 * t + 4, :])
                nc.gpsimd.collective_compute(
                    "AllGather", mybir.AluOpType.bypass,
                    replica_groups=RG,
                    ins=[s2_ins[t].opt()], outs=[s2_outs[t].opt()],
                )

            # wave 3: h0 for both g, then the h0 s2-chain, then h1
            for h in range(2):
                for g in range(2):
                    for r in range(8):
                        r1_mm(3, r, g, h)
                s2_chain(h)
                s2_stage(h)   # half t = h here: t0 = mb 0..3 needs s2T h0
            s2T_psx.close()

        # ================= merged tail: phases B + C + D ===============
        with tc.tile_pool(name="tail", bufs=1) as tl, \
             tc.tile_pool(name="psG", bufs=1, space="PSUM") as psG:
            aTs = [None, None, None]
            sqs = [None, None, None]
            embs_bf = [zf_bf, com_bf, zs_bf]

            def emit_attn_view(v, hs=(0, 1)):
                if aTs[v] is None:
                    aTs[v] = tl.tile([64, NL], F32, name=f"aT{v}")
                    sqs[v] = tl.tile([64, NL], F32, name=f"sq{v}")
                a_ps = psG.tile([64, NL], F32, name="aps", bufs=1)
                for h in hs:
                    sl = slice(h * 512, (h + 1) * 512)
                    nc.tensor.matmul(a_ps[:, sl], lhsT=wlW_sb,
                                     rhs=embs_bf[v][:, sl])
                    nc.vector.tensor_scalar_add(aTs[v][:, sl], a_ps[:, sl],
                                                wl_b_sb)
                    # square folds the wl_b bias: (a_ps + b)^2 on ACT
                    nc.scalar.activation(sqs[v][:, sl], a_ps[:, sl],
                                         AF.Square, bias=wl_b_sb)

            # com-independent attention views fill the AG-s2 gap
            emit_attn_view(0)
            emit_attn_view(2)
            sq02 = tl.tile([64, NL], F32)
            nc.gpsimd.tensor_add(sq02, sqs[0], sqs[2])

            # warmers through the AG-s2 mesh wait
            with tc.tile_pool(name="psW2", bufs=1, space="PSUM") as psW2:
                wps2 = psW2.tile([128, 512], F32)
                for w in range(14):
                    nc.tensor.matmul(
                        wps2, lhsT=conT8[:, 0:2, 0:128],
                        rhs=conT8[:, 2:4, 0:512],
                        start=True, stop=True, perf_mode=DR,
                    )

            # ---- round 2 (z2 = conT8-dom @ s2), 2 AG waves ----
            z2sb = tl.tile([128, NL], BF16)
            s3T_bf = tl.tile([64, NL], BF16)
            s3loc = tl.tile([128, 8, Z], F8)
            s2f = tl.tile([128, 8, 8, H2], F8)
            with tc.tile_pool(name="psD", bufs=1, space="PSUM") as psD, \
                 tc.tile_pool(name="psE", bufs=1, space="PSUM") as psE:
                z2_ps = psD.tile([128, NL], F32)
                s3T_ps = psE.tile([64, NL], F32)
                for t in range(2):
                    nc.sync.dma_start(
                        out=s2f[:, :, 4 * t : 4 * t + 4, :],
                        in_=s2_outs[t].rearrange("(r p) a c -> p r a c", p=128),
                    )

                def r2_mm(t, r, a0, h):
                    sl = slice(h * 512, (h + 1) * 512)
                    k = 8 * r + a0
                    nc.tensor.matmul(
                        z2_ps[:, sl],
                        lhsT=s2f[:, r, a0 : a0 + 2, :],
                        rhs=conT8[:, k : k + 2, sl],
                        start=(t == 0 and r == 0 and a0 == 0),
                        stop=(t == 1 and r == 7 and a0 == 6),
                        perf_mode=DR,
                    )

                for r in range(8):
                    for a0 in (0, 2):
                        for h in range(2):
                            r2_mm(0, r, a0, h)

                def s3_chain(h):
                    sl = slice(h * 512, (h + 1) * 512)
                    nc.vector.tensor_copy(z2sb[:, sl], z2_ps[:, sl])
                    nc.tensor.matmul(s3T_ps[:, sl], lhsT=W3_sb,
                                     rhs=z2sb[:, sl])
                    nc.scalar.activation(s3T_bf[:, sl], s3T_ps[:, sl],
                                         AF.Copy, scale=INV13)

                def s3_stage(t):
                    with tc.tile_pool(name=f"psT3{t}", bufs=2,
                                      space="PSUM") as p3:
                        for mb in range(4 * t, 4 * t + 4):
                            tp = p3.tile([128, 64], BF16, name="tp3")
                            nc.tensor.transpose(
                                tp, s3T_bf[:, mb * 128 : (mb + 1) * 128],
                                ident_bf[:64, :64],
                            )
                            nc.scalar.activation(s3loc[:, mb], tp, AF.Copy)
                    nc.scalar.dma_start(out=s3_ins[t][:, :, :],
                                        in_=s3loc[:, 4 * t : 4 * t + 4, :])
                    nc.gpsimd.collective_compute(
                        "AllGather", mybir.AluOpType.bypass,
                        replica_groups=RG,
                        ins=[s3_ins[t].opt()], outs=[s3_outs[t].opt()],
                    )

                # wave 1: h0 first, then the h0 s3-chain, then h1
                for h in range(2):
                    for r in range(8):
                        for a0 in (4, 6):
                            r2_mm(1, r, a0, h)
                    s3_chain(h)
                    s3_stage(h)

            # warmers through the AG-s3 mesh wait
            with tc.tile_pool(name="psW3", bufs=1, space="PSUM") as psW3:
                wps3 = psW3.tile([128, 512], F32)
                for w in range(10):
                    nc.tensor.matmul(
                        wps3, lhsT=conT8[:, 0:2, 0:128],
                        rhs=conT8[:, 2:4, 0:512],
                        start=True, stop=True, perf_mode=DR,
                    )

            # ---- round 3 (com = conT8-dom @ s3) + pipelined fusion ----
            s3f = tl.tile([128, 8, 8, Z], F8)
            sq = tl.tile([64, NL], F32)
            nrm = tl.tile([64, NL], F32)
            rec = tl.tile([64, NL], F32)
            us = [tl.tile([64, NL], BF16, name=f"u{v}") for v in range(3)]
            uts = [tl.tile([64, NL], F32, name=f"ut{v}") for v in range(3)]
            outT_sb = tl.tile([64, NL], F32)

            with tc.tile_pool(name="psF", bufs=1, space="PSUM") as psF:
                com_ps = psF.tile([64, NL], F32)
                out_ps = psF.tile([64, NL], F32, name="ops")
                for t in range(2):
                    nc.sync.dma_start(
                        out=s3f[:, :, 4 * t : 4 * t + 4, :],
                        in_=s3_outs[t].rearrange("(r p) a c -> p r a c", p=128),
                    )

                def r3_mm(t, r, a0, h):
                    sl = slice(h * 512, (h + 1) * 512)
                    k = 8 * r + a0
                    nc.tensor.matmul(
                        com_ps[:, sl],
                        lhsT=s3f[:, r, a0 : a0 + 2, :],
                        rhs=conT8[:, k : k + 2, sl],
                        start=(t == 0 and r == 0 and a0 == 0),
                        stop=(t == 1 and r == 7 and a0 == 6),
                        perf_mode=DR,
                    )

                for r in range(8):
                    for a0 in (0, 2):
                        for h in range(2):
                            r3_mm(0, r, a0, h)

                def fuse_half(h):
                    sl = slice(h * 512, (h + 1) * 512)
                    # com lands in bf16 via the ACT copy (scale folds 2^-13)
                    nc.scalar.activation(com_bf[:, sl], com_ps[:, sl],
                                         AF.Copy, scale=INV13)
                    emit_attn_view(1, hs=(h,))
                    nc.vector.tensor_add(sq[:, sl], sq02[:, sl],
                                         sqs[1][:, sl])
                    nc.scalar.activation(nrm[:, sl], sq[:, sl], AF.Sqrt)
                    nc.vector.reciprocal_approx_fast(rec[:, sl], nrm[:, sl])
                    for v, eng in ((1, nc.vector), (0, nc.gpsimd),
                                   (2, nc.gpsimd)):
                        eng.tensor_mul(uts[v][:, sl], aTs[v][:, sl],
                                       rec[:, sl])
                        eng.tensor_mul(us[v][:, sl], uts[v][:, sl],
                                       embs_bf[v][:, sl])
                    for vi, v in enumerate((1, 0, 2)):
                        nc.tensor.matmul(
                            out_ps[:, sl], lhsT=mlpW_sb[:, v],
                            rhs=us[v][:, sl],
                            start=(vi == 0), stop=(vi == 2),
                        )
                    nc.vector.tensor_scalar_add(outT_sb[:, sl],
                                                out_ps[:, sl], mlp_b_sb)
                    nc.sync.dma_start(out=outT[:, sl], in_=outT_sb[:, sl])

                # wave 1: h0 first, fuse h0 while h1 finishes
                for h in range(2):
                    for r in range(8):
                        for a0 in (4, 6):
                            r3_mm(1, r, a0, h)
                    fuse_half(h)


_CACHE = {}


def _build():
    if "nc" in _CACHE:
        return _CACHE["nc"]
    nc = bacc.Bacc("TRN2", target_bir_lowering=False, debug=False,
                   num_devices=N_CORES)
    io = {
        "adjT_f": nc.dram_tensor("adjT_f", [128, KC * NL], F8,
                                 kind="ExternalInput"),
        "adjT_s": nc.dram_tensor("adjT_s", [128, KC * NL], F8,
                                 kind="ExternalInput"),
        "xT": nc.dram_tensor("xT", [128, XC * NL], F8, kind="ExternalInput"),
        "zfT": nc.dram_tensor("zfT", [Z, NL], F32, kind="ExternalInput"),
        "zsT": nc.dram_tensor("zsT", [Z, NL], F32, kind="ExternalInput"),
        "W1": nc.dram_tensor("W1", [128, XC * H1], F8, kind="ExternalInput"),
        "W2": nc.dram_tensor("W2", [H1, H2], F32, kind="ExternalInput"),
        "W3": nc.dram_tensor("W3", [H2, Z], F32, kind="ExternalInput"),
        "wl_W": nc.dram_tensor("wl_W", [Z, Z], F32, kind="ExternalInput"),
        "mlp_W": nc.dram_tensor("mlp_W", [3 * Z, Z], F32, kind="ExternalInput"),
        "wl_b": nc.dram_tensor("wl_b", [Z], F32, kind="ExternalInput"),
        "mlp_b": nc.dram_tensor("mlp_b", [Z], F32, kind="ExternalInput"),
        "outT": nc.dram_tensor("outT", [Z, NL], F32, kind="ExternalOutput"),
    }
    with tile.TileContext(nc) as tc:
        _emit(nc, tc, io)
    nc.compile()
    _CACHE["nc"] = nc
    return nc


def _shard_inputs(inputs):
    """Full inputs -> per-core input maps (host-side sharding only)."""
    f32 = np.float32
    adj_f = np.asarray(inputs["adj_feature"], f32)
    adj_s = np.asarray(inputs["adj_spatial"], f32)
    x = np.asarray(inputs["x"], f32)
    zf = np.asarray(inputs["z_feature"], f32)
    zs = np.asarray(inputs["z_spatial"], f32)
    meta = float(np.asarray(inputs["meta"], f32).reshape(-1)[0])
    import ml_dtypes
    fp8 = ml_dtypes.float8_e4m3fn

    def interleave(mat, nch):
        """[nch*128, C] (zero-padded) -> [128, nch*C] per-partition
        contiguous chunk-major layout matching the SBUF tiles."""
        r, c = mat.shape
        pad = nch * 128 - r
        if pad:
            mat = np.concatenate(
                [mat, np.zeros((pad, c), mat.dtype)], axis=0)
        return np.ascontiguousarray(
            mat.reshape(nch, 128, c).transpose(1, 0, 2).reshape(128, nch * c))

    W1_8 = np.asarray(inputs["W1"], f32).astype(fp8)
    rep = {
        "W1": interleave(W1_8, XC),
        "W2": np.ascontiguousarray(np.asarray(inputs["W2"], f32)),
        "W3": np.ascontiguousarray(np.asarray(inputs["W3"], f32)),
        "wl_W": np.ascontiguousarray(np.asarray(inputs["wl_W"], f32)),
        "mlp_W": np.ascontiguousarray(np.asarray(inputs["mlp_W"], f32)),
        "wl_b": np.ascontiguousarray(np.asarray(inputs["wl_b"], f32)),
        "mlp_b": np.ascontiguousarray(np.asarray(inputs["mlp_b"], f32)),
    }
    # fp8 staging with a fixed 2^13 exponent shift; the meta /(1-meta)
    # blend weights fold into the per-matrix quantization scales so the
    # on-device blend is a pure add (ridden by the DMA CCE adder).
    adj_fT8 = (adj_f.T * (8192.0 * meta)).astype(fp8)
    adj_sT8 = (adj_s.T * (8192.0 * (1.0 - meta))).astype(fp8)
    xT = np.ascontiguousarray(x.T)
    zfT = np.ascontiguousarray(zf.T)
    zsT = np.ascontiguousarray(zs.T)
    in_maps = []
    for i in range(N_CORES):
        r = slice(NL * i, NL * (i + 1))
        m = {
            "adjT_f": interleave(np.ascontiguousarray(adj_fT8[:, r]), KC),
            "adjT_s": interleave(np.ascontiguousarray(adj_sT8[:, r]), KC),
            "xT": interleave(
                np.ascontiguousarray(xT[:, r]).astype(fp8), XC),
            "zfT": np.ascontiguousarray(zfT[:, r]),
            "zsT": np.ascontiguousarray(zsT[:, r]),
        }
        m.update(rep)
        in_maps.append(m)
    return in_maps


def run(trace=False, **inputs):
    nc = _build()
    in_maps = _shard_inputs(inputs)
    res = run_bass_kernel_spmd(nc, in_maps, list(range(N_CORES)), trace=trace)
    out = np.concatenate(
        [np.asarray(res.results[i]["outT"]).T for i in range(N_CORES)], axis=0
    ).astype(np.float32)
    return out, res


def kernel(**inputs):
    out, _ = run(trace=False, **inputs)
    return out


# revision 34
# speedup vs baseline: 1.1502x; 1.0449x over previous
"""Trainium2 Bass kernel for nn_CAM (GNN message passing, 8-core SPMD).

Strategy (per core i of 8, owning node rows R_i = [1024*i, 1024*(i+1))):
  - Host ships the TRANSPOSED column-block of each adjacency as
    fp8_e4m3 pre-scaled by meta*2^13 (feature) and (1-meta)*2^13
    (spatial).  The blend  con = meta*A_f + (1-meta)*A_s  then reduces
    to a pure ADD, which rides the DMA engines' inline CCE ALU:
    the spatial slab DMAs land in the resident conT8 tile and the
    feature slab DMAs accumulate into it (gpsimd SWDGE, accum_op=add).
    No vector-engine blend pass at all; the 2^-13 fold-back is a
    compile-time ACT scale.
  - All three adj@support rounds run as fp8 DoubleRow matmuls in the
    transposed [h, 1024] domain against the resident conT8.
  - Support matrices are exchanged across cores via AllGather bounced
    through shared DRAM.  s1 goes as FOUR quarter-AGs (Mesh algorithm,
    lower latency than RDH halves) consumed by four round-1 waves; s2
    and s3 as two half-AGs each.  The final wave of each round is
    ordered h0-before-h1 so the next support chain and its first AG
    trigger fire at the ~75% point of the round.
  - Discarded warmer matmuls bridge every collective wait so the PE's
    HAM clock gate stays released (cold PE runs at 1.2 GHz vs 2.4).
  - The attention fusion runs in the transposed [64, 1024] domain;
    com-independent views compute inside the AG-s2 window and the
    com-dependent tail is pipelined in two 512-column halves so the
    output DMA starts as soon as the first half of round 3 closes.
"""

import sys

if "/opt/trn_rl_repo" not in sys.path:
    sys.path.insert(0, "/opt/trn_rl_repo")

from contextlib import ExitStack

import numpy as np

import concourse.bass as bass
import concourse.tile as tile
from concourse import bacc, mybir
from concourse.bass_utils import run_bass_kernel_spmd
from concourse.masks import make_identity


F32 = mybir.dt.float32
BF16 = mybir.dt.bfloat16
F8 = mybir.dt.float8e4
AF = mybir.ActivationFunctionType
DR = mybir.MatmulPerfMode.DoubleRow
MUL = mybir.AluOpType.mult
ADD = mybir.AluOpType.add

N = 8192
D_IN = 3000
H1, H2, Z = 256, 128, 64
N_CORES = 8
NL = N // N_CORES           # 1024 local nodes per core
KC = N // 128               # 64 contraction chunks of 128
XC = (D_IN + 127) // 128    # 24 x-feature chunks (zero-padded to 3072)
XB = 8                      # x chunks per batched DMA
RG = [list(range(N_CORES))]
INV13 = 1.0 / 8192.0        # 2^-13 fold-back for the fp8 exponent shift

USE_ACCUM_BLEND = False     # blend via DMA CCE add (else DVE/Pool adds)
NSLAB = 16                  # adjacency stream slabs (KC/NSLAB chunks each)
SCH = KC // NSLAB


def _emit(nc, tc, io):
    adjT_f, adjT_s, xT = io["adjT_f"], io["adjT_s"], io["xT"]
    zfT, zsT = io["zfT"], io["zsT"]
    W1, W2, W3 = io["W1"], io["W2"], io["W3"]
    wl_W, mlp_W = io["wl_W"], io["mlp_W"]
    wl_b, mlp_b = io["wl_b"], io["mlp_b"]
    outT = io["outT"]

    ctx = ExitStack()
    with ctx:
        const = ctx.enter_context(tc.tile_pool(name="const", bufs=1))
        dram = ctx.enter_context(tc.tile_pool(name="dram", bufs=1, space="DRAM"))

        ident_bf = const.tile([128, 128], BF16)
        make_identity(nc, ident_bf)
        ones_sb = const.tile([128, 1], F32)
        nc.vector.memset(ones_sb, 1.0)
        wl_b_sb = const.tile([64, 1], F32)
        nc.gpsimd.dma_start(out=wl_b_sb, in_=wl_b[:, None])
        mlp_b_sb = const.tile([64, 1], F32)
        nc.gpsimd.dma_start(out=mlp_b_sb, in_=mlp_b[:, None])
        W2_sb = const.tile([128, 2, H2], BF16)
        nc.gpsimd.dma_start(out=W2_sb, in_=W2.rearrange("(b k) c -> k b c", b=2))
        W3_sb = const.tile([128, Z], BF16)
        nc.gpsimd.dma_start(out=W3_sb, in_=W3[:, :])

        # resident blended adjacency (transposed, fp8, x2^13)
        conT8 = const.tile([128, KC, NL], F8)
        z1sb = const.tile([128, 2, NL], BF16)

        # attention operands live across phases -> const pool
        com_bf = const.tile([64, NL], BF16)
        zf_bf = const.tile([64, NL], BF16)
        zs_bf = const.tile([64, NL], BF16)
        wlW_sb = const.tile([64, 64], BF16)
        mlpW_sb = const.tile([64, 3, 64], BF16)

        # AG bounce buffers (fp8); s1 as four quarters, s2/s3 as halves
        s1_ins = [dram.tile([128, 2, H1], F8, name=f"s1in{t}") for t in range(4)]
        s1_outs = [dram.tile([NL, 2, H1], F8, addr_space="Shared",
                             name=f"s1out{t}") for t in range(4)]
        s2_ins = [dram.tile([128, 4, H2], F8, name=f"s2in{t}") for t in range(2)]
        s2_outs = [dram.tile([NL, 4, H2], F8, addr_space="Shared",
                             name=f"s2out{t}") for t in range(2)]
        s3_ins = [dram.tile([128, 4, Z], F8, name=f"s3in{t}") for t in range(2)]
        s3_outs = [dram.tile([NL, 4, Z], F8, addr_space="Shared",
                             name=f"s3out{t}") for t in range(2)]
        # dummy AG: keeps the cc stream awake through the round-1 waves
        # so AG-s2-h0 doesn't pay the ~15us ncfw wakeup penalty
        dum_in = dram.tile([128, 4, 128], F8, name="dumin")
        dum_out = dram.tile([NL, 4, 128], F8, addr_space="Shared",
                            name="dumout")

        # ======== phase A: stream+blend, s1 chain, round 1 ============
        with tc.tile_pool(name="phaseA", bufs=1) as pA, \
             tc.tile_pool(name="psZ", bufs=1, space="PSUM") as psZ:
            s1T_bf = pA.tile([128, 2, NL], BF16)
            s1loc = pA.tile([128, 8, H1], F8)
            s1f = pA.tile([128, 8, 8, H1], F8)

            psA_ctx = ExitStack()
            psA = psA_ctx.enter_context(
                tc.tile_pool(name="psA", bufs=1, space="PSUM")
            )
            s1T_ps = [psA.tile([128, NL], F32, name=f"s1T{g}") for g in range(2)]

            # ---- batched fp8 x/W1 loads (host pre-interleaved so every
            # DMA is per-partition contiguous; x padded to 24 chunks) ----
            xbigs, w1bigs = [], []
            for b in range(3):
                eng = nc.sync if b < 2 else nc.scalar
                xb = pA.tile([128, XB, NL], F8, name="xbig", bufs=3)
                eng.dma_start(
                    out=xb, in_=xT[:, b * XB * NL : (b + 1) * XB * NL]
                )
                xbigs.append(xb)
                wb = pA.tile([128, XB, H1], F8, name="w1big", bufs=3)
                eng.dma_start(
                    out=wb, in_=W1[:, b * XB * H1 : (b + 1) * XB * H1]
                )
                w1bigs.append(wb)

            # ---- adjacency stream + blend ----
            # both pre-scaled matrices are staged (ALL on the sync ring:
            # the scalar ring must stay clear so the s1 chain is not
            # queued behind adjacency issue stalls) and summed into the
            # resident conT8 (out not aliased with inputs).  Early slabs
            # blend on Pool so its queue is clear when the collective
            # triggers need to fire; late slabs go to the vector engine.
            for j in range(NSLAB):
                r0, r1 = j * SCH * 128, (j + 1) * SCH * 128
                dst = conT8[:, j * SCH : (j + 1) * SCH, :]
                asl = pA.tile([128, SCH, NL], F8, name="asl", bufs=6)
                nc.sync.dma_start(
                    out=asl, in_=adjT_s[:, r0 * 8 : r1 * 8]
                )
                af = pA.tile([128, SCH, NL], F8, name="af", bufs=6)
                nc.sync.dma_start(
                    out=af, in_=adjT_f[:, r0 * 8 : r1 * 8]
                )
                # all on DVE: Pool's fp8 TT is ~2.5x slower and would
                # pace the staging-buffer recycle (and thus the stream)
                nc.vector.scalar_tensor_tensor(
                    out=dst, in0=af, scalar=ones_sb, in1=asl,
                    op0=MUL, op1=ADD,
                )

            # attention inputs (small; gpsimd SWDGE casts f32->bf16)
            nc.gpsimd.dma_start(out=zf_bf, in_=zfT[:, :])
            nc.gpsimd.dma_start(out=zs_bf, in_=zsT[:, :])
            nc.gpsimd.dma_start(out=wlW_sb, in_=wl_W[:, :])
            nc.gpsimd.dma_start(
                out=mlpW_sb, in_=mlp_W.rearrange("(v c) d -> c v d", v=3)
            )

            # ---- s1 = tanh(xT.T @ W1) in the transposed domain ----
            # early identity matmuls release the HAM clock gate before
            # the real s1 stream arrives; they scribble on s1T_ps, which
            # the first real accumulation (start=True) resets anyway
            for _ in range(24):
                nc.tensor.matmul(s1T_ps[0][:, 0:128], lhsT=ident_bf,
                                 rhs=ident_bf, start=True, stop=True)

            for kp2 in range(12):
                b, a = (2 * kp2) // XB, (2 * kp2) % XB
                for g in range(2):
                    for h in range(2):
                        nc.tensor.matmul(
                            s1T_ps[g][:, h * 512 : (h + 1) * 512],
                            lhsT=w1bigs[b][:, a : a + 2,
                                           g * 128 : (g + 1) * 128],
                            rhs=xbigs[b][:, a : a + 2,
                                         h * 512 : (h + 1) * 512],
                            start=(kp2 == 0),
                            stop=(kp2 == 11),
                            perf_mode=DR,
                        )

            # ---- s1 chain: tanh, transpose, stage quarter-AGs ----
            for g in range(2):
                nc.scalar.activation(s1T_bf[:, g], s1T_ps[g], AF.Tanh)
            psA_ctx.close()
            z1_ps = [psZ.tile([128, NL], F32, name=f"z1g{g}") for g in range(2)]
            with tc.tile_pool(name="psT", bufs=2, space="PSUM") as psT:
                for q in range(4):
                    for mb in (2 * q, 2 * q + 1):
                        for g in range(2):
                            tp = psT.tile([128, 128], BF16, name="tp")
                            nc.tensor.transpose(
                                tp,
                                s1T_bf[:, g, mb * 128 : (mb + 1) * 128],
                                ident_bf,
                            )
                            nc.scalar.activation(
                                s1loc[:, mb, g * 128 : (g + 1) * 128],
                                tp, AF.Copy,
                            )
                    nc.scalar.dma_start(out=s1_ins[q][:, :, :],
                                        in_=s1loc[:, 2 * q : 2 * q + 2, :])
                    nc.gpsimd.collective_compute(
                        "AllGather", mybir.AluOpType.bypass,
                        replica_groups=RG,
                        ins=[s1_ins[q].opt()], outs=[s1_outs[q].opt()],
                    )
            # stream-bridging dummy AG (runs right after the s1 quarters)
            nc.scalar.dma_start(out=dum_in, in_=conT8[:, 0:4, 0:128])
            nc.gpsimd.collective_compute(
                "AllGather", mybir.AluOpType.bypass, replica_groups=RG,
                ins=[dum_in.opt()], outs=[dum_out.opt()],
            )

            # warmers bridging the collectives-init barrier window so
            # round 1 opens at full clock
            with tc.tile_pool(name="psW1", bufs=1, space="PSUM") as psW1:
                wps = psW1.tile([128, 512], F32)
                for w in range(24):
                    nc.tensor.matmul(
                        wps, lhsT=w1bigs[0][:, 0:2, 0:128],
                        rhs=xbigs[0][:, 0:2, 0:512],
                        start=True, stop=True, perf_mode=DR,
                    )

            # ---- round 1: z1 = conT8.T-domain matmuls in 4 AG waves;
            # final wave ordered h0-first so the s2 chain starts early.
            # gathers ride the scalar ring: the sync ring's slab issues
            # stall on staging-buffer recycle until ~the stream's end.
            for q in range(4):
                nc.scalar.dma_start(
                    out=s1f[:, :, 2 * q : 2 * q + 2, :],
                    in_=s1_outs[q].rearrange("(r p) a c -> p r a c", p=128),
                )

            def r1_mm(q, r, g, h):
                sl = slice(h * 512, (h + 1) * 512)
                k = 8 * r + 2 * q
                nc.tensor.matmul(
                    z1_ps[g][:, sl],
                    lhsT=s1f[:, r, 2 * q : 2 * q + 2,
                             g * 128 : (g + 1) * 128],
                    rhs=conT8[:, k : k + 2, sl],
                    start=(q == 0 and r == 0),
                    stop=(q == 3 and r == 7),
                    perf_mode=DR,
                )

            for q in range(3):
                for r in range(8):
                    for g in range(2):
                        for h in range(2):
                            r1_mm(q, r, g, h)

            s2T_bf = pA.tile([128, NL], BF16)
            s2loc = pA.tile([128, 8, H2], F8)
            s2T_psx = ExitStack()
            psC = s2T_psx.enter_context(
                tc.tile_pool(name="psC", bufs=1, space="PSUM"))
            s2T_ps = psC.tile([128, NL], F32)

            def s2_chain(h):
                # z1sb copies ride DVE+ACT; the W2 matmuls interleave on
                # the PE behind the h1 wave
                sl = slice(h * 512, (h + 1) * 512)
                nc.vector.tensor_copy(z1sb[:, 0, sl], z1_ps[0][:, sl])
                nc.scalar.copy(z1sb[:, 1, sl], z1_ps[1][:, sl])
                for b in range(2):
                    nc.tensor.matmul(
                        s2T_ps[:, sl], lhsT=W2_sb[:, b], rhs=z1sb[:, b, sl],
                        start=(b == 0), stop=(b == 1),
                    )
                nc.scalar.activation(s2T_bf[:, sl], s2T_ps[:, sl],
                                     AF.Tanh, scale=INV13)

            def s2_stage(t):
                with tc.tile_pool(name=f"psT2{t}", bufs=2, space="PSUM") as p2:
                    for mb in range(4 * t, 4 * t + 4):
                        tp = p2.tile([128, 128], BF16, name="tp2")
                        nc.tensor.transpose(
                            tp, s2T_bf[:, mb * 128 : (mb + 1) * 128], ident_bf
                        )
                        nc.scalar.activation(s2loc[:, mb], tp, AF.Copy)
                nc.scalar.dma_start(out=s2_ins[t][:, :, :],
                                    in_=s2loc[:, 4 * t : 4 * t + 4, :])
                nc.gpsimd.collective_compute(
                    "AllGather", mybir.AluOpType.bypass,
                    replica_groups=RG,
                    ins=[s2_ins[t].opt()], outs=[s2_outs[t].opt()],
                )

            # wave 3: h0 for both g, then the h0 s2-chain, then h1
            for h in range(2):
                for g in range(2):
                    for r in range(8):
                        r1_mm(3, r, g, h)
                s2_chain(h)
                s2_stage(h)   # half t = h here: t0 = mb 0..3 needs s2T h0
            s2T_psx.close()

        # ================= merged tail: phases B + C + D ===============
        with tc.tile_pool(name="tail", bufs=1) as tl, \
             tc.tile_pool(name="psG", bufs=1, space="PSUM") as psG:
            aTs = [None, None, None]
            sqs = [None, None, None]
            embs_bf = [zf_bf, com_bf, zs_bf]

            def emit_attn_view(v, hs=(0, 1)):
                if aTs[v] is None:
                    aTs[v] = tl.tile([64, NL], F32, name=f"aT{v}")
                    sqs[v] = tl.tile([64, NL], F32, name=f"sq{v}")
                a_ps = psG.tile([64, NL], F32, name="aps", bufs=1)
                for h in hs:
                    sl = slice(h * 512, (h + 1) * 512)
                    nc.tensor.matmul(a_ps[:, sl], lhsT=wlW_sb,
                                     rhs=embs_bf[v][:, sl])
                    nc.vector.tensor_scalar_add(aTs[v][:, sl], a_ps[:, sl],
                                                wl_b_sb)
                    # square folds the wl_b bias: (a_ps + b)^2 on ACT
                    nc.scalar.activation(sqs[v][:, sl], a_ps[:, sl],
                                         AF.Square, bias=wl_b_sb)

            # com-independent attention views fill the AG-s2 gap
            emit_attn_view(0)
            emit_attn_view(2)
            sq02 = tl.tile([64, NL], F32)
            nc.gpsimd.tensor_add(sq02, sqs[0], sqs[2])

            # warmers through the AG-s2 mesh wait
            with tc.tile_pool(name="psW2", bufs=1, space="PSUM") as psW2:
                wps2 = psW2.tile([128, 512], F32)
                for w in range(14):
                    nc.tensor.matmul(
                        wps2, lhsT=conT8[:, 0:2, 0:128],
                        rhs=conT8[:, 2:4, 0:512],
                        start=True, stop=True, perf_mode=DR,
                    )

            # ---- round 2 (z2 = conT8-dom @ s2), 2 AG waves ----
            z2sb = tl.tile([128, NL], BF16)
            s3T_bf = tl.tile([64, NL], BF16)
            s3loc = tl.tile([128, 8, Z], F8)
            s2f = tl.tile([128, 8, 8, H2], F8)
            with tc.tile_pool(name="psD", bufs=1, space="PSUM") as psD, \
                 tc.tile_pool(name="psE", bufs=1, space="PSUM") as psE:
                z2_ps = psD.tile([128, NL], F32)
                s3T_ps = psE.tile([64, NL], F32)
                for t in range(2):
                    nc.scalar.dma_start(
                        out=s2f[:, :, 4 * t : 4 * t + 4, :],
                        in_=s2_outs[t].rearrange("(r p) a c -> p r a c", p=128),
                    )

                def r2_mm(t, r, a0, h):
                    sl = slice(h * 512, (h + 1) * 512)
                    k = 8 * r + a0
                    nc.tensor.matmul(
                        z2_ps[:, sl],
                        lhsT=s2f[:, r, a0 : a0 + 2, :],
                        rhs=conT8[:, k : k + 2, sl],
                        start=(t == 0 and r == 0 and a0 == 0),
                        stop=(t == 1 and r == 7 and a0 == 6),
                        perf_mode=DR,
                    )

                for r in range(8):
                    for a0 in (0, 2):
                        for h in range(2):
                            r2_mm(0, r, a0, h)

                def s3_chain(h):
                    sl = slice(h * 512, (h + 1) * 512)
                    nc.vector.tensor_copy(z2sb[:, sl], z2_ps[:, sl])
                    nc.tensor.matmul(s3T_ps[:, sl], lhsT=W3_sb,
                                     rhs=z2sb[:, sl])
                    nc.scalar.activation(s3T_bf[:, sl], s3T_ps[:, sl],
                                         AF.Copy, scale=INV13)

                def s3_stage(t):
                    with tc.tile_pool(name=f"psT3{t}", bufs=2,
                                      space="PSUM") as p3:
                        for mb in range(4 * t, 4 * t + 4):
                            tp = p3.tile([128, 64], BF16, name="tp3")
                            nc.tensor.transpose(
                                tp, s3T_bf[:, mb * 128 : (mb + 1) * 128],
                                ident_bf[:64, :64],
                            )
                            nc.scalar.activation(s3loc[:, mb], tp, AF.Copy)
                    nc.scalar.dma_start(out=s3_ins[t][:, :, :],
                                        in_=s3loc[:, 4 * t : 4 * t + 4, :])
                    nc.gpsimd.collective_compute(
                        "AllGather", mybir.AluOpType.bypass,
                        replica_groups=RG,
                        ins=[s3_ins[t].opt()], outs=[s3_outs[t].opt()],
                    )

                # wave 1: h0 first, then the h0 s3-chain, then h1
                for h in range(2):
                    for r in range(8):
                        for a0 in (4, 6):
                            r2_mm(1, r, a0, h)
                    s3_chain(h)
                    s3_stage(h)

            # warmers through the AG-s3 mesh wait
            with tc.tile_pool(name="psW3", bufs=1, space="PSUM") as psW3:
                wps3 = psW3.tile([128, 512], F32)
                for w in range(10):
                    nc.tensor.matmul(
                        wps3, lhsT=conT8[:, 0:2, 0:128],
                        rhs=conT8[:, 2:4, 0:512],
                        start=True, stop=True, perf_mode=DR,
                    )

            # ---- round 3 (com = conT8-dom @ s3) + pipelined fusion ----
            s3f = tl.tile([128, 8, 8, Z], F8)
            sq = tl.tile([64, NL], F32)
            nrm = tl.tile([64, NL], F32)
            rec = tl.tile([64, NL], F32)
            us = [tl.tile([64, NL], BF16, name=f"u{v}") for v in range(3)]
            uts = [tl.tile([64, NL], F32, name=f"ut{v}") for v in range(3)]
            outT_sb = tl.tile([64, NL], F32)

            with tc.tile_pool(name="psF", bufs=1, space="PSUM") as psF:
                com_ps = psF.tile([64, NL], F32)
                out_ps = psF.tile([64, NL], F32, name="ops")
                for t in range(2):
                    nc.scalar.dma_start(
                        out=s3f[:, :, 4 * t : 4 * t + 4, :],
                        in_=s3_outs[t].rearrange("(r p) a c -> p r a c", p=128),
                    )

                def r3_mm(t, r, a0, h):
                    sl = slice(h * 512, (h + 1) * 512)
                    k = 8 * r + a0
                    nc.tensor.matmul(
                        com_ps[:, sl],
                        lhsT=s3f[:, r, a0 : a0 + 2, :],
                        rhs=conT8[:, k : k + 2, sl],
                        start=(t == 0 and r == 0 and a0 == 0),
                        stop=(t == 1 and r == 7 and a0 == 6),
                        perf_mode=DR,
                    )

                for r in range(8):
                    for a0 in (0, 2):
                        for h in range(2):
                            r3_mm(0, r, a0, h)

                def fuse_half(h):
                    sl = slice(h * 512, (h + 1) * 512)
                    # com lands in bf16 via the ACT copy (scale folds 2^-13)
                    nc.scalar.activation(com_bf[:, sl], com_ps[:, sl],
                                         AF.Copy, scale=INV13)
                    emit_attn_view(1, hs=(h,))
                    nc.vector.tensor_add(sq[:, sl], sq02[:, sl],
                                         sqs[1][:, sl])
                    nc.scalar.activation(nrm[:, sl], sq[:, sl], AF.Sqrt)
                    nc.vector.reciprocal_approx_fast(rec[:, sl], nrm[:, sl])
                    for v, eng in ((1, nc.vector), (0, nc.gpsimd),
                                   (2, nc.gpsimd)):
                        eng.tensor_mul(uts[v][:, sl], aTs[v][:, sl],
                                       rec[:, sl])
                        eng.tensor_mul(us[v][:, sl], uts[v][:, sl],
                                       embs_bf[v][:, sl])
                    for vi, v in enumerate((1, 0, 2)):
                        nc.tensor.matmul(
                            out_ps[:, sl], lhsT=mlpW_sb[:, v],
                            rhs=us[v][:, sl],
                            start=(vi == 0), stop=(vi == 2),
                        )
                    nc.vector.tensor_scalar_add(outT_sb[:, sl],
                                                out_ps[:, sl], mlp_b_sb)
                    nc.sync.dma_start(out=outT[:, sl], in_=outT_sb[:, sl])

                # wave 1: h0 first, fuse h0 while h1 finishes
                for h in range(2):
                    for r in range(8):
                        for a0 in (4, 6):
                            r3_mm(1, r, a0, h)
                    fuse_half(h)


_CACHE = {}


def _build():
    if "nc" in _CACHE:
        return _CACHE["nc"]
    nc = bacc.Bacc("TRN2", target_bir_lowering=False, debug=False,
                   num_devices=N_CORES)
    io = {
        "adjT_f": nc.dram_tensor("adjT_f", [128, KC * NL], F8,
                                 kind="ExternalInput"),
        "adjT_s": nc.dram_tensor("adjT_s", [128, KC * NL], F8,
                                 kind="ExternalInput"),
        "xT": nc.dram_tensor("xT", [128, XC * NL], F8, kind="ExternalInput"),
        "zfT": nc.dram_tensor("zfT", [Z, NL], F32, kind="ExternalInput"),
        "zsT": nc.dram_tensor("zsT", [Z, NL], F32, kind="ExternalInput"),
        "W1": nc.dram_tensor("W1", [128, XC * H1], F8, kind="ExternalInput"),
        "W2": nc.dram_tensor("W2", [H1, H2], F32, kind="ExternalInput"),
        "W3": nc.dram_tensor("W3", [H2, Z], F32, kind="ExternalInput"),
        "wl_W": nc.dram_tensor("wl_W", [Z, Z], F32, kind="ExternalInput"),
        "mlp_W": nc.dram_tensor("mlp_W", [3 * Z, Z], F32, kind="ExternalInput"),
        "wl_b": nc.dram_tensor("wl_b", [Z], F32, kind="ExternalInput"),
        "mlp_b": nc.dram_tensor("mlp_b", [Z], F32, kind="ExternalInput"),
        "outT": nc.dram_tensor("outT", [Z, NL], F32, kind="ExternalOutput"),
    }
    with tile.TileContext(nc) as tc:
        _emit(nc, tc, io)
    nc.compile()
    _CACHE["nc"] = nc
    return nc


def _shard_inputs(inputs):
    """Full inputs -> per-core input maps (host-side sharding only)."""
    f32 = np.float32
    adj_f = np.asarray(inputs["adj_feature"], f32)
    adj_s = np.asarray(inputs["adj_spatial"], f32)
    x = np.asarray(inputs["x"], f32)
    zf = np.asarray(inputs["z_feature"], f32)
    zs = np.asarray(inputs["z_spatial"], f32)
    meta = float(np.asarray(inputs["meta"], f32).reshape(-1)[0])
    import ml_dtypes
    fp8 = ml_dtypes.float8_e4m3fn

    def interleave(mat, nch):
        """[nch*128, C] (zero-padded) -> [128, nch*C] per-partition
        contiguous chunk-major layout matching the SBUF tiles."""
        r, c = mat.shape
        pad = nch * 128 - r
        if pad:
            mat = np.concatenate(
                [mat, np.zeros((pad, c), mat.dtype)], axis=0)
        return np.ascontiguousarray(
            mat.reshape(nch, 128, c).transpose(1, 0, 2).reshape(128, nch * c))

    W1_8 = np.asarray(inputs["W1"], f32).astype(fp8)
    rep = {
        "W1": interleave(W1_8, XC),
        "W2": np.ascontiguousarray(np.asarray(inputs["W2"], f32)),
        "W3": np.ascontiguousarray(np.asarray(inputs["W3"], f32)),
        "wl_W": np.ascontiguousarray(np.asarray(inputs["wl_W"], f32)),
        "mlp_W": np.ascontiguousarray(np.asarray(inputs["mlp_W"], f32)),
        "wl_b": np.ascontiguousarray(np.asarray(inputs["wl_b"], f32)),
        "mlp_b": np.ascontiguousarray(np.asarray(inputs["mlp_b"], f32)),
    }
    # fp8 staging with a fixed 2^13 exponent shift; the meta /(1-meta)
    # blend weights fold into the per-matrix quantization scales so the
    # on-device blend is a pure add (ridden by the DMA CCE adder).
    adj_fT8 = (adj_f.T * (8192.0 * meta)).astype(fp8)
    adj_sT8 = (adj_s.T * (8192.0 * (1.0 - meta))).astype(fp8)
    xT = np.ascontiguousarray(x.T)
    zfT = np.ascontiguousarray(zf.T)
    zsT = np.ascontiguousarray(zs.T)
    in_maps = []
    for i in range(N_CORES):
        r = slice(NL * i, NL * (i + 1))
        m = {
            "adjT_f": interleave(np.ascontiguousarray(adj_fT8[:, r]), KC),
            "adjT_s": interleave(np.ascontiguousarray(adj_sT8[:, r]), KC),
            "xT": interleave(
                np.ascontiguousarray(xT[:, r]).astype(fp8), XC),
            "zfT": np.ascontiguousarray(zfT[:, r]),
            "zsT": np.ascontiguousarray(zsT[:, r]),
        }
        m.update(rep)
        in_maps.append(m)
    return in_maps


def run(trace=False, **inputs):
    nc = _build()
    in_maps = _shard_inputs(inputs)
    res = run_bass_kernel_spmd(nc, in_maps, list(range(N_CORES)), trace=trace)
    out = np.concatenate(
        [np.asarray(res.results[i]["outT"]).T for i in range(N_CORES)], axis=0
    ).astype(np.float32)
    return out, res


def kernel(**inputs):
    out, _ = run(trace=False, **inputs)
    return out


# revision 40
# speedup vs baseline: 1.1613x; 1.0096x over previous
"""Trainium2 Bass kernel for nn_CAM (GNN message passing, 8-core SPMD).

Strategy (per core i of 8, owning node rows R_i = [1024*i, 1024*(i+1))):
  - Host ships the TRANSPOSED column-block of each adjacency as
    fp8_e4m3 pre-scaled by meta*2^13 (feature) and (1-meta)*2^13
    (spatial).  The blend  con = meta*A_f + (1-meta)*A_s  then reduces
    to a pure ADD, which rides the DMA engines' inline CCE ALU:
    the spatial slab DMAs land in the resident conT8 tile and the
    feature slab DMAs accumulate into it (gpsimd SWDGE, accum_op=add).
    No vector-engine blend pass at all; the 2^-13 fold-back is a
    compile-time ACT scale.
  - All three adj@support rounds run as fp8 DoubleRow matmuls in the
    transposed [h, 1024] domain against the resident conT8.
  - Support matrices are exchanged across cores via AllGather bounced
    through shared DRAM.  s1 goes as FOUR quarter-AGs (Mesh algorithm,
    lower latency than RDH halves) consumed by four round-1 waves; s2
    and s3 as two half-AGs each.  The final wave of each round is
    ordered h0-before-h1 so the next support chain and its first AG
    trigger fire at the ~75% point of the round.
  - Discarded warmer matmuls bridge every collective wait so the PE's
    HAM clock gate stays released (cold PE runs at 1.2 GHz vs 2.4).
  - The attention fusion runs in the transposed [64, 1024] domain;
    com-independent views compute inside the AG-s2 window and the
    com-dependent tail is pipelined in two 512-column halves so the
    output DMA starts as soon as the first half of round 3 closes.
"""

import sys

if "/opt/trn_rl_repo" not in sys.path:
    sys.path.insert(0, "/opt/trn_rl_repo")

from contextlib import ExitStack

import numpy as np

import concourse.bass as bass
import concourse.tile as tile
from concourse import bacc, mybir
from concourse.bass_utils import run_bass_kernel_spmd
from concourse.masks import make_identity


F32 = mybir.dt.float32
BF16 = mybir.dt.bfloat16
F8 = mybir.dt.float8e4
AF = mybir.ActivationFunctionType
DR = mybir.MatmulPerfMode.DoubleRow
MUL = mybir.AluOpType.mult
ADD = mybir.AluOpType.add

N = 8192
D_IN = 3000
H1, H2, Z = 256, 128, 64
N_CORES = 8
NL = N // N_CORES           # 1024 local nodes per core
KC = N // 128               # 64 contraction chunks of 128
XC = (D_IN + 127) // 128    # 24 x-feature chunks (zero-padded to 3072)
XB = 8                      # x chunks per batched DMA
RG = [list(range(N_CORES))]
INV13 = 1.0 / 8192.0        # 2^-13 fold-back for the fp8 exponent shift

USE_ACCUM_BLEND = False     # blend via DMA CCE add (else DVE/Pool adds)
NSLAB = 16                  # adjacency stream slabs (KC/NSLAB chunks each)
SCH = KC // NSLAB


def _emit(nc, tc, io):
    adjT_f, adjT_s, xT = io["adjT_f"], io["adjT_s"], io["xT"]
    zfT, zsT = io["zfT"], io["zsT"]
    W1, W2, W3 = io["W1"], io["W2"], io["W3"]
    wl_W, mlp_W = io["wl_W"], io["mlp_W"]
    wl_b, mlp_b = io["wl_b"], io["mlp_b"]
    outT = io["outT"]

    ctx = ExitStack()
    with ctx:
        const = ctx.enter_context(tc.tile_pool(name="const", bufs=1))
        dram = ctx.enter_context(tc.tile_pool(name="dram", bufs=1, space="DRAM"))

        ident_bf = const.tile([128, 128], BF16)
        make_identity(nc, ident_bf)
        ones_sb = const.tile([128, 1], F32)
        nc.vector.memset(ones_sb, 1.0)
        wl_b_sb = const.tile([64, 1], F32)
        nc.gpsimd.dma_start(out=wl_b_sb, in_=wl_b[:, None])
        mlp_b_sb = const.tile([64, 1], F32)
        nc.gpsimd.dma_start(out=mlp_b_sb, in_=mlp_b[:, None])
        W2_sb = const.tile([128, 2, H2], BF16)
        nc.gpsimd.dma_start(out=W2_sb, in_=W2.rearrange("(b k) c -> k b c", b=2))
        W3_sb = const.tile([128, Z], BF16)
        nc.gpsimd.dma_start(out=W3_sb, in_=W3[:, :])

        # resident blended adjacency (transposed, fp8, x2^13)
        conT8 = const.tile([128, KC, NL], F8)
        z1sb = const.tile([128, 2, NL], BF16)

        # attention operands live across phases -> const pool
        com_bf = const.tile([64, NL], BF16)
        zf_bf = const.tile([64, NL], BF16)
        zs_bf = const.tile([64, NL], BF16)
        wlW_sb = const.tile([64, 64], BF16)
        mlpW_sb = const.tile([64, 3, 64], BF16)

        # AG bounce buffers (fp8); s1 as four quarters, s2/s3 as halves
        s1_ins = [dram.tile([128, 2, H1], F8, name=f"s1in{t}") for t in range(4)]
        s1_outs = [dram.tile([NL, 2, H1], F8, addr_space="Shared",
                             name=f"s1out{t}") for t in range(4)]
        s2_ins = [dram.tile([128, 4, H2], F8, name=f"s2in{t}") for t in range(2)]
        s2_outs = [dram.tile([NL, 4, H2], F8, addr_space="Shared",
                             name=f"s2out{t}") for t in range(2)]
        s3_ins = [dram.tile([128, 4, Z], F8, name=f"s3in{t}") for t in range(2)]
        s3_outs = [dram.tile([NL, 4, Z], F8, addr_space="Shared",
                             name=f"s3out{t}") for t in range(2)]
        # dummy AG: keeps the cc stream awake through the round-1 waves
        # so AG-s2-h0 doesn't pay the ~15us ncfw wakeup penalty
        dum_in = dram.tile([128, 4, 128], F8, name="dumin")
        dum_out = dram.tile([NL, 4, 128], F8, addr_space="Shared",
                            name="dumout")
        dum_out2 = dram.tile([NL, 4, 128], F8, addr_space="Shared",
                             name="dumout2")

        # ======== phase A: stream+blend, s1 chain, round 1 ============
        with tc.tile_pool(name="phaseA", bufs=1) as pA, \
             tc.tile_pool(name="psZ", bufs=1, space="PSUM") as psZ:
            s1T_bf = pA.tile([128, 2, NL], BF16)
            s1loc = pA.tile([128, 8, H1], F8)
            s1f = pA.tile([128, 8, 8, H1], F8)

            psA_ctx = ExitStack()
            psA = psA_ctx.enter_context(
                tc.tile_pool(name="psA", bufs=1, space="PSUM")
            )
            s1T_ps = [psA.tile([128, NL], F32, name=f"s1T{g}") for g in range(2)]

            # ---- batched fp8 x/W1 loads (host pre-interleaved so every
            # DMA is per-partition contiguous; x padded to 24 chunks) ----
            xbigs, w1bigs = [], []
            for b in range(3):
                eng = nc.sync if b < 2 else nc.scalar
                xb = pA.tile([128, XB, NL], F8, name="xbig", bufs=3)
                eng.dma_start(
                    out=xb, in_=xT[:, b * XB * NL : (b + 1) * XB * NL]
                )
                xbigs.append(xb)
                wb = pA.tile([128, XB, H1], F8, name="w1big", bufs=3)
                eng.dma_start(
                    out=wb, in_=W1[:, b * XB * H1 : (b + 1) * XB * H1]
                )
                w1bigs.append(wb)

            # ---- adjacency stream + blend ----
            # both pre-scaled matrices are staged (ALL on the sync ring:
            # the scalar ring must stay clear so the s1 chain is not
            # queued behind adjacency issue stalls) and summed into the
            # resident conT8 (out not aliased with inputs).  Early slabs
            # blend on Pool so its queue is clear when the collective
            # triggers need to fire; late slabs go to the vector engine.
            for j in range(NSLAB):
                r0, r1 = j * SCH * 128, (j + 1) * SCH * 128
                dst = conT8[:, j * SCH : (j + 1) * SCH, :]
                asl = pA.tile([128, SCH, NL], F8, name="asl", bufs=6)
                nc.sync.dma_start(
                    out=asl, in_=adjT_s[:, r0 * 8 : r1 * 8]
                )
                af = pA.tile([128, SCH, NL], F8, name="af", bufs=6)
                nc.sync.dma_start(
                    out=af, in_=adjT_f[:, r0 * 8 : r1 * 8]
                )
                # all on DVE: Pool's fp8 TT is ~2.5x slower and would
                # pace the staging-buffer recycle (and thus the stream)
                nc.vector.scalar_tensor_tensor(
                    out=dst, in0=af, scalar=ones_sb, in1=asl,
                    op0=MUL, op1=ADD,
                )

            # attention inputs (small; gpsimd SWDGE casts f32->bf16)
            nc.gpsimd.dma_start(out=zf_bf, in_=zfT[:, :])
            nc.gpsimd.dma_start(out=zs_bf, in_=zsT[:, :])
            nc.gpsimd.dma_start(out=wlW_sb, in_=wl_W[:, :])
            nc.gpsimd.dma_start(
                out=mlpW_sb, in_=mlp_W.rearrange("(v c) d -> c v d", v=3)
            )

            # ---- s1 = tanh(xT.T @ W1) in the transposed domain ----
            # early identity matmuls release the HAM clock gate before
            # the real s1 stream arrives; they scribble on s1T_ps, which
            # the first real accumulation (start=True) resets anyway
            for _ in range(24):
                nc.tensor.matmul(s1T_ps[0][:, 0:128], lhsT=ident_bf,
                                 rhs=ident_bf, start=True, stop=True)

            for kp2 in range(12):
                b, a = (2 * kp2) // XB, (2 * kp2) % XB
                for g in range(2):
                    for h in range(2):
                        nc.tensor.matmul(
                            s1T_ps[g][:, h * 512 : (h + 1) * 512],
                            lhsT=w1bigs[b][:, a : a + 2,
                                           g * 128 : (g + 1) * 128],
                            rhs=xbigs[b][:, a : a + 2,
                                         h * 512 : (h + 1) * 512],
                            start=(kp2 == 0),
                            stop=(kp2 == 11),
                            perf_mode=DR,
                        )

            # ---- s1 chain: tanh, transpose, stage quarter-AGs ----
            for g in range(2):
                nc.scalar.activation(s1T_bf[:, g], s1T_ps[g], AF.Tanh)
            psA_ctx.close()
            z1_ps = [psZ.tile([128, NL], F32, name=f"z1g{g}") for g in range(2)]
            with tc.tile_pool(name="psT", bufs=2, space="PSUM") as psT:
                for q in range(4):
                    for mb in (2 * q, 2 * q + 1):
                        for g in range(2):
                            tp = psT.tile([128, 128], BF16, name="tp")
                            nc.tensor.transpose(
                                tp,
                                s1T_bf[:, g, mb * 128 : (mb + 1) * 128],
                                ident_bf,
                            )
                            nc.scalar.activation(
                                s1loc[:, mb, g * 128 : (g + 1) * 128],
                                tp, AF.Copy,
                            )
                    nc.scalar.dma_start(out=s1_ins[q][:, :, :],
                                        in_=s1loc[:, 2 * q : 2 * q + 2, :])
                    nc.gpsimd.collective_compute(
                        "AllGather", mybir.AluOpType.bypass,
                        replica_groups=RG,
                        ins=[s1_ins[q].opt()], outs=[s1_outs[q].opt()],
                    )
            # stream-bridging dummy AG (runs right after the s1 quarters)
            nc.scalar.dma_start(out=dum_in, in_=conT8[:, 0:4, 0:128])
            nc.gpsimd.collective_compute(
                "AllGather", mybir.AluOpType.bypass, replica_groups=RG,
                ins=[dum_in.opt()], outs=[dum_out.opt()],
            )

            # warmers bridging the collectives-init barrier window so
            # round 1 opens at full clock
            with tc.tile_pool(name="psW1", bufs=1, space="PSUM") as psW1:
                wps = psW1.tile([128, 512], F32)
                for w in range(64):
                    nc.tensor.matmul(
                        wps, lhsT=w1bigs[0][:, 0:2, 0:128],
                        rhs=xbigs[0][:, 0:2, 0:512],
                        start=True, stop=True, perf_mode=DR,
                    )

            # ---- round 1: z1 = conT8.T-domain matmuls in 4 AG waves;
            # final wave ordered h0-first so the s2 chain starts early.
            # gathers ride the scalar ring: the sync ring's slab issues
            # stall on staging-buffer recycle until ~the stream's end.
            for q in range(4):
                nc.scalar.dma_start(
                    out=s1f[:, :, 2 * q : 2 * q + 2, :],
                    in_=s1_outs[q].rearrange("(r p) a c -> p r a c", p=128),
                )

            def r1_mm(q, r, g, h):
                sl = slice(h * 512, (h + 1) * 512)
                k = 8 * r + 2 * q
                nc.tensor.matmul(
                    z1_ps[g][:, sl],
                    lhsT=s1f[:, r, 2 * q : 2 * q + 2,
                             g * 128 : (g + 1) * 128],
                    rhs=conT8[:, k : k + 2, sl],
                    start=(q == 0 and r == 0),
                    stop=(q == 3 and r == 7),
                    perf_mode=DR,
                )

            for q in range(3):
                for r in range(8):
                    for g in range(2):
                        for h in range(2):
                            r1_mm(q, r, g, h)

            s2T_bf = pA.tile([128, NL], BF16)
            s2loc = pA.tile([128, 8, H2], F8)
            s2T_psx = ExitStack()
            psC = s2T_psx.enter_context(
                tc.tile_pool(name="psC", bufs=1, space="PSUM"))
            s2T_ps = psC.tile([128, NL], F32)

            def s2_chain(h):
                # z1sb copies ride DVE+ACT; the W2 matmuls interleave on
                # the PE behind the h1 wave
                sl = slice(h * 512, (h + 1) * 512)
                nc.vector.tensor_copy(z1sb[:, 0, sl], z1_ps[0][:, sl])
                nc.scalar.copy(z1sb[:, 1, sl], z1_ps[1][:, sl])
                for b in range(2):
                    nc.tensor.matmul(
                        s2T_ps[:, sl], lhsT=W2_sb[:, b], rhs=z1sb[:, b, sl],
                        start=(b == 0), stop=(b == 1),
                    )
                nc.scalar.activation(s2T_bf[:, sl], s2T_ps[:, sl],
                                     AF.Tanh, scale=INV13)

            def s2_stage(t):
                with tc.tile_pool(name=f"psT2{t}", bufs=2, space="PSUM") as p2:
                    for mb in range(4 * t, 4 * t + 4):
                        tp = p2.tile([128, 128], BF16, name="tp2")
                        nc.tensor.transpose(
                            tp, s2T_bf[:, mb * 128 : (mb + 1) * 128], ident_bf
                        )
                        nc.scalar.activation(s2loc[:, mb], tp, AF.Copy)
                nc.scalar.dma_start(out=s2_ins[t][:, :, :],
                                    in_=s2loc[:, 4 * t : 4 * t + 4, :])
                nc.gpsimd.collective_compute(
                    "AllGather", mybir.AluOpType.bypass,
                    replica_groups=RG,
                    ins=[s2_ins[t].opt()], outs=[s2_outs[t].opt()],
                )

            # wave 3: h0 for both g, then the h0 s2-chain, then h1
            for h in range(2):
                for g in range(2):
                    for r in range(8):
                        r1_mm(3, r, g, h)
                s2_chain(h)
                s2_stage(h)   # half t = h here: t0 = mb 0..3 needs s2T h0
            s2T_psx.close()
            # second stream-bridging dummy: covers the idle gap between
            # the s2 and s3 AllGathers
            nc.gpsimd.collective_compute(
                "AllGather", mybir.AluOpType.bypass, replica_groups=RG,
                ins=[dum_in.opt()], outs=[dum_out2.opt()],
            )

        # ================= merged tail: phases B + C + D ===============
        with tc.tile_pool(name="tail", bufs=1) as tl, \
             tc.tile_pool(name="psG", bufs=1, space="PSUM") as psG:
            aTs = [None, None, None]
            sqs = [None, None, None]
            embs_bf = [zf_bf, com_bf, zs_bf]

            def emit_attn_view(v, hs=(0, 1)):
                if aTs[v] is None:
                    aTs[v] = tl.tile([64, NL], F32, name=f"aT{v}")
                    sqs[v] = tl.tile([64, NL], F32, name=f"sq{v}")
                a_ps = psG.tile([64, NL], F32, name="aps", bufs=1)
                for h in hs:
                    sl = slice(h * 512, (h + 1) * 512)
                    nc.tensor.matmul(a_ps[:, sl], lhsT=wlW_sb,
                                     rhs=embs_bf[v][:, sl])
                    nc.vector.tensor_scalar_add(aTs[v][:, sl], a_ps[:, sl],
                                                wl_b_sb)
                    # square folds the wl_b bias: (a_ps + b)^2 on ACT
                    nc.scalar.activation(sqs[v][:, sl], a_ps[:, sl],
                                         AF.Square, bias=wl_b_sb)

            # com-independent attention views fill the AG-s2 gap
            emit_attn_view(0)
            emit_attn_view(2)
            sq02 = tl.tile([64, NL], F32)
            nc.gpsimd.tensor_add(sq02, sqs[0], sqs[2])

            # warmers through the AG-s2 mesh wait
            with tc.tile_pool(name="psW2", bufs=1, space="PSUM") as psW2:
                wps2 = psW2.tile([128, 512], F32)
                for w in range(40):
                    nc.tensor.matmul(
                        wps2, lhsT=conT8[:, 0:2, 0:128],
                        rhs=conT8[:, 2:4, 0:512],
                        start=True, stop=True, perf_mode=DR,
                    )

            # ---- round 2 (z2 = conT8-dom @ s2), 2 AG waves ----
            z2sb = tl.tile([128, NL], BF16)
            s3T_bf = tl.tile([64, NL], BF16)
            s3loc = tl.tile([128, 8, Z], F8)
            s2f = tl.tile([128, 8, 8, H2], F8)
            with tc.tile_pool(name="psD", bufs=1, space="PSUM") as psD, \
                 tc.tile_pool(name="psE", bufs=1, space="PSUM") as psE:
                z2_ps = psD.tile([128, NL], F32)
                s3T_ps = psE.tile([64, NL], F32)
                for t in range(2):
                    nc.scalar.dma_start(
                        out=s2f[:, :, 4 * t : 4 * t + 4, :],
                        in_=s2_outs[t].rearrange("(r p) a c -> p r a c", p=128),
                    )

                def r2_mm(t, r, a0, h):
                    sl = slice(h * 512, (h + 1) * 512)
                    k = 8 * r + a0
                    nc.tensor.matmul(
                        z2_ps[:, sl],
                        lhsT=s2f[:, r, a0 : a0 + 2, :],
                        rhs=conT8[:, k : k + 2, sl],
                        start=(t == 0 and r == 0 and a0 == 0),
                        stop=(t == 1 and r == 7 and a0 == 6),
                        perf_mode=DR,
                    )

                for r in range(8):
                    for a0 in (0, 2):
                        for h in range(2):
                            r2_mm(0, r, a0, h)

                def s3_chain(h):
                    sl = slice(h * 512, (h + 1) * 512)
                    nc.vector.tensor_copy(z2sb[:, sl], z2_ps[:, sl])
                    nc.tensor.matmul(s3T_ps[:, sl], lhsT=W3_sb,
                                     rhs=z2sb[:, sl])
                    nc.scalar.activation(s3T_bf[:, sl], s3T_ps[:, sl],
                                         AF.Copy, scale=INV13)

                def s3_stage(t):
                    with tc.tile_pool(name=f"psT3{t}", bufs=2,
                                      space="PSUM") as p3:
                        for mb in range(4 * t, 4 * t + 4):
                            tp = p3.tile([128, 64], BF16, name="tp3")
                            nc.tensor.transpose(
                                tp, s3T_bf[:, mb * 128 : (mb + 1) * 128],
                                ident_bf[:64, :64],
                            )
                            nc.scalar.activation(s3loc[:, mb], tp, AF.Copy)
                    nc.scalar.dma_start(out=s3_ins[t][:, :, :],
                                        in_=s3loc[:, 4 * t : 4 * t + 4, :])
                    nc.gpsimd.collective_compute(
                        "AllGather", mybir.AluOpType.bypass,
                        replica_groups=RG,
                        ins=[s3_ins[t].opt()], outs=[s3_outs[t].opt()],
                    )

                # wave 1: h0 first, then the h0 s3-chain, then h1
                for h in range(2):
                    for r in range(8):
                        for a0 in (4, 6):
                            r2_mm(1, r, a0, h)
                    s3_chain(h)
                    s3_stage(h)

            # warmers through the AG-s3 mesh wait
            with tc.tile_pool(name="psW3", bufs=1, space="PSUM") as psW3:
                wps3 = psW3.tile([128, 512], F32)
                for w in range(36):
                    nc.tensor.matmul(
                        wps3, lhsT=conT8[:, 0:2, 0:128],
                        rhs=conT8[:, 2:4, 0:512],
                        start=True, stop=True, perf_mode=DR,
                    )

            # ---- round 3 (com = conT8-dom @ s3) + pipelined fusion ----
            s3f = tl.tile([128, 8, 8, Z], F8)
            sq = tl.tile([64, NL], F32)
            nrm = tl.tile([64, NL], F32)
            rec = tl.tile([64, NL], F32)
            us = [tl.tile([64, NL], BF16, name=f"u{v}") for v in range(3)]
            uts = [tl.tile([64, NL], F32, name=f"ut{v}") for v in range(3)]
            outT_sb = tl.tile([64, NL], F32)

            with tc.tile_pool(name="psF", bufs=1, space="PSUM") as psF:
                com_ps = psF.tile([64, NL], F32)
                out_ps = psF.tile([64, NL], F32, name="ops")
                for t in range(2):
                    nc.scalar.dma_start(
                        out=s3f[:, :, 4 * t : 4 * t + 4, :],
                        in_=s3_outs[t].rearrange("(r p) a c -> p r a c", p=128),
                    )

                def r3_mm(t, r, a0, h):
                    sl = slice(h * 512, (h + 1) * 512)
                    k = 8 * r + a0
                    nc.tensor.matmul(
                        com_ps[:, sl],
                        lhsT=s3f[:, r, a0 : a0 + 2, :],
                        rhs=conT8[:, k : k + 2, sl],
                        start=(t == 0 and r == 0 and a0 == 0),
                        stop=(t == 1 and r == 7 and a0 == 6),
                        perf_mode=DR,
                    )

                for r in range(8):
                    for a0 in (0, 2):
                        for h in range(2):
                            r3_mm(0, r, a0, h)

                def fuse_half(h):
                    sl = slice(h * 512, (h + 1) * 512)
                    # com lands in bf16 via the ACT copy (scale folds 2^-13)
                    nc.scalar.activation(com_bf[:, sl], com_ps[:, sl],
                                         AF.Copy, scale=INV13)
                    emit_attn_view(1, hs=(h,))
                    nc.vector.tensor_add(sq[:, sl], sq02[:, sl],
                                         sqs[1][:, sl])
                    nc.scalar.activation(nrm[:, sl], sq[:, sl], AF.Sqrt)
                    nc.vector.reciprocal_approx_fast(rec[:, sl], nrm[:, sl])
                    for v, eng in ((1, nc.vector), (0, nc.gpsimd),
                                   (2, nc.gpsimd)):
                        eng.tensor_mul(uts[v][:, sl], aTs[v][:, sl],
                                       rec[:, sl])
                        eng.tensor_mul(us[v][:, sl], uts[v][:, sl],
                                       embs_bf[v][:, sl])
                    for vi, v in enumerate((1, 0, 2)):
                        nc.tensor.matmul(
                            out_ps[:, sl], lhsT=mlpW_sb[:, v],
                            rhs=us[v][:, sl],
                            start=(vi == 0), stop=(vi == 2),
                        )
                    nc.vector.tensor_scalar_add(outT_sb[:, sl],
                                                out_ps[:, sl], mlp_b_sb)
                    nc.sync.dma_start(out=outT[:, sl], in_=outT_sb[:, sl])

                # wave 1: h0 first, fuse h0 while h1 finishes
                for h in range(2):
                    for r in range(8):
                        for a0 in (4, 6):
                            r3_mm(1, r, a0, h)
                    fuse_half(h)


_CACHE = {}


def _build():
    if "nc" in _CACHE:
        return _CACHE["nc"]
    nc = bacc.Bacc("TRN2", target_bir_lowering=False, debug=False,
                   num_devices=N_CORES)
    io = {
        "adjT_f": nc.dram_tensor("adjT_f", [128, KC * NL], F8,
                                 kind="ExternalInput"),
        "adjT_s": nc.dram_tensor("adjT_s", [128, KC * NL], F8,
                                 kind="ExternalInput"),
        "xT": nc.dram_tensor("xT", [128, XC * NL], F8, kind="ExternalInput"),
        "zfT": nc.dram_tensor("zfT", [Z, NL], F32, kind="ExternalInput"),
        "zsT": nc.dram_tensor("zsT", [Z, NL], F32, kind="ExternalInput"),
        "W1": nc.dram_tensor("W1", [128, XC * H1], F8, kind="ExternalInput"),
        "W2": nc.dram_tensor("W2", [H1, H2], F32, kind="ExternalInput"),
        "W3": nc.dram_tensor("W3", [H2, Z], F32, kind="ExternalInput"),
        "wl_W": nc.dram_tensor("wl_W", [Z, Z], F32, kind="ExternalInput"),
        "mlp_W": nc.dram_tensor("mlp_W", [3 * Z, Z], F32, kind="ExternalInput"),
        "wl_b": nc.dram_tensor("wl_b", [Z], F32, kind="ExternalInput"),
        "mlp_b": nc.dram_tensor("mlp_b", [Z], F32, kind="ExternalInput"),
        "outT": nc.dram_tensor("outT", [Z, NL], F32, kind="ExternalOutput"),
    }
    with tile.TileContext(nc) as tc:
        _emit(nc, tc, io)
    nc.compile()
    _CACHE["nc"] = nc
    return nc


def _shard_inputs(inputs):
    """Full inputs -> per-core input maps (host-side sharding only)."""
    f32 = np.float32
    adj_f = np.asarray(inputs["adj_feature"], f32)
    adj_s = np.asarray(inputs["adj_spatial"], f32)
    x = np.asarray(inputs["x"], f32)
    zf = np.asarray(inputs["z_feature"], f32)
    zs = np.asarray(inputs["z_spatial"], f32)
    meta = float(np.asarray(inputs["meta"], f32).reshape(-1)[0])
    import ml_dtypes
    fp8 = ml_dtypes.float8_e4m3fn

    def interleave(mat, nch):
        """[nch*128, C] (zero-padded) -> [128, nch*C] per-partition
        contiguous chunk-major layout matching the SBUF tiles."""
        r, c = mat.shape
        pad = nch * 128 - r
        if pad:
            mat = np.concatenate(
                [mat, np.zeros((pad, c), mat.dtype)], axis=0)
        return np.ascontiguousarray(
            mat.reshape(nch, 128, c).transpose(1, 0, 2).reshape(128, nch * c))

    W1_8 = np.asarray(inputs["W1"], f32).astype(fp8)
    rep = {
        "W1": interleave(W1_8, XC),
        "W2": np.ascontiguousarray(np.asarray(inputs["W2"], f32)),
        "W3": np.ascontiguousarray(np.asarray(inputs["W3"], f32)),
        "wl_W": np.ascontiguousarray(np.asarray(inputs["wl_W"], f32)),
        "mlp_W": np.ascontiguousarray(np.asarray(inputs["mlp_W"], f32)),
        "wl_b": np.ascontiguousarray(np.asarray(inputs["wl_b"], f32)),
        "mlp_b": np.ascontiguousarray(np.asarray(inputs["mlp_b"], f32)),
    }
    # fp8 staging with a fixed 2^13 exponent shift; the meta /(1-meta)
    # blend weights fold into the per-matrix quantization scales so the
    # on-device blend is a pure add (ridden by the DMA CCE adder).
    adj_fT8 = (adj_f.T * (8192.0 * meta)).astype(fp8)
    adj_sT8 = (adj_s.T * (8192.0 * (1.0 - meta))).astype(fp8)
    xT = np.ascontiguousarray(x.T)
    zfT = np.ascontiguousarray(zf.T)
    zsT = np.ascontiguousarray(zs.T)
    in_maps = []
    for i in range(N_CORES):
        r = slice(NL * i, NL * (i + 1))
        m = {
            "adjT_f": interleave(np.ascontiguousarray(adj_fT8[:, r]), KC),
            "adjT_s": interleave(np.ascontiguousarray(adj_sT8[:, r]), KC),
            "xT": interleave(
                np.ascontiguousarray(xT[:, r]).astype(fp8), XC),
            "zfT": np.ascontiguousarray(zfT[:, r]),
            "zsT": np.ascontiguousarray(zsT[:, r]),
        }
        m.update(rep)
        in_maps.append(m)
    return in_maps


def run(trace=False, **inputs):
    nc = _build()
    in_maps = _shard_inputs(inputs)
    res = run_bass_kernel_spmd(nc, in_maps, list(range(N_CORES)), trace=trace)
    out = np.concatenate(
        [np.asarray(res.results[i]["outT"]).T for i in range(N_CORES)], axis=0
    ).astype(np.float32)
    return out, res


def kernel(**inputs):
    out, _ = run(trace=False, **inputs)
    return out


# revision 52
# speedup vs baseline: 1.2032x; 1.0361x over previous
"""Trainium2 Bass kernel for nn_CAM (GNN message passing, 8-core SPMD).

Strategy (per core i of 8, owning node rows R_i = [1024*i, 1024*(i+1))):
  - Host ships the TRANSPOSED column-block of each adjacency as
    fp8_e4m3 pre-scaled by meta*2^13 (feature) and (1-meta)*2^13
    (spatial).  The blend  con = meta*A_f + (1-meta)*A_s  then reduces
    to a pure ADD, which rides the DMA engines' inline CCE ALU:
    the spatial slab DMAs land in the resident conT8 tile and the
    feature slab DMAs accumulate into it (gpsimd SWDGE, accum_op=add).
    No vector-engine blend pass at all; the 2^-13 fold-back is a
    compile-time ACT scale.
  - All three adj@support rounds run as fp8 DoubleRow matmuls in the
    transposed [h, 1024] domain against the resident conT8.
  - Support matrices are exchanged across cores via AllGather bounced
    through shared DRAM.  s1 goes as FOUR quarter-AGs (Mesh algorithm,
    lower latency than RDH halves) consumed by four round-1 waves; s2
    and s3 as two half-AGs each.  The final wave of each round is
    ordered h0-before-h1 so the next support chain and its first AG
    trigger fire at the ~75% point of the round.
  - Discarded warmer matmuls bridge every collective wait so the PE's
    HAM clock gate stays released (cold PE runs at 1.2 GHz vs 2.4).
  - The attention fusion runs in the transposed [64, 1024] domain;
    com-independent views compute inside the AG-s2 window and the
    com-dependent tail is pipelined in two 512-column halves so the
    output DMA starts as soon as the first half of round 3 closes.
"""

import sys

if "/opt/trn_rl_repo" not in sys.path:
    sys.path.insert(0, "/opt/trn_rl_repo")

from contextlib import ExitStack

import numpy as np

import concourse.bass as bass
import concourse.tile as tile
from concourse import bacc, mybir
from concourse.bass_utils import run_bass_kernel_spmd
from concourse.masks import make_identity


F32 = mybir.dt.float32
BF16 = mybir.dt.bfloat16
F8 = mybir.dt.float8e4
AF = mybir.ActivationFunctionType
DR = mybir.MatmulPerfMode.DoubleRow
MUL = mybir.AluOpType.mult
ADD = mybir.AluOpType.add

N = 8192
D_IN = 3000
H1, H2, Z = 256, 128, 64
N_CORES = 8
NL = N // N_CORES           # 1024 local nodes per core
KC = N // 128               # 64 contraction chunks of 128
XC = (D_IN + 127) // 128    # 24 x-feature chunks (zero-padded to 3072)
XB = 8                      # x chunks per batched DMA
RG = [list(range(N_CORES))]
INV13 = 1.0 / 8192.0        # 2^-13 fold-back for the fp8 exponent shift

USE_ACCUM_BLEND = False     # blend via DMA CCE add (else DVE/Pool adds)
NSLAB = 16                  # adjacency stream slabs (KC/NSLAB chunks each)
SCH = KC // NSLAB


def _emit(nc, tc, io):
    adjT_f, adjT_s, xT = io["adjT_f"], io["adjT_s"], io["xT"]
    zfT, zsT = io["zfT"], io["zsT"]
    W1, W2, W3 = io["W1"], io["W2"], io["W3"]
    wl_W, mlp_W = io["wl_W"], io["mlp_W"]
    wl_b, mlp_b = io["wl_b"], io["mlp_b"]
    outT = io["outT"]

    ctx = ExitStack()
    with ctx:
        const = ctx.enter_context(tc.tile_pool(name="const", bufs=1))
        dram = ctx.enter_context(tc.tile_pool(name="dram", bufs=1, space="DRAM"))

        ident_bf = const.tile([128, 128], BF16)
        make_identity(nc, ident_bf)
        ones_sb = const.tile([128, 1], F32)
        nc.vector.memset(ones_sb, 1.0)
        wl_b_sb = const.tile([64, 1], F32)
        nc.gpsimd.dma_start(out=wl_b_sb, in_=wl_b[:, None])
        mlp_b_sb = const.tile([64, 1], F32)
        nc.gpsimd.dma_start(out=mlp_b_sb, in_=mlp_b[:, None])
        W2_sb = const.tile([128, 2, H2], BF16)
        nc.gpsimd.dma_start(out=W2_sb, in_=W2.rearrange("(b k) c -> k b c", b=2))
        W3_sb = const.tile([128, Z], BF16)
        nc.gpsimd.dma_start(out=W3_sb, in_=W3[:, :])

        # resident blended adjacency (transposed, fp8, x2^13)
        conT8 = const.tile([128, KC, NL], F8)
        z1sb = const.tile([128, 2, NL], BF16)

        # attention operands live across phases -> const pool
        com_bf = const.tile([64, NL], BF16)
        zf_bf = const.tile([64, NL], BF16)
        zs_bf = const.tile([64, NL], BF16)
        wlW_sb = const.tile([64, 64], BF16)
        mlpW_sb = const.tile([64, 3, 64], BF16)

        # AG bounce buffers (fp8); s1 as four quarters, s2/s3 as halves
        s1_ins = [dram.tile([128, 2, H1], F8, name=f"s1in{t}") for t in range(4)]
        s1_outs = [dram.tile([NL, 2, H1], F8, addr_space="Shared",
                             name=f"s1out{t}") for t in range(4)]
        s2_ins = [dram.tile([128, 4, H2], F8, name=f"s2in{t}") for t in range(2)]
        s2_outs = [dram.tile([NL, 4, H2], F8, addr_space="Shared",
                             name=f"s2out{t}") for t in range(2)]
        s3_ins = [dram.tile([128, 4, Z], F8, name=f"s3in{t}") for t in range(2)]
        s3_outs = [dram.tile([NL, 4, Z], F8, addr_space="Shared",
                             name=f"s3out{t}") for t in range(2)]
        # dummy AG: keeps the cc stream awake through the round-1 waves
        # so AG-s2-h0 doesn't pay the ~15us ncfw wakeup penalty
        dum_in = dram.tile([128, 4, 128], F8, name="dumin")
        dum_out = dram.tile([NL, 4, 128], F8, addr_space="Shared",
                            name="dumout")
        dum_out2 = dram.tile([NL, 4, 128], F8, addr_space="Shared",
                             name="dumout2")

        # ======== phase A: stream+blend, s1 chain, round 1 ============
        with tc.tile_pool(name="phaseA", bufs=1) as pA, \
             tc.tile_pool(name="psZ", bufs=1, space="PSUM") as psZ:
            s1T_bf = pA.tile([128, 2, NL], BF16)
            s1loc = pA.tile([128, 8, H1], F8)
            s1f = pA.tile([128, 8, 8, H1], F8)

            psA_ctx = ExitStack()
            psA = psA_ctx.enter_context(
                tc.tile_pool(name="psA", bufs=1, space="PSUM")
            )
            s1T_ps = [psA.tile([128, NL], F32, name=f"s1T{g}") for g in range(2)]

            # ---- batched fp8 x/W1 loads (host pre-interleaved so every
            # DMA is per-partition contiguous; x padded to 24 chunks) ----
            xbigs, w1bigs = [], []
            for b in range(3):
                eng = nc.sync if b < 2 else nc.scalar
                xb = pA.tile([128, XB, NL], F8, name="xbig", bufs=3)
                eng.dma_start(
                    out=xb, in_=xT[:, b * XB * NL : (b + 1) * XB * NL]
                )
                xbigs.append(xb)
                wb = pA.tile([128, XB, H1], F8, name="w1big", bufs=3)
                eng.dma_start(
                    out=wb, in_=W1[:, b * XB * H1 : (b + 1) * XB * H1]
                )
                w1bigs.append(wb)

            # ---- adjacency stream + blend ----
            # both pre-scaled matrices are staged (ALL on the sync ring:
            # the scalar ring must stay clear so the s1 chain is not
            # queued behind adjacency issue stalls) and summed into the
            # resident conT8 (out not aliased with inputs).  Early slabs
            # blend on Pool so its queue is clear when the collective
            # triggers need to fire; late slabs go to the vector engine.
            for j in range(NSLAB):
                r0, r1 = j * SCH * 128, (j + 1) * SCH * 128
                dst = conT8[:, j * SCH : (j + 1) * SCH, :]
                asl = pA.tile([128, SCH, NL], F8, name="asl", bufs=8)
                nc.sync.dma_start(
                    out=asl, in_=adjT_s[:, r0 * 8 : r1 * 8]
                )
                af = pA.tile([128, SCH, NL], F8, name="af", bufs=8)
                nc.sync.dma_start(
                    out=af, in_=adjT_f[:, r0 * 8 : r1 * 8]
                )
                # all on DVE: Pool's fp8 TT is ~2.5x slower and would
                # pace the staging-buffer recycle (and thus the stream)
                nc.vector.scalar_tensor_tensor(
                    out=dst, in0=af, scalar=ones_sb, in1=asl,
                    op0=MUL, op1=ADD,
                )

            # attention inputs (small; gpsimd SWDGE casts f32->bf16)
            nc.gpsimd.dma_start(out=zf_bf, in_=zfT[:, :])
            nc.gpsimd.dma_start(out=zs_bf, in_=zsT[:, :])
            nc.gpsimd.dma_start(out=wlW_sb, in_=wl_W[:, :])
            nc.gpsimd.dma_start(
                out=mlpW_sb, in_=mlp_W.rearrange("(v c) d -> c v d", v=3)
            )

            # ---- s1 = tanh(xT.T @ W1) in the transposed domain ----
            # early identity matmuls release the HAM clock gate before
            # the real s1 stream arrives; they scribble on s1T_ps, which
            # the first real accumulation (start=True) resets anyway
            for _ in range(24):
                nc.tensor.matmul(s1T_ps[0][:, 0:128], lhsT=ident_bf,
                                 rhs=ident_bf, start=True, stop=True)

            for kp2 in range(12):
                b, a = (2 * kp2) // XB, (2 * kp2) % XB
                for g in range(2):
                    for h in range(2):
                        nc.tensor.matmul(
                            s1T_ps[g][:, h * 512 : (h + 1) * 512],
                            lhsT=w1bigs[b][:, a : a + 2,
                                           g * 128 : (g + 1) * 128],
                            rhs=xbigs[b][:, a : a + 2,
                                         h * 512 : (h + 1) * 512],
                            start=(kp2 == 0),
                            stop=(kp2 == 11),
                            perf_mode=DR,
                        )

            # ---- s1 chain: tanh, transpose, stage quarter-AGs ----
            for g in range(2):
                nc.scalar.activation(s1T_bf[:, g], s1T_ps[g], AF.Tanh)
            psA_ctx.close()
            z1_ps = [psZ.tile([128, NL], F32, name=f"z1g{g}") for g in range(2)]
            with tc.tile_pool(name="psT", bufs=2, space="PSUM") as psT:
                for q in range(4):
                    for mb in (2 * q, 2 * q + 1):
                        for g in range(2):
                            tp = psT.tile([128, 128], BF16, name="tp")
                            nc.tensor.transpose(
                                tp,
                                s1T_bf[:, g, mb * 128 : (mb + 1) * 128],
                                ident_bf,
                            )
                            nc.scalar.activation(
                                s1loc[:, mb, g * 128 : (g + 1) * 128],
                                tp, AF.Copy,
                            )
                    nc.scalar.dma_start(out=s1_ins[q][:, :, :],
                                        in_=s1loc[:, 2 * q : 2 * q + 2, :])
                    nc.gpsimd.collective_compute(
                        "AllGather", mybir.AluOpType.bypass,
                        replica_groups=RG,
                        ins=[s1_ins[q].opt()], outs=[s1_outs[q].opt()],
                    )
            # stream-bridging dummy AG (runs right after the s1 quarters)
            nc.scalar.dma_start(out=dum_in, in_=conT8[:, 0:4, 0:128])
            nc.gpsimd.collective_compute(
                "AllGather", mybir.AluOpType.bypass, replica_groups=RG,
                ins=[dum_in.opt()], outs=[dum_out.opt()],
            )

            # warmers bridging the collectives-init barrier window so
            # round 1 opens at full clock
            with tc.tile_pool(name="psW1", bufs=1, space="PSUM") as psW1:
                wps = psW1.tile([128, 512], F32)
                for w in range(64):
                    nc.tensor.matmul(
                        wps, lhsT=w1bigs[0][:, 0:2, 0:128],
                        rhs=xbigs[0][:, 0:2, 0:512],
                        start=True, stop=True, perf_mode=DR,
                    )

            # ---- round 1: z1 = conT8.T-domain matmuls in 4 AG waves;
            # final wave ordered h0-first so the s2 chain starts early.
            # gathers ride the scalar ring: the sync ring's slab issues
            # stall on staging-buffer recycle until ~the stream's end.
            for q in range(4):
                nc.scalar.dma_start(
                    out=s1f[:, :, 2 * q : 2 * q + 2, :],
                    in_=s1_outs[q].rearrange("(r p) a c -> p r a c", p=128),
                )

            def r1_mm(q, r, g, h):
                sl = slice(h * 512, (h + 1) * 512)
                k = 8 * r + 2 * q
                nc.tensor.matmul(
                    z1_ps[g][:, sl],
                    lhsT=s1f[:, r, 2 * q : 2 * q + 2,
                             g * 128 : (g + 1) * 128],
                    rhs=conT8[:, k : k + 2, sl],
                    start=(q == 0 and r == 0),
                    stop=(q == 3 and r == 7),
                    perf_mode=DR,
                )

            # inter-wave warmer bursts keep the HAM gate released
            # through the ~3.5us gather gaps between AG waves
            psWr_ctx = ExitStack()
            psWr = psWr_ctx.enter_context(
                tc.tile_pool(name="psWr", bufs=1, space="PSUM"))
            wrps = psWr.tile([128, 512], F32)

            def warm(n):
                for _ in range(n):
                    nc.tensor.matmul(
                        wrps, lhsT=conT8[:, 0:2, 0:128],
                        rhs=conT8[:, 2:4, 0:512],
                        start=True, stop=True, perf_mode=DR,
                    )

            for q in range(3):
                for r in range(8):
                    for g in range(2):
                        for h in range(2):
                            r1_mm(q, r, g, h)
                warm(8)
            psWr_ctx.close()

            s2T_bf = pA.tile([128, NL], BF16)
            s2loc = pA.tile([128, 8, H2], F8)
            s2T_psx = ExitStack()
            psC = s2T_psx.enter_context(
                tc.tile_pool(name="psC", bufs=1, space="PSUM"))
            s2T_ps = psC.tile([128, NL], F32)

            def s2_chain(h):
                # z1sb copies ride DVE+ACT; the W2 matmuls interleave on
                # the PE behind the h1 wave
                sl = slice(h * 512, (h + 1) * 512)
                nc.vector.tensor_copy(z1sb[:, 0, sl], z1_ps[0][:, sl])
                nc.scalar.copy(z1sb[:, 1, sl], z1_ps[1][:, sl])
                for b in range(2):
                    nc.tensor.matmul(
                        s2T_ps[:, sl], lhsT=W2_sb[:, b], rhs=z1sb[:, b, sl],
                        start=(b == 0), stop=(b == 1),
                    )
                nc.scalar.activation(s2T_bf[:, sl], s2T_ps[:, sl],
                                     AF.Tanh, scale=INV13)

            def s2_stage(t):
                with tc.tile_pool(name=f"psT2{t}", bufs=2, space="PSUM") as p2:
                    for mb in range(4 * t, 4 * t + 4):
                        tp = p2.tile([128, 128], BF16, name="tp2")
                        nc.tensor.transpose(
                            tp, s2T_bf[:, mb * 128 : (mb + 1) * 128], ident_bf
                        )
                        nc.scalar.activation(s2loc[:, mb], tp, AF.Copy)
                nc.scalar.dma_start(out=s2_ins[t][:, :, :],
                                    in_=s2loc[:, 4 * t : 4 * t + 4, :])
                nc.gpsimd.collective_compute(
                    "AllGather", mybir.AluOpType.bypass,
                    replica_groups=RG,
                    ins=[s2_ins[t].opt()], outs=[s2_outs[t].opt()],
                )

            # wave 3: h0 for both g, then the h0 s2-chain, then h1
            for h in range(2):
                for g in range(2):
                    for r in range(8):
                        r1_mm(3, r, g, h)
                s2_chain(h)
                s2_stage(h)   # half t = h here: t0 = mb 0..3 needs s2T h0
            s2T_psx.close()
            # second stream-bridging dummy: covers the idle gap between
            # the s2 and s3 AllGathers
            nc.gpsimd.collective_compute(
                "AllGather", mybir.AluOpType.bypass, replica_groups=RG,
                ins=[dum_in.opt()], outs=[dum_out2.opt()],
            )

        # ================= merged tail: phases B + C + D ===============
        with tc.tile_pool(name="tail", bufs=1) as tl, \
             tc.tile_pool(name="psG", bufs=1, space="PSUM") as psG:
            aTs = [None, None, None]
            sqs = [None, None, None]
            embs_bf = [zf_bf, com_bf, zs_bf]

            def emit_attn_view(v, hs=(0, 1)):
                if aTs[v] is None:
                    aTs[v] = tl.tile([64, NL], F32, name=f"aT{v}")
                    sqs[v] = tl.tile([64, NL], F32, name=f"sq{v}")
                a_ps = psG.tile([64, NL], F32, name="aps", bufs=1)
                for h in hs:
                    sl = slice(h * 512, (h + 1) * 512)
                    nc.tensor.matmul(a_ps[:, sl], lhsT=wlW_sb,
                                     rhs=embs_bf[v][:, sl])
                    nc.vector.tensor_scalar_add(aTs[v][:, sl], a_ps[:, sl],
                                                wl_b_sb)
                    # square folds the wl_b bias: (a_ps + b)^2 on ACT
                    nc.scalar.activation(sqs[v][:, sl], a_ps[:, sl],
                                         AF.Square, bias=wl_b_sb)

            # com-independent attention views fill the AG-s2 gap
            emit_attn_view(0)
            emit_attn_view(2)
            sq02 = tl.tile([64, NL], F32)
            nc.gpsimd.tensor_add(sq02, sqs[0], sqs[2])

            # tail-wide warmer pool; bursts bridge every AG wait
            psWt_ctx = ExitStack()
            psWt = psWt_ctx.enter_context(
                tc.tile_pool(name="psWt", bufs=1, space="PSUM"))
            wtps = psWt.tile([128, 512], F32)

            def warm2(n):
                for _ in range(n):
                    nc.tensor.matmul(
                        wtps, lhsT=conT8[:, 0:2, 0:128],
                        rhs=conT8[:, 2:4, 0:512],
                        start=True, stop=True, perf_mode=DR,
                    )

            warm2(40)

            # ---- round 2 (z2 = conT8-dom @ s2), 2 AG waves ----
            z2sb = tl.tile([128, NL], BF16)
            s3T_bf = tl.tile([64, NL], BF16)
            s3loc = tl.tile([128, 8, Z], F8)
            s2f = tl.tile([128, 8, 8, H2], F8)
            with tc.tile_pool(name="psD", bufs=1, space="PSUM") as psD, \
                 tc.tile_pool(name="psE", bufs=1, space="PSUM") as psE:
                z2_ps = psD.tile([128, NL], F32)
                s3T_ps = psE.tile([64, NL], F32)
                for t in range(2):
                    nc.scalar.dma_start(
                        out=s2f[:, :, 4 * t : 4 * t + 4, :],
                        in_=s2_outs[t].rearrange("(r p) a c -> p r a c", p=128),
                    )

                def r2_mm(t, r, a0, h):
                    sl = slice(h * 512, (h + 1) * 512)
                    k = 8 * r + a0
                    nc.tensor.matmul(
                        z2_ps[:, sl],
                        lhsT=s2f[:, r, a0 : a0 + 2, :],
                        rhs=conT8[:, k : k + 2, sl],
                        start=(t == 0 and r == 0 and a0 == 0),
                        stop=(t == 1 and r == 7 and a0 == 6),
                        perf_mode=DR,
                    )

                for r in range(8):
                    for a0 in (0, 2):
                        for h in range(2):
                            r2_mm(0, r, a0, h)
                warm2(8)

                def s3_chain(h):
                    sl = slice(h * 512, (h + 1) * 512)
                    nc.vector.tensor_copy(z2sb[:, sl], z2_ps[:, sl])
                    nc.tensor.matmul(s3T_ps[:, sl], lhsT=W3_sb,
                                     rhs=z2sb[:, sl])
                    nc.scalar.activation(s3T_bf[:, sl], s3T_ps[:, sl],
                                         AF.Copy, scale=INV13)

                def s3_stage(t):
                    with tc.tile_pool(name=f"psT3{t}", bufs=1,
                                      space="PSUM") as p3:
                        for mb in range(4 * t, 4 * t + 4):
                            tp = p3.tile([128, 64], BF16, name="tp3")
                            nc.tensor.transpose(
                                tp, s3T_bf[:, mb * 128 : (mb + 1) * 128],
                                ident_bf[:64, :64],
                            )
                            nc.scalar.activation(s3loc[:, mb], tp, AF.Copy)
                    nc.scalar.dma_start(out=s3_ins[t][:, :, :],
                                        in_=s3loc[:, 4 * t : 4 * t + 4, :])
                    nc.gpsimd.collective_compute(
                        "AllGather", mybir.AluOpType.bypass,
                        replica_groups=RG,
                        ins=[s3_ins[t].opt()], outs=[s3_outs[t].opt()],
                    )

                # wave 1: h0 first, then the h0 s3-chain, then h1
                for h in range(2):
                    for r in range(8):
                        for a0 in (4, 6):
                            r2_mm(1, r, a0, h)
                    s3_chain(h)
                    s3_stage(h)

            # warmers through the AG-s3 mesh wait
            warm2(36)

            # ---- round 3 (com = conT8-dom @ s3) + pipelined fusion ----
            s3f = tl.tile([128, 8, 8, Z], F8)
            sq = tl.tile([64, NL], F32)
            nrm = tl.tile([64, NL], F32)
            rec = tl.tile([64, NL], F32)
            us = [tl.tile([64, NL], BF16, name=f"u{v}") for v in range(3)]
            uts = [tl.tile([64, NL], F32, name=f"ut{v}") for v in range(3)]
            outT_sb = tl.tile([64, NL], F32)

            with tc.tile_pool(name="psF", bufs=1, space="PSUM") as psF:
                com_ps = psF.tile([64, NL], F32)
                out_ps = psF.tile([64, NL], F32, name="ops")
                for t in range(2):
                    nc.scalar.dma_start(
                        out=s3f[:, :, 4 * t : 4 * t + 4, :],
                        in_=s3_outs[t].rearrange("(r p) a c -> p r a c", p=128),
                    )

                def r3_mm(t, r, a0, h):
                    sl = slice(h * 512, (h + 1) * 512)
                    k = 8 * r + a0
                    nc.tensor.matmul(
                        com_ps[:, sl],
                        lhsT=s3f[:, r, a0 : a0 + 2, :],
                        rhs=conT8[:, k : k + 2, sl],
                        start=(t == 0 and r == 0 and a0 == 0),
                        stop=(t == 1 and r == 7 and a0 == 6),
                        perf_mode=DR,
                    )

                for r in range(8):
                    for a0 in (0, 2):
                        for h in range(2):
                            r3_mm(0, r, a0, h)
                warm2(8)

                def fuse_pre(h):
                    # quick PE/ACT/DVE work; no PE wait on the DVE chain
                    sl = slice(h * 512, (h + 1) * 512)
                    # com lands in bf16 via the ACT copy (scale folds 2^-13)
                    nc.scalar.activation(com_bf[:, sl], com_ps[:, sl],
                                         AF.Copy, scale=INV13)
                    emit_attn_view(1, hs=(h,))
                    nc.vector.tensor_add(sq[:, sl], sq02[:, sl],
                                         sqs[1][:, sl])
                    nc.scalar.activation(nrm[:, sl], sq[:, sl], AF.Sqrt)
                    nc.vector.reciprocal_approx_fast(rec[:, sl], nrm[:, sl])
                    for v, eng in ((1, nc.vector), (0, nc.gpsimd),
                                   (2, nc.gpsimd)):
                        eng.tensor_mul(uts[v][:, sl], aTs[v][:, sl],
                                       rec[:, sl])
                        eng.tensor_mul(us[v][:, sl], uts[v][:, sl],
                                       embs_bf[v][:, sl])

                def fuse_post(h):
                    # PE out-matmuls, deferred so they never stall the
                    # round-3 h1 wave behind the h0 DVE chain
                    sl = slice(h * 512, (h + 1) * 512)
                    for vi, v in enumerate((1, 0, 2)):
                        nc.tensor.matmul(
                            out_ps[:, sl], lhsT=mlpW_sb[:, v],
                            rhs=us[v][:, sl],
                            start=(vi == 0), stop=(vi == 2),
                        )
                    nc.vector.tensor_scalar_add(outT_sb[:, sl],
                                                out_ps[:, sl], mlp_b_sb)
                    nc.sync.dma_start(out=outT[:, sl], in_=outT_sb[:, sl])

                # wave 1: h0 first; h0's output matmuls run after the
                # h1 wave so its DVE chain hides under the h1 matmuls
                for r in range(8):
                    for a0 in (4, 6):
                        r3_mm(1, r, a0, 0)
                fuse_pre(0)
                for r in range(8):
                    for a0 in (4, 6):
                        r3_mm(1, r, a0, 1)
                fuse_post(0)
                fuse_pre(1)
                fuse_post(1)
            psWt_ctx.close()


_CACHE = {}


def _build():
    if "nc" in _CACHE:
        return _CACHE["nc"]
    nc = bacc.Bacc("TRN2", target_bir_lowering=False, debug=False,
                   num_devices=N_CORES)
    io = {
        "adjT_f": nc.dram_tensor("adjT_f", [128, KC * NL], F8,
                                 kind="ExternalInput"),
        "adjT_s": nc.dram_tensor("adjT_s", [128, KC * NL], F8,
                                 kind="ExternalInput"),
        "xT": nc.dram_tensor("xT", [128, XC * NL], F8, kind="ExternalInput"),
        "zfT": nc.dram_tensor("zfT", [Z, NL], F32, kind="ExternalInput"),
        "zsT": nc.dram_tensor("zsT", [Z, NL], F32, kind="ExternalInput"),
        "W1": nc.dram_tensor("W1", [128, XC * H1], F8, kind="ExternalInput"),
        "W2": nc.dram_tensor("W2", [H1, H2], F32, kind="ExternalInput"),
        "W3": nc.dram_tensor("W3", [H2, Z], F32, kind="ExternalInput"),
        "wl_W": nc.dram_tensor("wl_W", [Z, Z], F32, kind="ExternalInput"),
        "mlp_W": nc.dram_tensor("mlp_W", [3 * Z, Z], F32, kind="ExternalInput"),
        "wl_b": nc.dram_tensor("wl_b", [Z], F32, kind="ExternalInput"),
        "mlp_b": nc.dram_tensor("mlp_b", [Z], F32, kind="ExternalInput"),
        "outT": nc.dram_tensor("outT", [Z, NL], F32, kind="ExternalOutput"),
    }
    with tile.TileContext(nc) as tc:
        _emit(nc, tc, io)
    nc.compile()
    _CACHE["nc"] = nc
    return nc


def _shard_inputs(inputs):
    """Full inputs -> per-core input maps (host-side sharding only)."""
    f32 = np.float32
    adj_f = np.asarray(inputs["adj_feature"], f32)
    adj_s = np.asarray(inputs["adj_spatial"], f32)
    x = np.asarray(inputs["x"], f32)
    zf = np.asarray(inputs["z_feature"], f32)
    zs = np.asarray(inputs["z_spatial"], f32)
    meta = float(np.asarray(inputs["meta"], f32).reshape(-1)[0])
    import ml_dtypes
    fp8 = ml_dtypes.float8_e4m3fn

    def interleave(mat, nch):
        """[nch*128, C] (zero-padded) -> [128, nch*C] per-partition
        contiguous chunk-major layout matching the SBUF tiles."""
        r, c = mat.shape
        pad = nch * 128 - r
        if pad:
            mat = np.concatenate(
                [mat, np.zeros((pad, c), mat.dtype)], axis=0)
        return np.ascontiguousarray(
            mat.reshape(nch, 128, c).transpose(1, 0, 2).reshape(128, nch * c))

    W1_8 = np.asarray(inputs["W1"], f32).astype(fp8)
    rep = {
        "W1": interleave(W1_8, XC),
        "W2": np.ascontiguousarray(np.asarray(inputs["W2"], f32)),
        "W3": np.ascontiguousarray(np.asarray(inputs["W3"], f32)),
        "wl_W": np.ascontiguousarray(np.asarray(inputs["wl_W"], f32)),
        "mlp_W": np.ascontiguousarray(np.asarray(inputs["mlp_W"], f32)),
        "wl_b": np.ascontiguousarray(np.asarray(inputs["wl_b"], f32)),
        "mlp_b": np.ascontiguousarray(np.asarray(inputs["mlp_b"], f32)),
    }
    # fp8 staging with a fixed 2^13 exponent shift; the meta /(1-meta)
    # blend weights fold into the per-matrix quantization scales so the
    # on-device blend is a pure add (ridden by the DMA CCE adder).
    adj_fT8 = (adj_f.T * (8192.0 * meta)).astype(fp8)
    adj_sT8 = (adj_s.T * (8192.0 * (1.0 - meta))).astype(fp8)
    xT = np.ascontiguousarray(x.T)
    zfT = np.ascontiguousarray(zf.T)
    zsT = np.ascontiguousarray(zs.T)
    in_maps = []
    for i in range(N_CORES):
        r = slice(NL * i, NL * (i + 1))
        m = {
            "adjT_f": interleave(np.ascontiguousarray(adj_fT8[:, r]), KC),
            "adjT_s": interleave(np.ascontiguousarray(adj_sT8[:, r]), KC),
            "xT": interleave(
                np.ascontiguousarray(xT[:, r]).astype(fp8), XC),
            "zfT": np.ascontiguousarray(zfT[:, r]),
            "zsT": np.ascontiguousarray(zsT[:, r]),
        }
        m.update(rep)
        in_maps.append(m)
    return in_maps


def run(trace=False, **inputs):
    nc = _build()
    in_maps = _shard_inputs(inputs)
    res = run_bass_kernel_spmd(nc, in_maps, list(range(N_CORES)), trace=trace)
    out = np.concatenate(
        [np.asarray(res.results[i]["outT"]).T for i in range(N_CORES)], axis=0
    ).astype(np.float32)
    return out, res


def kernel(**inputs):
    out, _ = run(trace=False, **inputs)
    return out
